# revision 1
# baseline (speedup 1.0000x reference)
"""Trainium2 Bass kernel for KeOps multi-head latent attention.

Reference computation (B=2, N=2048, DIM=1024, LATENT=512, HEADS=16, HD=64):
    q = x @ wq * scale
    k = relu((x @ wkv[:, :D]) @ lk1) @ lk2      (folded: relu(x @ W1k) @ lk2)
    v = relu((x @ wkv[:, D:]) @ lv1) @ lv2      (folded: relu(x @ W1v) @ lv2)
    per head: e = exp(q k^T + maskbias); out = (e @ v) / (e.sum + 1e-6)
    y = out @ wout + bout

Strategy (8 cores, one SPMD NEFF):
  - tokens sharded 512/core (cores 0-3 batch0, 4-7 batch1)
  - masked keys compacted on host; each core computes k/v for P assigned
    active-key slots; one grouped AllGather [[0-3],[4-7]] shares k/v in FP8
    (e4m3).  A constant softmax shift C (folded into the per-key ACT bias,
    including the reference's +1e-6 eps key) keeps exp() inside fp8 range
    while leaving the numer/denom ratio bit-exact vs the unshifted formula.
  - attention: local 512 queries x all gathered keys, keys on partitions
    so the mask bias is a per-partition ACT bias and the denominator is a
    free ones-column in the v matmul; q/k/v/e operands fp8, fp32 PSUM
    accumulation; softmax reciprocal broadcast runs on the (otherwise
    idle) GPSIMD engine; numer/denom evacuated to SBUF immediately to
    free the PSUM banks for the next head-pair
  - weights and x in bf16; coarse rearranged-AP DMA loads (few large
    descriptors) and careful SP/ACT queue placement keep the DMA issue
    rate off the critical path
"""

import sys

sys.path.insert(0, "/opt/trn_rl_repo")
import numpy as np
import ml_dtypes
import concourse.bass as bass
import concourse.mybir as mybir
import concourse.tile as tile
from concourse import bacc
from concourse.bass_utils import run_bass_kernel_spmd

DIM, LATENT, HEADS, HD = 1024, 512, 16, 64
B, N, NC, T = 2, 2048, 8, 512
SCALE = HD ** -0.5
BF16, F32, FP8 = mybir.dt.bfloat16, mybir.dt.float32, mybir.dt.float8e4
NPBF16 = ml_dtypes.bfloat16
NEG = -10000.0

_cache: dict = {}
LAST_RESULTS = None


def _pieces(lo, hi, P):
    """Split global row range [lo,hi) into per-chunk pieces (chunk, clo, n, dst)."""
    out, r = [], lo
    while r < hi:
        c = r // P
        e = min(hi, (c + 1) * P)
        out.append((c, r - c * P, e - r, r - lo))
        r = e
    return out


def _build(NB):
    """NB = gathered key blocks of 128 per batch; P = NB*32 slots per producer."""
    P = NB * 32
    TB = (P + 127) // 128
    LK, LV = DIM * P, P * 1040
    LTOT = LK + LV

    nc = bacc.Bacc("TRN2", target_bir_lowering=False, num_devices=NC)
    xq_d = nc.dram_tensor("xq", [DIM, T], BF16, kind="ExternalInput")
    xkv_d = nc.dram_tensor("xkv", [DIM, P], BF16, kind="ExternalInput")
    wq_d = nc.dram_tensor("wq", [DIM, DIM], BF16, kind="ExternalInput")
    w1k_d = nc.dram_tensor("w1k", [DIM, LATENT], BF16, kind="ExternalInput")
    lk2_d = nc.dram_tensor("lk2", [LATENT, DIM], BF16, kind="ExternalInput")
    w1v_d = nc.dram_tensor("w1v", [DIM, LATENT], BF16, kind="ExternalInput")
    lv2_d = nc.dram_tensor("lv2", [LATENT, DIM], BF16, kind="ExternalInput")
    wout_d = nc.dram_tensor("wout", [DIM, DIM], BF16, kind="ExternalInput")
    bout_d = nc.dram_tensor("bout", [128, 8], F32, kind="ExternalInput")
    kb_d = nc.dram_tensor("kb", [128, NB], F32, kind="ExternalInput")
    y_d = nc.dram_tensor("yT", [DIM, T], F32, kind="ExternalOutput")

    from contextlib import ExitStack
    with ExitStack() as ctx:
        tc = ctx.enter_context(tile.TileContext(nc))
        pool = lambda **kw: ctx.enter_context(tc.tile_pool(**kw))
        pw = pool(name="pw", bufs=4)            # big weight tiles (transient)
        pwbig = pool(name="pwbig", bufs=2)      # wq/wout
        px = pool(name="px", bufs=2)            # xq/xkv
        ph = pool(name="ph", bufs=8)            # hk/hv latent activations
        pst = pool(name="pst", bufs=3)          # cin staging (kT, v)
        pqt = pool(name="pqt", bufs=8)          # q tiles (persist)
        pkt = pool(name="pkt", bufs=1)          # gathered kT per head-pair
        pvg = pool(name="pvg", bufs=1)          # gathered v (one big tile)
        pkb = pool(name="pkb", bufs=1)
        pe_ = pool(name="pe", bufs=4)           # exp tiles
        patt = pool(name="patt", bufs=8)        # attention outputs (persist)
        pattB = pool(name="pattB", bufs=1)
        pnmc = pool(name="pnmc", bufs=4)        # numer psum evacuation
        pd = pool(name="pd", bufs=2)            # reciprocal rows
        pbb = pool(name="pbb", bufs=2)          # broadcast reciprocal
        pbo = pool(name="pbo", bufs=1)
        posb = pool(name="posb", bufs=3)
        ps1 = pool(name="ps1", bufs=2, space="PSUM")
        pssc = pool(name="pssc", bufs=2, space="PSUM")
        psnm = pool(name="psnm", bufs=2, space="PSUM")
        dram = pool(name="dram", bufs=1, space="DRAM")
        if True:
            # ---------------- phase 1: kv path (feeds the collective) --------
            xkv_sb = px.tile([128, 8 * P], BF16, tag="xkv")
            w1k_sb = pw.tile([128, 8 * LATENT], BF16, tag="w1")
            w1v_sb = pw.tile([128, 8 * LATENT], BF16, tag="w1")
            lk2_sb = pw.tile([128, 4 * DIM], BF16, tag="w2")
            lv2_sb = pw.tile([128, 4 * DIM], BF16, tag="w2")

            for h2 in range(2):
                dd = slice(512 * h2, 512 * (h2 + 1))
                nc.sync.dma_start(
                    xkv_sb[:, 4 * P * h2:4 * P * (h2 + 1)]
                    .rearrange("p (d n) -> p d n", d=4),
                    xkv_d.ap()[dd, :].rearrange("(d p) n -> p d n", p=128))
                nc.sync.dma_start(
                    w1k_sb[:, 4 * LATENT * h2:4 * LATENT * (h2 + 1)]
                    .rearrange("p (d l) -> p d l", d=4),
                    w1k_d.ap()[dd, :].rearrange("(d p) l -> p d l", p=128))
                nc.sync.dma_start(
                    w1v_sb[:, 4 * LATENT * h2:4 * LATENT * (h2 + 1)]
                    .rearrange("p (d l) -> p d l", d=4),
                    w1v_d.ap()[dd, :].rearrange("(d p) l -> p d l", p=128))
            nc.sync.dma_start(
                lk2_sb[:].rearrange("p (l c) -> p l c", l=4),
                lk2_d.ap().rearrange("(l p) c -> p l c", p=128))
            nc.sync.dma_start(
                lv2_sb[:].rearrange("p (l c) -> p l c", l=4),
                lv2_d.ap().rearrange("(l p) c -> p l c", p=128))
            kbt = pkb.tile([128, NB], F32, tag="kb")
            nc.sync.dma_start(kbt[:], kb_d.ap())

            hk, hv = [], []
            for w_sb, dst in ((w1k_sb, hk), (w1v_sb, hv)):
                for l in range(4):
                    ps = ps1.tile([128, P], F32, tag="p1")
                    for d in range(8):
                        nc.tensor.matmul(
                            ps[:], w_sb[:, d * LATENT + 128 * l:d * LATENT + 128 * (l + 1)],
                            xkv_sb[:, d * P:(d + 1) * P],
                            start=(d == 0), stop=(d == 7))
                    h = ph.tile([128, P], BF16, tag="h")
                    nc.scalar.activation(h[:], ps[:], mybir.ActivationFunctionType.Relu)
                    dst.append(h)

            cin = dram.tile([LTOT], FP8)
            cout = dram.tile([4 * LTOT], FP8)

            # kT payload [DIM, P] feature-major, staged then one DMA
            kstage = px.tile([128, 8 * P], FP8, tag="kst")
            for cb in range(8):
                ps = ps1.tile([128, P], F32, tag="p1")
                for l in range(4):
                    nc.tensor.matmul(
                        ps[:], lk2_sb[:, l * DIM + 128 * cb:l * DIM + 128 * (cb + 1)],
                        hk[l][:], start=(l == 0), stop=(l == 3))
                with nc.allow_low_precision(reason="bf16 payload"):
                    nc.vector.tensor_copy(kstage[:, cb * P:(cb + 1) * P], ps[:])
            nc.sync.dma_start(
                cin[0:LK].rearrange("(i p s) -> p i s", i=8, p=128),
                kstage[:].rearrange("p (i s) -> p i s", i=8))

            # v payload [P, 1040] token-major, head-interleaved with ones cols
            for tb in range(TB):
                m = min(128, P - 128 * tb)
                vsb = pst.tile([128, 1040], FP8, tag="vsb")
                nc.gpsimd.memset(vsb[:m, :], 1.0)
                for ch in range(2):
                    ps = ps1.tile([128, 512], F32, tag="p1")
                    for l in range(4):
                        nc.tensor.matmul(
                            ps[:m, :], hv[l][:, 128 * tb:128 * tb + m],
                            lv2_sb[:, l * DIM + 512 * ch:l * DIM + 512 * (ch + 1)],
                            start=(l == 0), stop=(l == 3))
                    dst = vsb[0:m, 520 * ch:520 * (ch + 1)] \
                        .rearrange("p (g c) -> p g c", c=65)[:, :, 0:64]
                    src = ps[0:m, :].rearrange("p (g c) -> p g c", c=64)
                    with nc.allow_low_precision(reason="bf16 payload"):
                        nc.vector.tensor_copy(dst, src)
                nc.sync.dma_start(
                    cin[LK + 128 * tb * 1040: LK + (128 * tb + m) * 1040]
                    .rearrange("(p f) -> p f", p=m),
                    vsb[:m, :])

            xq_a = px.tile([128, 4 * T], BF16, tag="xq")
            xq_b = px.tile([128, 4 * T], BF16, tag="xq")
            nc.sync.dma_start(
                xq_a[:].rearrange("p (d n) -> p d n", d=4),
                xq_d.ap()[0:512, :].rearrange("(d p) n -> p d n", p=128))
            nc.sync.dma_start(
                xq_b[:].rearrange("p (d n) -> p d n", d=4),
                xq_d.ap()[512:1024, :].rearrange("(d p) n -> p d n", p=128))
            wq_a = pwbig.tile([128, 4 * DIM], BF16, tag="wq")
            wq_b = pwbig.tile([128, 4 * DIM], BF16, tag="wq")
            nc.sync.dma_start(
                wq_a[:].rearrange("p (d c) -> p d c", d=4),
                wq_d.ap()[0:512, :].rearrange("(d p) c -> p d c", p=128))
            nc.sync.dma_start(
                wq_b[:].rearrange("p (d c) -> p d c", d=4),
                wq_d.ap()[512:1024, :].rearrange("(d p) c -> p d c", p=128))

            nc.gpsimd.collective_compute(
                "AllGather", mybir.AluOpType.bypass,
                replica_groups=[[0, 1, 2, 3], [4, 5, 6, 7]],
                ins=[cin.opt()], outs=[cout.opt()],
            )

            # ---------------- q path (overlaps the collective) ---------------
            qt = []
            for cb in range(8):
                ps = ps1.tile([128, T], F32, tag="p1")
                for d in range(8):
                    xq_h = xq_a if d < 4 else xq_b
                    wq_h = wq_a if d < 4 else wq_b
                    nc.tensor.matmul(
                        ps[:], wq_h[:, (d % 4) * DIM + 128 * cb:(d % 4) * DIM + 128 * (cb + 1)],
                        xq_h[:, (d % 4) * T:(d % 4 + 1) * T],
                        start=(d == 0), stop=(d == 7))
                q = pqt.tile([128, T], FP8, tag="qt")
                with nc.allow_low_precision(reason="bf16 q"):
                    nc.vector.tensor_copy(q[:], ps[:])
                qt.append(q)

            # ---------------- gathered kv loads ------------------------------
            cout2 = cout[:].rearrange("(t x) -> t x", t=4)
            kt_all = pkt.tile([128, 8 * NB * 128], FP8, tag="ktg")
            for t4 in range(4):
                nc.sync.dma_start(
                    kt_all[:].rearrange("p (i t s) -> p t i s", i=8, t=4)[:, t4],
                    cout2[t4:t4 + 1, 0:LK]
                    .rearrange("t (i p s) -> p (t i) s", i=8, p=128))
            kt = [kt_all[:, i * NB * 128:(i + 1) * NB * 128] for i in range(8)]
            vg = pvg.tile([128, NB * 1040], FP8, tag="vg")
            for j in range(NB):
                for (c, clo, n, dst) in _pieces(128 * j, 128 * (j + 1), P):
                    nc.sync.dma_start(
                        vg[dst:dst + n, 1040 * j:1040 * (j + 1)],
                        cout[c * LTOT + LK + clo * 1040:
                             c * LTOT + LK + (clo + n) * 1040]
                        .rearrange("(p f) -> p f", p=n))

            boutt = pbo.tile([128, 8], F32, tag="bo")
            nc.sync.dma_start(boutt[:], bout_d.ap())
            wout_a = pwbig.tile([128, 4 * DIM], BF16, tag="wo")
            wout_b = pwbig.tile([128, 4 * DIM], BF16, tag="wo")
            nc.sync.dma_start(
                wout_a[:].rearrange("p (r c) -> p r c", r=4),
                wout_d.ap()[0:512, :].rearrange("(r p) c -> p r c", p=128))
            nc.sync.dma_start(
                wout_b[:].rearrange("p (r c) -> p r c", r=4),
                wout_d.ap()[512:1024, :].rearrange("(r p) c -> p r c", p=128))

            # ---------------- attention -------------------------------------
            att = []
            Exp = mybir.ActivationFunctionType.Exp
            for i in range(8):
                nA = psnm.tile([65, 512], F32, tag="nm")
                nB = psnm.tile([65, 512], F32, tag="nm")
                for j in range(NB):
                    ktj = kt[i][:, 128 * j:128 * (j + 1)]
                    sc = pssc.tile([128, 1024], F32, tag="sc")
                    nc.tensor.matmul(sc[:, 0:512], ktj[0:64, :], qt[i][0:64, :],
                                     start=True, stop=True)
                    nc.tensor.matmul(sc[:, 512:1024], ktj[64:128, :], qt[i][64:128, :],
                                     start=True, stop=True)
                    e = pe_.tile([128, 1024], FP8, tag="e")
                    with nc.allow_low_precision(reason="bf16 softmax weights"):
                        nc.scalar.activation(e[:], sc[:], Exp, bias=kbt[:, j:j + 1])
                    nc.tensor.matmul(nA[:], vg[:, 1040 * j + 130 * i:1040 * j + 130 * i + 65],
                                     e[:, 0:512], start=(j == 0), stop=(j == NB - 1))
                    nc.tensor.matmul(nB[:], vg[:, 1040 * j + 130 * i + 65:1040 * j + 130 * i + 130],
                                     e[:, 512:1024], start=(j == 0), stop=(j == NB - 1))
                ap_t = patt.tile([128, 512], BF16, tag="att")
                aB = pattB.tile([64, 512], BF16, tag="attB")
                if i < 7:
                    # evacuate PSUM first so the banks free for the next pair
                    for half, (nm, outap) in enumerate(((nA, ap_t[0:64, :]), (nB, aB[:]))):
                        nmc = pnmc.tile([65, 512], BF16, tag="nmc")
                        d_sb = pd.tile([1, 512], BF16, tag="d")
                        bb = pbb.tile([64, 512], BF16, tag="bb")
                        with nc.allow_low_precision(reason="bf16 softmax normalize"):
                            nc.vector.tensor_copy(nmc[:], nm[:])
                            nc.vector.reciprocal(d_sb[:], nmc[64:65, :])
                            nc.gpsimd.partition_broadcast(bb[:], d_sb[:])
                            nc.vector.tensor_mul(outap, nmc[0:64, :], bb[:])
                else:
                    # last pair: nothing needs the banks next — divide straight
                    # from PSUM with A/B interleaved to shorten the tail chain
                    d_a = pd.tile([1, 512], BF16, tag="d")
                    d_b = pd.tile([1, 512], BF16, tag="d", name="d_b")
                    bb_a = pbb.tile([64, 512], BF16, tag="bb")
                    bb_b = pbb.tile([64, 512], BF16, tag="bb", name="bb_b")
                    with nc.allow_low_precision(reason="bf16 softmax normalize"):
                        nc.vector.reciprocal(d_a[:], nA[64:65, :])
                        nc.vector.reciprocal(d_b[:], nB[64:65, :])
                        nc.gpsimd.partition_broadcast(bb_a[:], d_a[:])
                        nc.gpsimd.partition_broadcast(bb_b[:], d_b[:])
                        nc.vector.tensor_mul(ap_t[0:64, :], nA[0:64, :], bb_a[:])
                        nc.vector.tensor_mul(aB[:], nB[0:64, :], bb_b[:])
                nc.sync.dma_start(ap_t[64:128, :], aB[:])
                att.append(ap_t)

            # ---------------- output projection ------------------------------
            for cb in range(8):
                ps = ps1.tile([128, T], F32, tag="p1")
                for i in range(8):
                    wo_h = wout_a if i < 4 else wout_b
                    nc.tensor.matmul(
                        ps[:], wo_h[:, (i % 4) * DIM + 128 * cb:(i % 4) * DIM + 128 * (cb + 1)],
                        att[i][:], start=(i == 0), stop=(i == 7))
                osb = posb.tile([128, T], F32, tag="osb")
                nc.vector.tensor_scalar_add(osb[:], ps[:], boutt[:, cb:cb + 1])
                nc.sync.dma_start(y_d.ap()[128 * cb:128 * (cb + 1), :], osb[:])

    nc.compile()
    return nc


def kernel(x, mask, wq, wkv, lk1, lk2, lv1, lv2, wout, bout, **kw):
    global LAST_RESULTS
    x = np.asarray(x, np.float32)
    mask = np.asarray(mask)
    wq_s = (np.asarray(wq, np.float32) * np.float32(SCALE)).astype(NPBF16)
    w1k = (np.asarray(wkv[:, :DIM], np.float32) @ np.asarray(lk1, np.float32)).astype(NPBF16)
    w1v = (np.asarray(wkv[:, DIM:], np.float32) @ np.asarray(lv1, np.float32)).astype(NPBF16)
    lk2 = np.ascontiguousarray(np.asarray(lk2, np.float32)).astype(NPBF16)
    lv2 = np.ascontiguousarray(np.asarray(lv2, np.float32)).astype(NPBF16)
    wout = np.ascontiguousarray(np.asarray(wout, np.float32)).astype(NPBF16)
    bout2 = np.ascontiguousarray(np.asarray(bout, np.float32).reshape(8, 128).T)

    x_flat = x.reshape(B * N, DIM)
    act = [np.nonzero(np.asarray(mask[b]) == 1)[0] for b in range(B)]
    A = [len(a) for a in act]
    NB = max(1, (max(A) + 1 + 127) // 128)
    P = NB * 32

    # per-batch kv slot -> global token (or -1 pad) and key bias
    slot_tok = np.full((B, NB * 128), -1, np.int64)
    kb = np.full((B, NB * 128), NEG, np.float32)
    # constant softmax shift: e' = exp(s - C) keeps the numer/denom ratio
    # exact (the eps key shifts too) while keeping e' within fp8e4m3 range
    C = 3.5
    for b in range(B):
        slot_tok[b, :A[b]] = b * N + act[b]
        kb[b, :A[b]] = -C
        kb[b, A[b]] = np.log(1e-6) - C  # reference's denom + 1e-6
    # [slot] -> [128, NB] with slot = 128*j + p
    kb2 = np.ascontiguousarray(kb.reshape(B, NB, 128).transpose(0, 2, 1))

    if NB not in _cache:
        _cache[NB] = _build(NB)
    nc = _cache[NB]

    in_maps = []
    for c in range(NC):
        b = c // 4
        toks = slot_tok[b, (c % 4) * P:(c % 4 + 1) * P]
        xkv = np.zeros((DIM, P), NPBF16)
        real = toks >= 0
        xkv[:, real] = x_flat[toks[real]].T.astype(NPBF16)
        in_maps.append({
            "xq": np.ascontiguousarray(x_flat[c * T:(c + 1) * T].T.astype(NPBF16)),
            "xkv": xkv,
            "wq": wq_s, "w1k": w1k, "lk2": lk2, "w1v": w1v, "lv2": lv2,
            "wout": wout, "bout": bout2, "kb": kb2[b],
        })

    res = run_bass_kernel_spmd(nc, in_maps, core_ids=list(range(NC)))
    LAST_RESULTS = res
    y = np.empty((B * N, DIM), np.float32)
    for c in range(NC):
        y[c * T:(c + 1) * T] = res.results[c]["yT"].T
    return y.reshape(B, N, DIM)



# revision 4
# speedup vs baseline: 1.4639x; 1.4639x over previous
"""Trainium2 Bass kernel for KeOps multi-head latent attention (v2).

Reference (B=2, N=2048, DIM=1024, LATENT=512, HEADS=16, HD=64):
    q = x @ wq * scale
    k = relu((x @ wkv[:, :D]) @ lk1) @ lk2   (folded: relu(x @ w1k) @ lk2)
    v = relu((x @ wkv[:, D:]) @ lv1) @ lv2
    per head: e = exp(q k^T + maskbias); out = (e @ v) / (e.sum + 1e-6)
    y = out @ wout + bout

Strategy (8 cores, one SPMD NEFF, NO collective):
  - queries: tokens sharded 512/core (cores 0-3 batch0, 4-7 batch1).
  - keys: masked keys compacted on host; EVERY core computes k/v for its
    batch's full active-key set (NB*128 slots) locally — redundant compute
    is far cheaper than the modeled AllGather (15us + 40GB/s).
  - all matmuls fp8e4m3 with DoubleRow (2x modeled PE throughput) except
    the output projection (wout bf16: fp8 weight quantization error passes
    straight to the output; score-side fp8 noise is attenuated by softmax
    averaging).
  - v-path mean-centering: h_v has positive mean (relu); subtracting a
    host-computed statistical mean per latent (fp8-snapped) removes the
    coherent component of the lv2-fp8 quantization error; the mean path
    v0 = c @ lv2 rides through the output bias in fp64 (exact since
    softmax weights sum to 1). K-path coherent errors cancel in softmax.
  - scores per head via DoubleRow on 32-partition quadrants
    (tile_position): head dims split 32+32 across the two DR planes.
  - denominator: 64 'ones' columns interleaved with v give a PE-broadcast
    denominator on psum partitions 64:127 (free), so normalize is one
    reciprocal + one multiply on DVE per head.
  - exp split between ACT (true exp, fp8 out) and DVE (Schraudolph uint8
    bit-trick -> fp8e4m3, bit-exact validated on HW) to balance engines.
"""

import sys

sys.path.insert(0, "/opt/trn_rl_repo")
import numpy as np
import ml_dtypes
import concourse.bass as bass
import concourse.mybir as mybir
import concourse.tile as tile
from concourse import bacc
from concourse.bass_utils import run_bass_kernel_spmd

DIM, LATENT, HEADS, HD = 1024, 512, 16, 64
B, N, NC, T = 2, 2048, 8, 512
SCALE = HD ** -0.5
BF16, F32, FP8 = mybir.dt.bfloat16, mybir.dt.float32, mybir.dt.float8e4
U8 = mybir.dt.uint8
NPBF16 = ml_dtypes.bfloat16
NPFP8 = ml_dtypes.float8_e4m3
DR = mybir.MatmulPerfMode.DoubleRow

LN2 = float(np.log(2.0))
C_SHIFT = 5 * LN2            # exp shift; e^-C folded via bias, 2^-5 exact
NEGB = -35.0                 # pad-kill bias
C8 = 0.0435                  # schraudolph tuning constant
A8 = 8.0 / LN2
A8S = A8 * SCALE / 2.0       # DVE schr multiplier on raw scores
KB2R = 8.0 * (7.0 - C8) - A8 * C_SHIFT   # schr bias, real keys

_cache: dict = {}
LAST_RESULTS = None


def _build(NB, FPB):
    """NB = key blocks of 128 per batch; FPB = first block containing pads
    (blocks < FPB use constant exp bias; blocks >= FPB use per-slot AP)."""
    NK = NB * 128
    Exp = mybir.ActivationFunctionType.Exp
    Relu = mybir.ActivationFunctionType.Relu
    Copy = mybir.ActivationFunctionType.Copy
    Ident = mybir.ActivationFunctionType.Identity
    Alu = mybir.AluOpType

    nc = bacc.Bacc("TRN2", target_bir_lowering=False, num_devices=NC)
    xq_d = nc.dram_tensor("xq", [DIM, T], FP8, kind="ExternalInput")
    xkv_d = nc.dram_tensor("xkv", [DIM, NK], FP8, kind="ExternalInput")
    wq_d = nc.dram_tensor("wq", [DIM, DIM], FP8, kind="ExternalInput")
    w1k_d = nc.dram_tensor("w1k", [DIM, LATENT], FP8, kind="ExternalInput")
    w1v_d = nc.dram_tensor("w1v", [DIM, LATENT], FP8, kind="ExternalInput")
    lk2_d = nc.dram_tensor("lk2", [LATENT, DIM], FP8, kind="ExternalInput")
    lv2_d = nc.dram_tensor("lv2", [LATENT, DIM], FP8, kind="ExternalInput")
    wout_d = nc.dram_tensor("wout", [DIM, DIM], BF16, kind="ExternalInput")
    bout_d = nc.dram_tensor("bout2", [128, 8], F32, kind="ExternalInput")
    kbt_d = nc.dram_tensor("kbt", [128, NB], F32, kind="ExternalInput")
    kbt2_d = nc.dram_tensor("kbt2", [128, NB], F32, kind="ExternalInput")
    cv_d = nc.dram_tensor("cv", [128, 4], F32, kind="ExternalInput")
    y_d = nc.dram_tensor("yT", [DIM, T], F32, kind="ExternalOutput")

    from contextlib import ExitStack
    with ExitStack() as ctx:
        tc = ctx.enter_context(tile.TileContext(nc))
        pool = lambda **kw: ctx.enter_context(tc.tile_pool(**kw))
        pw1 = pool(name="pw1", bufs=2)
        pl2 = pool(name="pl2", bufs=2)
        pwq = pool(name="pwq", bufs=1)
        pwo = pool(name="pwo", bufs=1)
        px = pool(name="px", bufs=1)
        ph = pool(name="ph", bufs=2)
        pkt = pool(name="pkt", bufs=1)
        pv = pool(name="pv", bufs=1)
        pqt = pool(name="pqt", bufs=1)
        patt = pool(name="patt", bufs=1)
        pe_ = pool(name="pe", bufs=6)
        pr = pool(name="pr", bufs=3)
        posb = pool(name="posb", bufs=2)
        psm = pool(name="psm", bufs=1)
        psA = pool(name="psA", bufs=2, space="PSUM")   # [128,1024] x2 = 8KB
        psN = pool(name="psN", bufs=2, space="PSUM")   # [128,1024] x2 = 8KB

        # ---------------- input DMAs ------------------------------------
        w1k_sb = pw1.tile([128, 8 * LATENT], FP8, tag="w1")
        w1v_sb = pw1.tile([128, 8 * LATENT], FP8, tag="w1")
        xkv_sb = px.tile([128, 8 * NK], FP8, tag="xkv")
        cvt = psm.tile([128, 4], F32, tag="cv")
        nc.sync.dma_start(
            w1k_sb[:].rearrange("p (d l) -> p d l", d=8),
            w1k_d.ap().rearrange("(d p) l -> p d l", p=128))
        nc.sync.dma_start(
            w1v_sb[:].rearrange("p (d l) -> p d l", d=8),
            w1v_d.ap().rearrange("(d p) l -> p d l", p=128))
        nc.sync.dma_start(
            xkv_sb[:].rearrange("p (d n) -> p d n", d=8),
            xkv_d.ap().rearrange("(d p) n -> p d n", p=128))
        nc.sync.dma_start(cvt[:], cv_d.ap())

        lk2_sb = pl2.tile([128, 4 * DIM], FP8, tag="l2")
        lv2_sb = pl2.tile([128, 4 * DIM], FP8, tag="l2")
        nc.sync.dma_start(
            lk2_sb[:].rearrange("p (l c) -> p l c", l=4),
            lk2_d.ap().rearrange("(l p) c -> p l c", p=128))
        nc.sync.dma_start(
            lv2_sb[:].rearrange("p (l c) -> p l c", l=4),
            lv2_d.ap().rearrange("(l p) c -> p l c", p=128))

        wq_sb = pwq.tile([128, 8 * DIM], FP8, tag="wq")
        xq_sb = px.tile([128, 8 * T], FP8, tag="xq")
        nc.sync.dma_start(
            wq_sb[:].rearrange("p (d c) -> p d c", d=8),
            wq_d.ap().rearrange("(d p) c -> p d c", p=128))
        nc.sync.dma_start(
            xq_sb[:].rearrange("p (d n) -> p d n", d=8),
            xq_d.ap().rearrange("(d p) n -> p d n", p=128))

        kbt = psm.tile([128, NB], F32, tag="kbt")
        kbt2 = psm.tile([128, NB], F32, tag="kbt2")
        nc.sync.dma_start(kbt[:], kbt_d.ap())
        nc.sync.dma_start(kbt2[:], kbt2_d.ap())

        wout_sb = pwo.tile([128, 8 * DIM], BF16, tag="wo")
        boutt = psm.tile([128, 8], F32, tag="bo")
        nc.sync.dma_start(
            wout_sb[:].rearrange("p (d c) -> p d c", d=8),
            wout_d.ap().rearrange("(d p) c -> p d c", p=128))
        nc.sync.dma_start(boutt[:], bout_d.ap())

        # 3D chunk-major views
        w1k3 = w1k_sb[:].rearrange("p (d l) -> p d l", d=8)
        w1v3 = w1v_sb[:].rearrange("p (d l) -> p d l", d=8)
        xkv3 = xkv_sb[:].rearrange("p (d n) -> p d n", d=8)
        lk23 = lk2_sb[:].rearrange("p (l c) -> p l c", l=4)
        lv23 = lv2_sb[:].rearrange("p (l c) -> p l c", l=4)
        wq3 = wq_sb[:].rearrange("p (d c) -> p d c", d=8)
        xq3 = xq_sb[:].rearrange("p (d n) -> p d n", d=8)

        hk_sb = ph.tile([128, 4 * NK], FP8, tag="h")
        hv_sb = ph.tile([128, 4 * NK], FP8, tag="h")
        hk3 = hk_sb[:].rearrange("p (l n) -> p l n", l=4)
        hv3 = hv_sb[:].rearrange("p (l n) -> p l n", l=4)
        kt_sb = pkt.tile([128, 8 * NK], FP8, tag="kt")
        v_sb = pv.tile([128, NB * 2048], FP8, tag="v")
        qt_sb = pqt.tile([128, 8 * T], FP8, tag="qt")
        att_sb = patt.tile([128, 8 * T], BF16, tag="att")

        KEY_CH = [(0, 1024)] + ([(1024, NK - 1024)] if NK > 1024 else [])

        # ones columns for the PE-broadcast denominator (Pool, idle engine)
        v4 = v_sb[:].rearrange("p (j h two d) -> p (j h) two d",
                               j=NB, h=HEADS, two=2)
        nc.gpsimd.memset(v4[:, :, 1, :], 1.0)

        def h_path(w13, dst3, is_v):
            for l in range(4):
                ps = psA.tile([128, 1024], F32, tag="big")
                pst = psN.tile([128, 1024], F32, tag="nm")
                for g0 in (0, 512):
                    for dp in range(4):
                        nc.tensor.matmul(
                            ps[:, g0:g0 + 512],
                            w13[:, 2 * dp:2 * dp + 2, 128 * l:128 * (l + 1)],
                            xkv3[:, 2 * dp:2 * dp + 2, g0:g0 + 512],
                            start=(dp == 0), stop=(dp == 3), perf_mode=DR)
                if NK > 1024:
                    for dp in range(4):
                        nc.tensor.matmul(
                            pst[:, 0:NK - 1024],
                            w13[:, 2 * dp:2 * dp + 2, 128 * l:128 * (l + 1)],
                            xkv3[:, 2 * dp:2 * dp + 2, 1024:NK],
                            start=(dp == 0), stop=(dp == 3), perf_mode=DR)
                with nc.allow_low_precision(reason="fp8 latents"):
                    if is_v:
                        # (max(ps,0) - cv) on DVE; centering the v-latents
                        nc.vector.tensor_scalar(
                            dst3[:, l, 0:1024], ps[:], 0.0, cvt[:, l:l + 1],
                            Alu.max, Alu.subtract)
                        if NK > 1024:
                            nc.vector.tensor_scalar(
                                dst3[:, l, 1024:NK], pst[:, 0:NK - 1024],
                                0.0, cvt[:, l:l + 1], Alu.max, Alu.subtract)
                    else:
                        nc.scalar.activation(dst3[:, l, 0:1024], ps[:], Relu,
                                             scale=2.0 ** -5)
                        if NK > 1024:
                            nc.scalar.activation(dst3[:, l, 1024:NK],
                                                 pst[:, 0:NK - 1024], Relu,
                                                 scale=2.0 ** -5)

        h_path(w1k3, hk3, False)
        h_path(w1v3, hv3, True)

        # ---------------- kT (8 chunks), v (NB blocks), q (4 pairs) ------
        def kt_chunk(c8):
            ps = psA.tile([128, 1024], F32, tag="big")
            pst = psN.tile([128, 1024], F32, tag="nm")
            for g0 in (0, 512):
                for lp in range(2):
                    nc.tensor.matmul(
                        ps[:, g0:g0 + 512],
                        lk23[:, 2 * lp:2 * lp + 2, 128 * c8:128 * (c8 + 1)],
                        hk3[:, 2 * lp:2 * lp + 2, g0:g0 + 512],
                        start=(lp == 0), stop=(lp == 1), perf_mode=DR)
            if NK > 1024:
                for lp in range(2):
                    nc.tensor.matmul(
                        pst[:, 0:NK - 1024],
                        lk23[:, 2 * lp:2 * lp + 2, 128 * c8:128 * (c8 + 1)],
                        hk3[:, 2 * lp:2 * lp + 2, 1024:NK],
                        start=(lp == 0), stop=(lp == 1), perf_mode=DR)
            with nc.allow_low_precision(reason="fp8 k"):
                nc.scalar.activation(kt_sb[:, c8 * NK:c8 * NK + 1024], ps[:],
                                     Copy, scale=2.0 ** -4)
                if NK > 1024:
                    nc.scalar.activation(kt_sb[:, c8 * NK + 1024:(c8 + 1) * NK],
                                         pst[:, 0:NK - 1024], Copy,
                                         scale=2.0 ** -4)

        def v_block(j):
            ps = psA.tile([128, 1024], F32, tag="big")
            for ch in range(2):
                for lp in range(2):
                    nc.tensor.matmul(
                        ps[:, 512 * ch:512 * (ch + 1)],
                        hv3[:, 2 * lp:2 * lp + 2, 128 * j:128 * (j + 1)],
                        lv23[:, 2 * lp:2 * lp + 2, 512 * ch:512 * (ch + 1)],
                        start=(lp == 0), stop=(lp == 1), perf_mode=DR)
            dst = v_sb[:, j * 2048:(j + 1) * 2048] \
                .rearrange("p (h two d) -> p h two d", h=16, two=2)[:, :, 0, :]
            with nc.allow_low_precision(reason="fp8 v"):
                nc.scalar.activation(
                    dst, ps[:].rearrange("p (h d) -> p h d", h=16),
                    Copy, scale=2.0 ** -8)

        def q_pair(t):
            ps = psA.tile([128, 1024], F32, tag="big")
            for pl in range(2):
                for dp in range(4):
                    nc.tensor.matmul(
                        ps[:, 512 * pl:512 * (pl + 1)],
                        wq3[:, 2 * dp:2 * dp + 2,
                            (2 * t + pl) * 128:(2 * t + pl + 1) * 128],
                        xq3[:, 2 * dp:2 * dp + 2, :],
                        start=(dp == 0), stop=(dp == 3), perf_mode=DR)
            with nc.allow_low_precision(reason="fp8 q"):
                nc.scalar.activation(qt_sb[:, t * 1024:(t + 1) * 1024], ps[:],
                                     Copy)

        # interleave kT / v / q emission so the psA ring stays busy
        order = []
        for i in range(max(8, NB, 4)):
            if i < 8:
                order.append(("k", i))
            if i < NB:
                order.append(("v", i))
            if i < 4:
                order.append(("q", i))
        for kind, i in order:
            (kt_chunk if kind == "k" else v_block if kind == "v" else q_pair)(i)

        # ---------------- attention -------------------------------------
        NPAIR = NB // 2
        v3 = v_sb[:].rearrange("p (j x) -> p j x", j=NB)
        for hpair in range(8):
            nm = psN.tile([128, 1024], F32, tag="nm")
            for sub in range(2):
                h = 2 * hpair + sub
                t, g = h // 4, h % 4
                kt3 = kt_sb[:, t * 2 * NK:(t + 1) * 2 * NK] \
                    .rearrange("p (pl k) -> p pl k", pl=2)
                qt3 = qt_sb[:, t * 1024:(t + 1) * 1024] \
                    .rearrange("p (pl n) -> p pl n", pl=2)
                lhq = qt3[32 * g:32 * (g + 1), :, :]
                nmh = nm[:, 512 * sub:512 * (sub + 1)]
                for bp in range(NPAIR):
                    sc = psA.tile([128, 1024], F32, tag="big")
                    for half in range(2):
                        j = 2 * bp + half
                        nc.tensor.matmul(
                            sc[:, 512 * half:512 * (half + 1)],
                            kt3[32 * g:32 * (g + 1), :, 128 * j:128 * (j + 1)],
                            lhq, start=True, stop=True, perf_mode=DR,
                            tile_position=(32 * g, 0))
                    e = pe_.tile([128, 1024], FP8, tag="e")
                    with nc.allow_low_precision(reason="fp8 softmax"):
                        if 2 * bp + 1 < FPB:
                            # pad-free pair: one big op, constant bias
                            if bp < 2:
                                nc.scalar.activation(e[:], sc[:], Exp,
                                                     bias=kbt[:, 0:1],
                                                     scale=SCALE / 2)
                            else:
                                nc.vector.tensor_scalar(
                                    e[:].bitcast(U8), sc[:], A8S, KB2R,
                                    Alu.mult, Alu.add)
                        else:
                            for half in range(2):
                                j = 2 * bp + half
                                esl = e[:, 512 * half:512 * (half + 1)]
                                ssl = sc[:, 512 * half:512 * (half + 1)]
                                if j < FPB:
                                    nc.vector.tensor_scalar(
                                        esl.bitcast(U8), ssl, A8S, KB2R,
                                        Alu.mult, Alu.add)
                                else:
                                    nc.vector.tensor_scalar(
                                        esl.bitcast(U8), ssl, A8S,
                                        kbt2[:, j:j + 1], Alu.mult, Alu.add)
                    nc.tensor.matmul(
                        nmh, v3[:, 2 * bp:2 * bp + 2, 128 * h:128 * (h + 1)],
                        e[:].rearrange("p (two n) -> p two n", two=2),
                        start=(bp == 0), stop=False, perf_mode=DR)
                # leftover odd block(s)
                for j in range(2 * NPAIR, NB):
                    sc1 = psA.tile([128, 1024], F32, tag="big")
                    nc.tensor.matmul(
                        sc1[:, 0:512],
                        kt3[32 * g:32 * (g + 1), :, 128 * j:128 * (j + 1)],
                        lhq, start=True, stop=True, perf_mode=DR,
                        tile_position=(32 * g, 0))
                    e1 = pe_.tile([128, 512], FP8, tag="e1")
                    with nc.allow_low_precision(reason="fp8 softmax"):
                        if h % 2 == 0:
                            nc.scalar.activation(e1[:], sc1[:, 0:512], Exp,
                                                 bias=kbt[:, j:j + 1],
                                                 scale=SCALE / 2)
                        else:
                            nc.vector.tensor_scalar(
                                e1[:].bitcast(U8), sc1[:, 0:512], A8S,
                                kbt2[:, j:j + 1], Alu.mult, Alu.add)
                    nc.tensor.matmul(
                        nmh, v3[:, j, 128 * h:128 * (h + 1)], e1[:],
                        start=False, stop=(j == NB - 1), skip_group_check=True)
            # normalize both heads of the pair: recip over the PE-broadcast
            # denominators, then numer * recip -> att (bf16)
            rr = pr.tile([64, 1024], F32, tag="r")
            nc.vector.reciprocal(rr[:], nm[64:128, :])
            with nc.allow_low_precision(reason="bf16 att"):
                for sub in range(2):
                    h = 2 * hpair + sub
                    nc.vector.tensor_mul(
                        att_sb[64 * (h % 2):64 * (h % 2) + 64,
                               (h // 2) * T:(h // 2 + 1) * T],
                        nm[0:64, 512 * sub:512 * (sub + 1)],
                        rr[:, 512 * sub:512 * (sub + 1)])

        # ---------------- output projection ------------------------------
        for cb2 in range(4):
            ps = psN.tile([128, 1024], F32, tag="nm")
            for half in range(2):
                cb = 2 * cb2 + half
                for c in range(8):
                    nc.tensor.matmul(
                        ps[:, 512 * half:512 * (half + 1)],
                        wout_sb[:, c * DIM + 128 * cb:c * DIM + 128 * (cb + 1)],
                        att_sb[:, c * T:(c + 1) * T],
                        start=(c == 0), stop=(c == 7))
            osb = posb.tile([128, 1024], F32, tag="osb")
            for half in range(2):
                cb = 2 * cb2 + half
                nc.scalar.activation(osb[:, 512 * half:512 * (half + 1)],
                                     ps[:, 512 * half:512 * (half + 1)],
                                     Ident, bias=boutt[:, cb:cb + 1])
            nc.sync.dma_start(
                y_d.ap()[256 * cb2:256 * (cb2 + 1), :]
                .rearrange("(c p) n -> p c n", p=128),
                osb[:].rearrange("p (c n) -> p c n", c=2))

    nc.compile()
    return nc


def _f8(x):
    return np.asarray(x, np.float32).astype(NPFP8)


def kernel(x, mask, wq, wkv, lk1, lk2, lv1, lv2, wout, bout, **kw):
    global LAST_RESULTS
    x = np.asarray(x, np.float32)
    mask = np.asarray(mask)
    wq = np.asarray(wq, np.float64)
    wkv = np.asarray(wkv, np.float64)
    lk1 = np.asarray(lk1, np.float64)
    lk2 = np.asarray(lk2, np.float64)
    lv1 = np.asarray(lv1, np.float64)
    lv2 = np.asarray(lv2, np.float64)
    wout = np.asarray(wout, np.float64)
    bout = np.asarray(bout, np.float64)

    act = [np.nonzero(np.asarray(mask[b]) == 1)[0] for b in range(B)]
    A = [len(a) for a in act]
    NB = max(1, (max(A) + 127) // 128)
    NK = NB * 128
    FPB = min(A) // 128          # first block that contains pad slots

    # column permutation for the DR-32 scores layout:
    # psum chunk (t,pl) partitions = [head 4t+g, dims 32pl..32pl+32]
    perm = np.array([64 * (4 * t + g) + 32 * pl + i
                     for t in range(4) for pl in range(2)
                     for g in range(4) for i in range(32)])

    w1k = wkv[:, :DIM] @ lk1
    w1v = wkv[:, DIM:] @ lv1
    w1k8 = _f8(32 * w1k)
    w1v8 = _f8(32 * w1v)
    lk28 = _f8(32 * lk2[:, perm])
    lv28 = _f8(32 * lv2)
    wq8 = _f8(wq[:, perm])
    woutb = np.asarray(wout / 4.0, np.float32).astype(NPBF16)

    # v-path centering: statistical mean of relu(w1v8 . x) per latent,
    # snapped to the fp8 grid so exact relu-zeros quantize exactly
    xr = float(np.sqrt((x.astype(np.float64) ** 2).mean()))
    colv = np.sqrt((w1v8.astype(np.float64) ** 2).sum(0)) * xr
    cv32 = _f8(0.39894228 * colv).astype(np.float64)       # scale-32 units
    v0 = (cv32 / 32.0) @ lv2                               # exact lv2
    bout2 = bout + v0 @ wout
    bout2_t = np.ascontiguousarray(
        bout2.reshape(8, 128).T.astype(np.float32))
    cv_t = np.ascontiguousarray(
        cv32.reshape(4, 128).T.astype(np.float32))

    # exp biases per key slot (per batch)
    kbt = np.full((B, NK), NEGB, np.float32)
    kbt2 = np.full((B, NK), KB2R + A8 * (NEGB + C_SHIFT), np.float32)
    for b in range(B):
        kbt[b, :A[b]] = -C_SHIFT
        kbt2[b, :A[b]] = KB2R
    kbt_t = [np.ascontiguousarray(kbt[b].reshape(NB, 128).T) for b in range(B)]
    kbt2_t = [np.ascontiguousarray(kbt2[b].reshape(NB, 128).T) for b in range(B)]

    key = (NB, FPB)
    if key not in _cache:
        _cache[key] = _build(NB, FPB)
    nc = _cache[key]

    x_flat = x.reshape(B * N, DIM)
    xkv_b = []
    for b in range(B):
        xkv = np.zeros((DIM, NK), NPFP8)
        xkv[:, :A[b]] = _f8(x_flat[b * N + act[b]].T)
        xkv_b.append(xkv)

    in_maps = []
    for c in range(NC):
        b = c // 4
        in_maps.append({
            "xq": np.ascontiguousarray(_f8(x_flat[c * T:(c + 1) * T].T)),
            "xkv": xkv_b[b],
            "wq": wq8, "w1k": w1k8, "w1v": w1v8, "lk2": lk28, "lv2": lv28,
            "wout": woutb, "bout2": bout2_t, "kbt": kbt_t[b],
            "kbt2": kbt2_t[b], "cv": cv_t,
        })

    res = run_bass_kernel_spmd(nc, in_maps, core_ids=list(range(NC)))
    LAST_RESULTS = res
    y = np.empty((B * N, DIM), np.float32)
    for c in range(NC):
        y[c * T:(c + 1) * T] = res.results[c]["yT"].T
    return y.reshape(B, N, DIM)


# revision 44
# speedup vs baseline: 1.9911x; 1.3602x over previous
"""Trainium2 Bass kernel for KeOps multi-head latent attention (v2).

Reference (B=2, N=2048, DIM=1024, LATENT=512, HEADS=16, HD=64):
    q = x @ wq * scale
    k = relu((x @ wkv[:, :D]) @ lk1) @ lk2   (folded: relu(x @ w1k) @ lk2)
    v = relu((x @ wkv[:, D:]) @ lv1) @ lv2
    per head: e = exp(q k^T + maskbias); out = (e @ v) / (e.sum + 1e-6)
    y = out @ wout + bout

Strategy (8 cores, one SPMD NEFF, NO collective):
  - queries: tokens sharded 512/core (cores 0-3 batch0, 4-7 batch1).
  - keys: masked keys compacted on host; EVERY core computes k/v for its
    batch's full active-key set (NB*128 slots) locally — redundant compute
    is far cheaper than the modeled AllGather (15us + 40GB/s).
  - all matmuls fp8e4m3 with DoubleRow (2x modeled PE throughput) except
    the output projection (wout bf16: fp8 weight quantization error passes
    straight to the output; score-side fp8 noise is attenuated by softmax
    averaging).
  - v-path mean-centering: h_v has positive mean (relu); subtracting a
    host-computed statistical mean per latent (fp8-snapped) removes the
    coherent component of the lv2-fp8 quantization error; the mean path
    v0 = c @ lv2 rides through the output bias in fp64 (exact since
    softmax weights sum to 1). K-path coherent errors cancel in softmax.
  - scores per head via DoubleRow on 32-partition quadrants
    (tile_position): head dims split 32+32 across the two DR planes.
  - denominator: 64 'ones' columns interleaved with v give a PE-broadcast
    denominator on psum partitions 64:127 (free), so normalize is one
    reciprocal + one multiply on DVE per head.
  - exp split between ACT (true exp, fp8 out) and DVE (Schraudolph uint8
    bit-trick -> fp8e4m3, bit-exact validated on HW) to balance engines.
"""

import sys

sys.path.insert(0, "/opt/trn_rl_repo")
import numpy as np
import ml_dtypes
import concourse.bass as bass
import concourse.mybir as mybir
import concourse.tile as tile
from concourse import bacc
from concourse.bass_utils import run_bass_kernel_spmd

DIM, LATENT, HEADS, HD = 1024, 512, 16, 64
B, N, NC, T = 2, 2048, 8, 512
SCALE = HD ** -0.5
BF16, F32, FP8 = mybir.dt.bfloat16, mybir.dt.float32, mybir.dt.float8e4
U8 = mybir.dt.uint8
NPBF16 = ml_dtypes.bfloat16
NPFP8 = ml_dtypes.float8_e4m3
DR = mybir.MatmulPerfMode.DoubleRow

LN2 = float(np.log(2.0))
C_SHIFT = 5 * LN2            # exp shift; e^-C folded via bias, 2^-5 exact
NEGB = -35.0                 # pad-kill bias
C8 = 0.0435                  # schraudolph tuning constant
A8 = 8.0 / LN2
A8S = A8 * SCALE / 2.0       # DVE schr multiplier on raw scores
KB2R = 8.0 * (7.0 - C8) - A8 * C_SHIFT   # schr bias, real keys

_cache: dict = {}
LAST_RESULTS = None


def _build(NB, FPB):
    """NB = key blocks of 128 per batch; FPB = first block containing pads
    (blocks < FPB use constant exp bias; blocks >= FPB use per-slot AP)."""
    NK = NB * 128
    Exp = mybir.ActivationFunctionType.Exp
    Relu = mybir.ActivationFunctionType.Relu
    Copy = mybir.ActivationFunctionType.Copy
    Ident = mybir.ActivationFunctionType.Identity
    Alu = mybir.AluOpType

    nc = bacc.Bacc("TRN2", target_bir_lowering=False, num_devices=NC)
    xq_d = nc.dram_tensor("xq", [DIM, T], FP8, kind="ExternalInput")
    xkv_d = nc.dram_tensor("xkv", [DIM, NK], FP8, kind="ExternalInput")
    wq_d = nc.dram_tensor("wq", [DIM, DIM], FP8, kind="ExternalInput")
    w1k_d = nc.dram_tensor("w1k", [DIM, LATENT], FP8, kind="ExternalInput")
    w1v_d = nc.dram_tensor("w1v", [DIM, LATENT], FP8, kind="ExternalInput")
    lk2_d = nc.dram_tensor("lk2", [LATENT, DIM], FP8, kind="ExternalInput")
    lv2_d = nc.dram_tensor("lv2", [LATENT, DIM], FP8, kind="ExternalInput")
    wout_d = nc.dram_tensor("wout", [DIM, DIM], BF16, kind="ExternalInput")
    bout_d = nc.dram_tensor("bout2", [128, 8], F32, kind="ExternalInput")
    kbt_d = nc.dram_tensor("kbt", [128, NB], F32, kind="ExternalInput")
    kbt2_d = nc.dram_tensor("kbt2", [128, NB], F32, kind="ExternalInput")
    cv_d = nc.dram_tensor("cv", [128, 4], F32, kind="ExternalInput")
    y_d = nc.dram_tensor("yT", [DIM, T], F32, kind="ExternalOutput")

    from contextlib import ExitStack
    with ExitStack() as ctx:
        tc = ctx.enter_context(tile.TileContext(nc))
        pool = lambda **kw: ctx.enter_context(tc.tile_pool(**kw))
        pw1 = pool(name="pw1", bufs=2)
        pl2 = pool(name="pl2", bufs=2)
        pwq = pool(name="pwq", bufs=1)
        pwo = pool(name="pwo", bufs=1)
        px = pool(name="px", bufs=1)
        ph = pool(name="ph", bufs=2)
        pkt = pool(name="pkt", bufs=1)
        pv = pool(name="pv", bufs=1)
        pqt = pool(name="pqt", bufs=1)
        patt = pool(name="patt", bufs=1)
        pe_ = pool(name="pe", bufs=6)
        pr = pool(name="pr", bufs=3)
        posb = pool(name="posb", bufs=2)
        psm = pool(name="psm", bufs=1)
        psA = pool(name="psA", bufs=3, space="PSUM")   # [128,1024] x3 = 12KB
        psN = pool(name="psN", bufs=2, space="PSUM")   # [128,512]  x2 =  4KB

        # ---------------- input DMAs ------------------------------------
        # split the critical first loads in halves so h matmuls start early
        w1k_sb = pw1.tile([128, 8 * LATENT], FP8, tag="w1")
        w1v_sb = pw1.tile([128, 8 * LATENT], FP8, tag="w1")
        xkv_sb = px.tile([128, 8 * NK], FP8, tag="xkv")
        cvt = psm.tile([128, 4], F32, tag="cv")
        xkv3v = xkv_sb[:].rearrange("p (d n) -> p d n", d=8)
        w1k3v = w1k_sb[:].rearrange("p (d l) -> p d l", d=8)
        w1v3v = w1v_sb[:].rearrange("p (d l) -> p d l", d=8)
        for hf in range(2):
            dd = slice(512 * hf, 512 * (hf + 1))
            nc.sync.dma_start(
                xkv3v[:, 4 * hf:4 * hf + 4],
                xkv_d.ap()[dd, :].rearrange("(d p) n -> p d n", p=128))
            nc.sync.dma_start(
                w1k3v[:, 4 * hf:4 * hf + 4],
                w1k_d.ap()[dd, :].rearrange("(d p) l -> p d l", p=128))
            nc.sync.dma_start(
                w1v3v[:, 4 * hf:4 * hf + 4],
                w1v_d.ap()[dd, :].rearrange("(d p) l -> p d l", p=128))
        nc.sync.dma_start(cvt[:], cv_d.ap())

        lk2_sb = pl2.tile([128, 4 * DIM], FP8, tag="l2")
        lv2_sb = pl2.tile([128, 4 * DIM], FP8, tag="l2")
        nc.sync.dma_start(
            lk2_sb[:].rearrange("p (l c) -> p l c", l=4),
            lk2_d.ap().rearrange("(l p) c -> p l c", p=128))
        nc.sync.dma_start(
            lv2_sb[:].rearrange("p (l c) -> p l c", l=4),
            lv2_d.ap().rearrange("(l p) c -> p l c", p=128))

        wq_sb = pwq.tile([128, 8 * DIM], FP8, tag="wq")
        xq_sb = px.tile([128, 8 * T], FP8, tag="xq")
        nc.sync.dma_start(
            wq_sb[:].rearrange("p (d c) -> p d c", d=8),
            wq_d.ap().rearrange("(d p) c -> p d c", p=128))
        nc.sync.dma_start(
            xq_sb[:].rearrange("p (d n) -> p d n", d=8),
            xq_d.ap().rearrange("(d p) n -> p d n", p=128))

        kbt = psm.tile([128, NB], F32, tag="kbt")
        kbt2 = psm.tile([128, NB], F32, tag="kbt2")
        nc.sync.dma_start(kbt[:], kbt_d.ap())
        nc.sync.dma_start(kbt2[:], kbt2_d.ap())

        wout_sb = pwo.tile([128, 8 * DIM], BF16, tag="wo")
        boutt = psm.tile([128, 8], F32, tag="bo")
        nc.sync.dma_start(
            wout_sb[:].rearrange("p (d c) -> p d c", d=8),
            wout_d.ap().rearrange("(d p) c -> p d c", p=128))
        nc.sync.dma_start(boutt[:], bout_d.ap())

        # 3D chunk-major views
        w1k3 = w1k_sb[:].rearrange("p (d l) -> p d l", d=8)
        w1v3 = w1v_sb[:].rearrange("p (d l) -> p d l", d=8)
        xkv3 = xkv_sb[:].rearrange("p (d n) -> p d n", d=8)
        lk23 = lk2_sb[:].rearrange("p (l c) -> p l c", l=4)
        lv23 = lv2_sb[:].rearrange("p (l c) -> p l c", l=4)
        wq3 = wq_sb[:].rearrange("p (d c) -> p d c", d=8)
        xq3 = xq_sb[:].rearrange("p (d n) -> p d n", d=8)

        hk_sb = ph.tile([128, 4 * NK], FP8, tag="h")
        hv_sb = ph.tile([128, 4 * NK], FP8, tag="h")
        hk3 = hk_sb[:].rearrange("p (l n) -> p l n", l=4)
        hv3 = hv_sb[:].rearrange("p (l n) -> p l n", l=4)
        kt_sb = pkt.tile([128, 8 * NK], FP8, tag="kt")
        v_sb = pv.tile([128, NB * 2048], FP8, tag="v")
        qt_sb = pqt.tile([128, 8 * T], FP8, tag="qt")
        att_sb = patt.tile([128, 8 * T], BF16, tag="att")

        KEY_CH = [(0, 1024)] + ([(1024, NK - 1024)] if NK > 1024 else [])

        # ones columns for the PE-broadcast denominator (Pool, idle engine)
        v4 = v_sb[:].rearrange("p (j h two d) -> p (j h) two d",
                               j=NB, h=HEADS, two=2)
        nc.gpsimd.memset(v4[:, :, 1, :], 1.0)

        def h_path_l(w13, dst3, is_v, l):
            if True:
                ps = psA.tile([128, 1024], F32, tag="big")
                pst = psN.tile([128, 512], F32, tag="nm")
                for g0 in (0, 512):
                    for dp in range(4):
                        nc.tensor.matmul(
                            ps[:, g0:g0 + 512],
                            w13[:, 2 * dp:2 * dp + 2, 128 * l:128 * (l + 1)],
                            xkv3[:, 2 * dp:2 * dp + 2, g0:g0 + 512],
                            start=(dp == 0), stop=(dp == 3), perf_mode=DR)
                if NK > 1024:
                    for dp in range(4):
                        nc.tensor.matmul(
                            pst[:, 0:NK - 1024],
                            w13[:, 2 * dp:2 * dp + 2, 128 * l:128 * (l + 1)],
                            xkv3[:, 2 * dp:2 * dp + 2, 1024:NK],
                            start=(dp == 0), stop=(dp == 3), perf_mode=DR)
                with nc.allow_low_precision(reason="fp8 latents"):
                    if is_v:
                        # (max(ps,0) - cv) on DVE; centering the v-latents
                        nc.vector.tensor_scalar(
                            dst3[:, l, 0:1024], ps[:], 0.0, cvt[:, l:l + 1],
                            Alu.max, Alu.subtract)
                        if NK > 1024:
                            nc.vector.tensor_scalar(
                                dst3[:, l, 1024:NK], pst[:, 0:NK - 1024],
                                0.0, cvt[:, l:l + 1], Alu.max, Alu.subtract)
                    else:
                        nc.scalar.activation(dst3[:, l, 0:1024], ps[:], Relu,
                                             scale=2.0 ** -5)
                        if NK > 1024:
                            nc.scalar.activation(dst3[:, l, 1024:NK],
                                                 pst[:, 0:NK - 1024], Relu,
                                                 scale=2.0 ** -5)

        for l in range(4):
            h_path_l(w1k3, hk3, False, l)
            h_path_l(w1v3, hv3, True, l)

        # ---------------- kT (8 chunks), v (NB blocks), q (4 pairs) ------
        def kt_chunk(c8):
            ps = psA.tile([128, 1024], F32, tag="big")
            pst = psN.tile([128, 512], F32, tag="nm")
            for g0 in (0, 512):
                for lp in range(2):
                    nc.tensor.matmul(
                        ps[:, g0:g0 + 512],
                        lk23[:, 2 * lp:2 * lp + 2, 128 * c8:128 * (c8 + 1)],
                        hk3[:, 2 * lp:2 * lp + 2, g0:g0 + 512],
                        start=(lp == 0), stop=(lp == 1), perf_mode=DR)
            if NK > 1024:
                for lp in range(2):
                    nc.tensor.matmul(
                        pst[:, 0:NK - 1024],
                        lk23[:, 2 * lp:2 * lp + 2, 128 * c8:128 * (c8 + 1)],
                        hk3[:, 2 * lp:2 * lp + 2, 1024:NK],
                        start=(lp == 0), stop=(lp == 1), perf_mode=DR)
            # kT evac on DVE (kv-phase engine balance: ACT has relu+v+q)
            with nc.allow_low_precision(reason="fp8 k"):
                nc.vector.tensor_scalar(
                    kt_sb[:, c8 * NK:c8 * NK + 1024], ps[:],
                    2.0 ** -4, None, Alu.mult)
                if NK > 1024:
                    nc.vector.tensor_scalar(
                        kt_sb[:, c8 * NK + 1024:(c8 + 1) * NK],
                        pst[:, 0:NK - 1024], 2.0 ** -4, None, Alu.mult)

        def v_block(j):
            ps = psA.tile([128, 1024], F32, tag="big")
            for ch in range(2):
                for lp in range(2):
                    nc.tensor.matmul(
                        ps[:, 512 * ch:512 * (ch + 1)],
                        hv3[:, 2 * lp:2 * lp + 2, 128 * j:128 * (j + 1)],
                        lv23[:, 2 * lp:2 * lp + 2, 512 * ch:512 * (ch + 1)],
                        start=(lp == 0), stop=(lp == 1), perf_mode=DR)
            dst = v_sb[:, j * 2048:(j + 1) * 2048] \
                .rearrange("p (h two d) -> p h two d", h=16, two=2)[:, :, 0, :]
            with nc.allow_low_precision(reason="fp8 v"):
                nc.scalar.activation(
                    dst, ps[:].rearrange("p (h d) -> p h d", h=16),
                    Copy, scale=2.0 ** -8)

        def q_pair(t):
            ps = psA.tile([128, 1024], F32, tag="big")
            for pl in range(2):
                for dp in range(4):
                    nc.tensor.matmul(
                        ps[:, 512 * pl:512 * (pl + 1)],
                        wq3[:, 2 * dp:2 * dp + 2,
                            (2 * t + pl) * 128:(2 * t + pl + 1) * 128],
                        xq3[:, 2 * dp:2 * dp + 2, :],
                        start=(dp == 0), stop=(dp == 3), perf_mode=DR)
            with nc.allow_low_precision(reason="fp8 q"):
                nc.scalar.activation(qt_sb[:, t * 1024:(t + 1) * 1024], ps[:],
                                     Copy)

        order = [("q", 0), ("q", 1), ("k", 0), ("q", 2), ("k", 1),
                 ("q", 3), ("k", 2), ("v", 0), ("k", 3), ("v", 1),
                 ("k", 4), ("v", 2), ("k", 5), ("v", 3), ("k", 6),
                 ("v", 4), ("k", 7), ("v", 5), ("v", 6), ("v", 7), ("v", 8)]
        order = [(k, i) for (k, i) in order if
                 (k != "v" or i < NB) and (k != "k" or i < 8)]
        for kind, i in order:
            (kt_chunk if kind == "k" else v_block if kind == "v" else q_pair)(i)

        def drain_kv(n):
            pass

        # ---------------- attention -------------------------------------
        # flat software pipeline across all (head, unit) pairs: scores run
        # two units ahead of exp; numer three behind; each head's normalize
        # is deferred until after the NEXT head's DVE exps so the
        # exp->numer->recip round trip never stalls DVE.  exp split: ACT
        # gets pad-free pairs 0,1,2 (const bias); DVE gets pair 3 + the
        # leftover block + reciprocal + normalize multiply.
        NPAIR = NB // 2
        NU = NPAIR + (1 if NB % 2 else 0)
        v3 = v_sb[:].rearrange("p (j x) -> p j x", j=NB)

        scs = {}
        es = {}
        nms = {}
        pending_norm = []

        def head_views(h):
            t, g = h // 4, h % 4
            kt3 = kt_sb[:, t * 2 * NK:(t + 1) * 2 * NK] \
                .rearrange("p (pl k) -> p pl k", pl=2)
            qt3 = qt_sb[:, t * 1024:(t + 1) * 1024] \
                .rearrange("p (pl n) -> p pl n", pl=2)
            return kt3, qt3[32 * g:32 * (g + 1), :, :], 32 * g

        def emit_sc(h, u):
            kt3, lhq, bp32 = head_views(h)
            if u < NPAIR:
                sc = psA.tile([128, 1024], F32, tag="big", name=f"sc{h}_{u}")
            else:
                # leftover unit lives in the psN ring so the psA ring
                # advances by 4 (not 5) per head: the next head's first sc
                # then reuses a slot freed by an early DVE exp, not ACT's
                # last one
                sc = psN.tile([128, 512], F32, tag="nm", name=f"sc{h}_{u}")
            scs[(h, u)] = sc
            nj = 2 if u < NPAIR else 1
            for half in range(nj):
                j = 2 * u + half
                nc.tensor.matmul(
                    sc[:, 512 * half:512 * (half + 1)],
                    kt3[bp32:bp32 + 32, :, 128 * j:128 * (j + 1)],
                    lhq, start=True, stop=True, perf_mode=DR,
                    tile_position=(bp32, 0))

        def emit_exp(h, u):
            sc = scs[(h, u)]
            if u < NPAIR:
                e = pe_.tile([128, 1024], FP8, tag="e")
                with nc.allow_low_precision(reason="fp8 softmax"):
                    if 2 * u + 1 < FPB and u < 3:
                        nc.scalar.activation(e[:], sc[:], Exp,
                                             bias=kbt[:, 0:1],
                                             scale=SCALE / 2)
                    else:
                        for half in range(2):
                            j = 2 * u + half
                            nc.vector.tensor_scalar(
                                e[:, 512 * half:512 * (half + 1)].bitcast(U8),
                                sc[:, 512 * half:512 * (half + 1)], A8S,
                                KB2R if j < FPB else kbt2[:, j:j + 1],
                                Alu.mult, Alu.add)
            else:
                j = 2 * NPAIR
                e = pe_.tile([128, 512], FP8, tag="e1")
                with nc.allow_low_precision(reason="fp8 softmax"):
                    if h % 2 == 0:
                        nc.scalar.activation(e[:], sc[:], Exp,
                                             bias=kbt[:, j:j + 1],
                                             scale=SCALE / 2)
                    else:
                        nc.vector.tensor_scalar(
                            e[:].bitcast(U8), sc[:], A8S,
                            KB2R if j < FPB else kbt2[:, j:j + 1],
                            Alu.mult, Alu.add)
            es[(h, u)] = e
            # emit the deferred normalize after this head's first DVE exp
            if u == 3 and pending_norm:
                pending_norm.pop(0)()

        def emit_nm(h, u):
            if u == 0:
                nms[h] = psN.tile([128, 512], F32, tag="nm", name=f"nm{h}")
            nm = nms[h]
            if u < NPAIR:
                nc.tensor.matmul(
                    nm[:], v3[:, 2 * u:2 * u + 2, 128 * h:128 * (h + 1)],
                    es[(h, u)][:].rearrange("p (two n) -> p two n", two=2),
                    start=(u == 0), stop=(u == NU - 1), perf_mode=DR)
            else:
                nc.tensor.matmul(
                    nm[:], v3[:, 2 * NPAIR, 128 * h:128 * (h + 1)],
                    es[(h, u)][:], start=False, stop=True,
                    skip_group_check=True)
            if u == NU - 1:
                def normalize(h=h, nm=nm):
                    rr = pr.tile([64, 512], F32, tag="r")
                    nc.vector.reciprocal(rr[:], nm[64:128, :])
                    with nc.allow_low_precision(reason="bf16 att"):
                        nc.vector.tensor_mul(
                            att_sb[64 * (h % 2):64 * (h % 2) + 64,
                                   (h // 2) * T:(h // 2 + 1) * T],
                            nm[0:64, :], rr[:])
                pending_norm.append(normalize)

        units = [(h, u) for h in range(HEADS) for u in range(NU)]
        LA_E, LA_N = 1, 3
        for i in range(len(units) + LA_N):
            if i < len(units):
                emit_sc(*units[i])
            if LA_E <= i and i - LA_E < len(units):
                emit_exp(*units[i - LA_E])
            drain_kv(1)
            if LA_N <= i and i - LA_N < len(units):
                emit_nm(*units[i - LA_N])
        while pending_norm:
            pending_norm.pop(0)()

        # ---------------- output projection ------------------------------
        for cb in range(8):
            ps = psA.tile([128, 1024], F32, tag="big")
            for c in range(8):
                nc.tensor.matmul(
                    ps[:, 0:512],
                    wout_sb[:, c * DIM + 128 * cb:c * DIM + 128 * (cb + 1)],
                    att_sb[:, c * T:(c + 1) * T],
                    start=(c == 0), stop=(c == 7))
            osb = posb.tile([128, 512], F32, tag="osb")
            nc.scalar.activation(osb[:], ps[:, 0:512], Ident,
                                 bias=boutt[:, cb:cb + 1])
            nc.sync.dma_start(y_d.ap()[128 * cb:128 * (cb + 1), :], osb[:])

    nc.compile()
    return nc


def _f8(x):
    return np.asarray(x, np.float32).astype(NPFP8)


def kernel(x, mask, wq, wkv, lk1, lk2, lv1, lv2, wout, bout, **kw):
    global LAST_RESULTS
    x = np.asarray(x, np.float32)
    mask = np.asarray(mask)
    wq = np.asarray(wq, np.float64)
    wkv = np.asarray(wkv, np.float64)
    lk1 = np.asarray(lk1, np.float64)
    lk2 = np.asarray(lk2, np.float64)
    lv1 = np.asarray(lv1, np.float64)
    lv2 = np.asarray(lv2, np.float64)
    wout = np.asarray(wout, np.float64)
    bout = np.asarray(bout, np.float64)

    act = [np.nonzero(np.asarray(mask[b]) == 1)[0] for b in range(B)]
    A = [len(a) for a in act]
    NB = max(1, (max(A) + 127) // 128)
    NK = NB * 128
    FPB = min(A) // 128          # first block that contains pad slots

    # column permutation for the DR-32 scores layout:
    # psum chunk (t,pl) partitions = [head 4t+g, dims 32pl..32pl+32]
    perm = np.array([64 * (4 * t + g) + 32 * pl + i
                     for t in range(4) for pl in range(2)
                     for g in range(4) for i in range(32)])

    w1k = wkv[:, :DIM] @ lk1
    w1v = wkv[:, DIM:] @ lv1
    w1k8 = _f8(32 * w1k)
    w1v8 = _f8(32 * w1v)
    lk28 = _f8(32 * lk2[:, perm])
    lv28 = _f8(32 * lv2)
    wq8 = _f8(wq[:, perm])
    woutb = np.asarray(wout / 4.0, np.float32).astype(NPBF16)

    # v-path centering: statistical mean of relu(w1v8 . x) per latent,
    # snapped to the fp8 grid so exact relu-zeros quantize exactly
    xr = float(np.sqrt((x.astype(np.float64) ** 2).mean()))
    colv = np.sqrt((w1v8.astype(np.float64) ** 2).sum(0)) * xr
    cv32 = _f8(0.39894228 * colv).astype(np.float64)       # scale-32 units
    v0 = (cv32 / 32.0) @ lv2                               # exact lv2
    bout2 = bout + v0 @ wout
    bout2_t = np.ascontiguousarray(
        bout2.reshape(8, 128).T.astype(np.float32))
    cv_t = np.ascontiguousarray(
        cv32.reshape(4, 128).T.astype(np.float32))

    # exp biases per key slot (per batch)
    kbt = np.full((B, NK), NEGB, np.float32)
    kbt2 = np.full((B, NK), KB2R + A8 * (NEGB + C_SHIFT), np.float32)
    for b in range(B):
        kbt[b, :A[b]] = -C_SHIFT
        kbt2[b, :A[b]] = KB2R
    kbt_t = [np.ascontiguousarray(kbt[b].reshape(NB, 128).T) for b in range(B)]
    kbt2_t = [np.ascontiguousarray(kbt2[b].reshape(NB, 128).T) for b in range(B)]

    key = (NB, FPB)
    if key not in _cache:
        _cache[key] = _build(NB, FPB)
    nc = _cache[key]

    x_flat = x.reshape(B * N, DIM)
    xkv_b = []
    for b in range(B):
        xkv = np.zeros((DIM, NK), NPFP8)
        xkv[:, :A[b]] = _f8(x_flat[b * N + act[b]].T)
        xkv_b.append(xkv)

    in_maps = []
    for c in range(NC):
        b = c // 4
        in_maps.append({
            "xq": np.ascontiguousarray(_f8(x_flat[c * T:(c + 1) * T].T)),
            "xkv": xkv_b[b],
            "wq": wq8, "w1k": w1k8, "w1v": w1v8, "lk2": lk28, "lv2": lv28,
            "wout": woutb, "bout2": bout2_t, "kbt": kbt_t[b],
            "kbt2": kbt2_t[b], "cv": cv_t,
        })

    res = run_bass_kernel_spmd(nc, in_maps, core_ids=list(range(NC)))
    LAST_RESULTS = res
    y = np.empty((B * N, DIM), np.float32)
    for c in range(NC):
        y[c * T:(c + 1) * T] = res.results[c]["yT"].T
    return y.reshape(B, N, DIM)


# revision 53
# speedup vs baseline: 2.0404x; 1.0247x over previous
"""Trainium2 Bass kernel for KeOps multi-head latent attention (v2).

Reference (B=2, N=2048, DIM=1024, LATENT=512, HEADS=16, HD=64):
    q = x @ wq * scale
    k = relu((x @ wkv[:, :D]) @ lk1) @ lk2   (folded: relu(x @ w1k) @ lk2)
    v = relu((x @ wkv[:, D:]) @ lv1) @ lv2
    per head: e = exp(q k^T + maskbias); out = (e @ v) / (e.sum + 1e-6)
    y = out @ wout + bout

Strategy (8 cores, one SPMD NEFF, NO collective):
  - queries: tokens sharded 512/core (cores 0-3 batch0, 4-7 batch1).
  - keys: masked keys compacted on host; EVERY core computes k/v for its
    batch's full active-key set (NB*128 slots) locally — redundant compute
    is far cheaper than the modeled AllGather (15us + 40GB/s).
  - all matmuls fp8e4m3 with DoubleRow (2x modeled PE throughput) except
    the output projection (wout bf16: fp8 weight quantization error passes
    straight to the output; score-side fp8 noise is attenuated by softmax
    averaging).
  - v-path mean-centering: h_v has positive mean (relu); subtracting a
    host-computed statistical mean per latent (fp8-snapped) removes the
    coherent component of the lv2-fp8 quantization error; the mean path
    v0 = c @ lv2 rides through the output bias in fp64 (exact since
    softmax weights sum to 1). K-path coherent errors cancel in softmax.
  - scores per head via DoubleRow on 32-partition quadrants
    (tile_position): head dims split 32+32 across the two DR planes.
  - denominator: 64 'ones' columns interleaved with v give a PE-broadcast
    denominator on psum partitions 64:127 (free), so normalize is one
    reciprocal + one multiply on DVE per head.
  - exp split between ACT (true exp, fp8 out) and DVE (Schraudolph uint8
    bit-trick -> fp8e4m3, bit-exact validated on HW) to balance engines.
"""

import sys

sys.path.insert(0, "/opt/trn_rl_repo")
import numpy as np
import ml_dtypes
import concourse.bass as bass
import concourse.mybir as mybir
import concourse.tile as tile
from concourse import bacc
from concourse.bass_utils import run_bass_kernel_spmd

DIM, LATENT, HEADS, HD = 1024, 512, 16, 64
B, N, NC, T = 2, 2048, 8, 512
SCALE = HD ** -0.5
BF16, F32, FP8 = mybir.dt.bfloat16, mybir.dt.float32, mybir.dt.float8e4
U8 = mybir.dt.uint8
NPBF16 = ml_dtypes.bfloat16
NPFP8 = ml_dtypes.float8_e4m3
DR = mybir.MatmulPerfMode.DoubleRow

LN2 = float(np.log(2.0))
C_SHIFT = 5 * LN2            # exp shift; e^-C folded via bias, 2^-5 exact
NEGB = -35.0                 # pad-kill bias
C8 = 0.0435                  # schraudolph tuning constant
A8 = 8.0 / LN2
A8S = A8 * SCALE / 2.0       # DVE schr multiplier on raw scores
KB2R = 8.0 * (7.0 - C8) - A8 * C_SHIFT   # schr bias, real keys

_cache: dict = {}
LAST_RESULTS = None


def _build(NB, FPB):
    """NB = key blocks of 128 per batch; FPB = first block containing pads
    (blocks < FPB use constant exp bias; blocks >= FPB use per-slot AP)."""
    NK = NB * 128
    Exp = mybir.ActivationFunctionType.Exp
    Relu = mybir.ActivationFunctionType.Relu
    Copy = mybir.ActivationFunctionType.Copy
    Ident = mybir.ActivationFunctionType.Identity
    Alu = mybir.AluOpType

    nc = bacc.Bacc("TRN2", target_bir_lowering=False, num_devices=NC)
    xq_d = nc.dram_tensor("xq", [DIM, T], FP8, kind="ExternalInput")
    xkv_d = nc.dram_tensor("xkv", [DIM, NK], FP8, kind="ExternalInput")
    wq_d = nc.dram_tensor("wq", [DIM, DIM], FP8, kind="ExternalInput")
    w1k_d = nc.dram_tensor("w1k", [DIM, LATENT], FP8, kind="ExternalInput")
    w1v_d = nc.dram_tensor("w1v", [DIM, LATENT], FP8, kind="ExternalInput")
    lk2_d = nc.dram_tensor("lk2", [LATENT, DIM], FP8, kind="ExternalInput")
    lv2_d = nc.dram_tensor("lv2", [LATENT, DIM], FP8, kind="ExternalInput")
    wout_d = nc.dram_tensor("wout", [DIM, DIM], BF16, kind="ExternalInput")
    bout_d = nc.dram_tensor("bout2", [128, 8], F32, kind="ExternalInput")
    kbt_d = nc.dram_tensor("kbt", [128, NB], F32, kind="ExternalInput")
    kbt2_d = nc.dram_tensor("kbt2", [128, NB], F32, kind="ExternalInput")
    cv_d = nc.dram_tensor("cv", [128, 4], F32, kind="ExternalInput")
    y_d = nc.dram_tensor("yT", [DIM, T], F32, kind="ExternalOutput")

    from contextlib import ExitStack
    with ExitStack() as ctx:
        tc = ctx.enter_context(tile.TileContext(nc))
        pool = lambda **kw: ctx.enter_context(tc.tile_pool(**kw))
        pw1 = pool(name="pw1", bufs=2)
        pl2 = pool(name="pl2", bufs=2)
        pwq = pool(name="pwq", bufs=1)
        pwo = pool(name="pwo", bufs=1)
        px = pool(name="px", bufs=1)
        ph = pool(name="ph", bufs=2)
        pkt = pool(name="pkt", bufs=1)
        pv = pool(name="pv", bufs=1)
        pqt = pool(name="pqt", bufs=1)
        patt = pool(name="patt", bufs=1)
        pe_ = pool(name="pe", bufs=6)
        pr = pool(name="pr", bufs=3)
        posb = pool(name="posb", bufs=3)
        psm = pool(name="psm", bufs=1)
        psA = pool(name="psA", bufs=3, space="PSUM")   # [128,1024] x3 = 12KB
        psN = pool(name="psN", bufs=2, space="PSUM")   # [128,512]  x2 =  4KB

        # ---------------- input DMAs ------------------------------------
        # split the critical first loads in halves so h matmuls start early
        w1k_sb = pw1.tile([128, 8 * LATENT], FP8, tag="w1")
        w1v_sb = pw1.tile([128, 8 * LATENT], FP8, tag="w1")
        xkv_sb = px.tile([128, 8 * NK], FP8, tag="xkv")
        cvt = psm.tile([128, 4], F32, tag="cv")
        xkv3v = xkv_sb[:].rearrange("p (d n) -> p d n", d=8)
        w1k3v = w1k_sb[:].rearrange("p (d l) -> p d l", d=8)
        w1v3v = w1v_sb[:].rearrange("p (d l) -> p d l", d=8)
        for hf in range(2):
            dd = slice(512 * hf, 512 * (hf + 1))
            nc.sync.dma_start(
                xkv3v[:, 4 * hf:4 * hf + 4],
                xkv_d.ap()[dd, :].rearrange("(d p) n -> p d n", p=128))
            nc.sync.dma_start(
                w1k3v[:, 4 * hf:4 * hf + 4],
                w1k_d.ap()[dd, :].rearrange("(d p) l -> p d l", p=128))
            nc.sync.dma_start(
                w1v3v[:, 4 * hf:4 * hf + 4],
                w1v_d.ap()[dd, :].rearrange("(d p) l -> p d l", p=128))
        nc.sync.dma_start(cvt[:], cv_d.ap())

        lk2_sb = pl2.tile([128, 4 * DIM], FP8, tag="l2")
        lv2_sb = pl2.tile([128, 4 * DIM], FP8, tag="l2")
        nc.sync.dma_start(
            lk2_sb[:].rearrange("p (l c) -> p l c", l=4),
            lk2_d.ap().rearrange("(l p) c -> p l c", p=128))
        nc.sync.dma_start(
            lv2_sb[:].rearrange("p (l c) -> p l c", l=4),
            lv2_d.ap().rearrange("(l p) c -> p l c", p=128))

        wq_sb = pwq.tile([128, 8 * DIM], FP8, tag="wq")
        xq_sb = px.tile([128, 8 * T], FP8, tag="xq")
        nc.sync.dma_start(
            wq_sb[:].rearrange("p (d c) -> p d c", d=8),
            wq_d.ap().rearrange("(d p) c -> p d c", p=128))
        nc.sync.dma_start(
            xq_sb[:].rearrange("p (d n) -> p d n", d=8),
            xq_d.ap().rearrange("(d p) n -> p d n", p=128))

        kbt = psm.tile([128, NB], F32, tag="kbt")
        kbt2 = psm.tile([128, NB], F32, tag="kbt2")
        nc.sync.dma_start(kbt[:], kbt_d.ap())
        nc.sync.dma_start(kbt2[:], kbt2_d.ap())

        wout_sb = pwo.tile([128, 8 * DIM], BF16, tag="wo")
        boutt = psm.tile([128, 8], F32, tag="bo")
        nc.sync.dma_start(
            wout_sb[:].rearrange("p (d c) -> p d c", d=8),
            wout_d.ap().rearrange("(d p) c -> p d c", p=128))
        nc.sync.dma_start(boutt[:], bout_d.ap())

        # 3D chunk-major views
        w1k3 = w1k_sb[:].rearrange("p (d l) -> p d l", d=8)
        w1v3 = w1v_sb[:].rearrange("p (d l) -> p d l", d=8)
        xkv3 = xkv_sb[:].rearrange("p (d n) -> p d n", d=8)
        lk23 = lk2_sb[:].rearrange("p (l c) -> p l c", l=4)
        lv23 = lv2_sb[:].rearrange("p (l c) -> p l c", l=4)
        wq3 = wq_sb[:].rearrange("p (d c) -> p d c", d=8)
        xq3 = xq_sb[:].rearrange("p (d n) -> p d n", d=8)

        hk_sb = ph.tile([128, 4 * NK], FP8, tag="h")
        hv_sb = ph.tile([128, 4 * NK], FP8, tag="h")
        hk3 = hk_sb[:].rearrange("p (l n) -> p l n", l=4)
        hv3 = hv_sb[:].rearrange("p (l n) -> p l n", l=4)
        kt_sb = pkt.tile([128, 8 * NK], FP8, tag="kt")
        v_sb = pv.tile([128, NB * 2048], FP8, tag="v")
        qt_sb = pqt.tile([128, 8 * T], FP8, tag="qt")
        att_sb = patt.tile([128, 8 * T], BF16, tag="att")

        KEY_CH = [(0, 1024)] + ([(1024, NK - 1024)] if NK > 1024 else [])

        # ones columns for the PE-broadcast denominator (Pool, idle engine)
        v4 = v_sb[:].rearrange("p (j h two d) -> p (j h) two d",
                               j=NB, h=HEADS, two=2)
        nc.gpsimd.memset(v4[:, :, 1, :], 1.0)

        def h_path_l(w13, dst3, is_v, l):
            if True:
                ps = psA.tile([128, 1024], F32, tag="big")
                pst = psN.tile([128, 512], F32, tag="nm")
                for g0 in (0, 512):
                    for dp in range(4):
                        nc.tensor.matmul(
                            ps[:, g0:g0 + 512],
                            w13[:, 2 * dp:2 * dp + 2, 128 * l:128 * (l + 1)],
                            xkv3[:, 2 * dp:2 * dp + 2, g0:g0 + 512],
                            start=(dp == 0), stop=(dp == 3), perf_mode=DR)
                if NK > 1024:
                    for dp in range(4):
                        nc.tensor.matmul(
                            pst[:, 0:NK - 1024],
                            w13[:, 2 * dp:2 * dp + 2, 128 * l:128 * (l + 1)],
                            xkv3[:, 2 * dp:2 * dp + 2, 1024:NK],
                            start=(dp == 0), stop=(dp == 3), perf_mode=DR)
                with nc.allow_low_precision(reason="fp8 latents"):
                    if is_v:
                        # (max(ps,0) - cv) on DVE; centering the v-latents
                        nc.vector.tensor_scalar(
                            dst3[:, l, 0:1024], ps[:], 0.0, cvt[:, l:l + 1],
                            Alu.max, Alu.subtract)
                        if NK > 1024:
                            nc.vector.tensor_scalar(
                                dst3[:, l, 1024:NK], pst[:, 0:NK - 1024],
                                0.0, cvt[:, l:l + 1], Alu.max, Alu.subtract)
                    else:
                        nc.scalar.activation(dst3[:, l, 0:1024], ps[:], Relu,
                                             scale=2.0 ** -5)
                        if NK > 1024:
                            nc.scalar.activation(dst3[:, l, 1024:NK],
                                                 pst[:, 0:NK - 1024], Relu,
                                                 scale=2.0 ** -5)

        for l in range(4):
            h_path_l(w1k3, hk3, False, l)
            h_path_l(w1v3, hv3, True, l)

        # ---------------- kT (8 chunks), v (NB blocks), q (4 pairs) ------
        def kt_chunk(c8):
            ps = psA.tile([128, 1024], F32, tag="big")
            pst = psN.tile([128, 512], F32, tag="nm")
            for g0 in (0, 512):
                for lp in range(2):
                    nc.tensor.matmul(
                        ps[:, g0:g0 + 512],
                        lk23[:, 2 * lp:2 * lp + 2, 128 * c8:128 * (c8 + 1)],
                        hk3[:, 2 * lp:2 * lp + 2, g0:g0 + 512],
                        start=(lp == 0), stop=(lp == 1), perf_mode=DR)
            if NK > 1024:
                for lp in range(2):
                    nc.tensor.matmul(
                        pst[:, 0:NK - 1024],
                        lk23[:, 2 * lp:2 * lp + 2, 128 * c8:128 * (c8 + 1)],
                        hk3[:, 2 * lp:2 * lp + 2, 1024:NK],
                        start=(lp == 0), stop=(lp == 1), perf_mode=DR)
            # kT evac on DVE (kv-phase engine balance: ACT has relu+v+q)
            with nc.allow_low_precision(reason="fp8 k"):
                nc.vector.tensor_scalar(
                    kt_sb[:, c8 * NK:c8 * NK + 1024], ps[:],
                    2.0 ** -4, None, Alu.mult)
                if NK > 1024:
                    nc.vector.tensor_scalar(
                        kt_sb[:, c8 * NK + 1024:(c8 + 1) * NK],
                        pst[:, 0:NK - 1024], 2.0 ** -4, None, Alu.mult)

        def v_block(j):
            ps = psA.tile([128, 1024], F32, tag="big")
            for ch in range(2):
                for lp in range(2):
                    nc.tensor.matmul(
                        ps[:, 512 * ch:512 * (ch + 1)],
                        hv3[:, 2 * lp:2 * lp + 2, 128 * j:128 * (j + 1)],
                        lv23[:, 2 * lp:2 * lp + 2, 512 * ch:512 * (ch + 1)],
                        start=(lp == 0), stop=(lp == 1), perf_mode=DR)
            dst = v_sb[:, j * 2048:(j + 1) * 2048] \
                .rearrange("p (h two d) -> p h two d", h=16, two=2)[:, :, 0, :]
            with nc.allow_low_precision(reason="fp8 v"):
                nc.scalar.activation(
                    dst, ps[:].rearrange("p (h d) -> p h d", h=16),
                    Copy, scale=2.0 ** -8)

        def q_pair(t):
            ps = psA.tile([128, 1024], F32, tag="big")
            for pl in range(2):
                for dp in range(4):
                    nc.tensor.matmul(
                        ps[:, 512 * pl:512 * (pl + 1)],
                        wq3[:, 2 * dp:2 * dp + 2,
                            (2 * t + pl) * 128:(2 * t + pl + 1) * 128],
                        xq3[:, 2 * dp:2 * dp + 2, :],
                        start=(dp == 0), stop=(dp == 3), perf_mode=DR)
            with nc.allow_low_precision(reason="fp8 q"):
                nc.scalar.activation(qt_sb[:, t * 1024:(t + 1) * 1024], ps[:],
                                     Copy)

        order = [("q", 0), ("q", 1), ("k", 0), ("q", 2), ("k", 1),
                 ("q", 3), ("k", 2), ("v", 0), ("k", 3), ("v", 1),
                 ("k", 4), ("v", 2), ("k", 5), ("v", 3), ("k", 6),
                 ("v", 4), ("k", 7), ("v", 5), ("v", 6), ("v", 7), ("v", 8)]
        order = [(k, i) for (k, i) in order if
                 (k != "v" or i < NB) and (k != "k" or i < 8)]
        for kind, i in order:
            (kt_chunk if kind == "k" else v_block if kind == "v" else q_pair)(i)

        def drain_kv(n):
            pass

        # ---------------- attention -------------------------------------
        # flat software pipeline across all (head, unit) pairs: scores run
        # two units ahead of exp; numer three behind; each head's normalize
        # is deferred until after the NEXT head's DVE exps so the
        # exp->numer->recip round trip never stalls DVE.  exp split: ACT
        # gets pad-free pairs 0,1,2 (const bias); DVE gets pair 3 + the
        # leftover block + reciprocal + normalize multiply.
        NPAIR = NB // 2
        NU = NPAIR + (1 if NB % 2 else 0)
        v3 = v_sb[:].rearrange("p (j x) -> p j x", j=NB)

        scs = {}
        es = {}
        nms = {}
        pending_norm = []

        def head_views(h):
            t, g = h // 4, h % 4
            kt3 = kt_sb[:, t * 2 * NK:(t + 1) * 2 * NK] \
                .rearrange("p (pl k) -> p pl k", pl=2)
            qt3 = qt_sb[:, t * 1024:(t + 1) * 1024] \
                .rearrange("p (pl n) -> p pl n", pl=2)
            return kt3, qt3[32 * g:32 * (g + 1), :, :], 32 * g

        def emit_sc(h, u):
            kt3, lhq, bp32 = head_views(h)
            if u < NPAIR:
                sc = psA.tile([128, 1024], F32, tag="big", name=f"sc{h}_{u}")
            else:
                # leftover unit lives in the psN ring so the psA ring
                # advances by 4 (not 5) per head: the next head's first sc
                # then reuses a slot freed by an early DVE exp, not ACT's
                # last one
                sc = psN.tile([128, 512], F32, tag="nm", name=f"sc{h}_{u}")
            scs[(h, u)] = sc
            nj = 2 if u < NPAIR else 1
            for half in range(nj):
                j = 2 * u + half
                nc.tensor.matmul(
                    sc[:, 512 * half:512 * (half + 1)],
                    kt3[bp32:bp32 + 32, :, 128 * j:128 * (j + 1)],
                    lhq, start=True, stop=True, perf_mode=DR,
                    tile_position=(bp32, 0))

        def emit_exp(h, u):
            sc = scs[(h, u)]
            if u < NPAIR:
                e = pe_.tile([128, 1024], FP8, tag="e")
                with nc.allow_low_precision(reason="fp8 softmax"):
                    if 2 * u + 1 < FPB and u < 3:
                        nc.scalar.activation(e[:], sc[:], Exp,
                                             bias=kbt[:, 0:1],
                                             scale=SCALE / 2)
                    else:
                        for half in range(2):
                            j = 2 * u + half
                            nc.vector.tensor_scalar(
                                e[:, 512 * half:512 * (half + 1)].bitcast(U8),
                                sc[:, 512 * half:512 * (half + 1)], A8S,
                                KB2R if j < FPB else kbt2[:, j:j + 1],
                                Alu.mult, Alu.add)
            else:
                j = 2 * NPAIR
                e = pe_.tile([128, 512], FP8, tag="e1")
                with nc.allow_low_precision(reason="fp8 softmax"):
                    if h % 2 == 0:
                        nc.scalar.activation(e[:], sc[:], Exp,
                                             bias=kbt[:, j:j + 1],
                                             scale=SCALE / 2)
                    else:
                        nc.vector.tensor_scalar(
                            e[:].bitcast(U8), sc[:], A8S,
                            KB2R if j < FPB else kbt2[:, j:j + 1],
                            Alu.mult, Alu.add)
            es[(h, u)] = e
            # emit the deferred normalize after this head's first DVE exp
            if u == 3 and pending_norm:
                pending_norm.pop(0)()

        def emit_nm(h, u):
            if u == 0:
                nms[h] = psN.tile([128, 512], F32, tag="nm", name=f"nm{h}")
            nm = nms[h]
            if u < NPAIR:
                nc.tensor.matmul(
                    nm[:], v3[:, 2 * u:2 * u + 2, 128 * h:128 * (h + 1)],
                    es[(h, u)][:].rearrange("p (two n) -> p two n", two=2),
                    start=(u == 0), stop=(u == NU - 1), perf_mode=DR)
            else:
                nc.tensor.matmul(
                    nm[:], v3[:, 2 * NPAIR, 128 * h:128 * (h + 1)],
                    es[(h, u)][:], start=False, stop=True,
                    skip_group_check=True)
            if u == NU - 1:
                def normalize(h=h, nm=nm):
                    rr = pr.tile([64, 512], F32, tag="r")
                    nc.vector.reciprocal(rr[:], nm[64:128, :])
                    with nc.allow_low_precision(reason="bf16 att"):
                        nc.vector.tensor_mul(
                            att_sb[64 * (h % 2):64 * (h % 2) + 64,
                                   (h // 2) * T:(h // 2 + 1) * T],
                            nm[0:64, :], rr[:])
                pending_norm.append(normalize)

        units = [(h, u) for h in range(HEADS) for u in range(NU)]
        LA_E, LA_N = 1, 3
        for i in range(len(units) + LA_N):
            if i < len(units):
                emit_sc(*units[i])
            if LA_E <= i and i - LA_E < len(units):
                emit_exp(*units[i - LA_E])
            drain_kv(1)
            if LA_N <= i and i - LA_N < len(units):
                emit_nm(*units[i - LA_N])
        while pending_norm:
            pending_norm.pop(0)()

        # ---------------- output projection ------------------------------
        for cb in range(8):
            ps = psA.tile([128, 1024], F32, tag="big")
            for c in range(8):
                nc.tensor.matmul(
                    ps[:, 0:512],
                    wout_sb[:, c * DIM + 128 * cb:c * DIM + 128 * (cb + 1)],
                    att_sb[:, c * T:(c + 1) * T],
                    start=(c == 0), stop=(c == 7))
            osb = posb.tile([128, 512], F32, tag="osb")
            nc.scalar.activation(osb[:], ps[:, 0:512], Ident,
                                 bias=boutt[:, cb:cb + 1])
            nc.sync.dma_start(y_d.ap()[128 * cb:128 * (cb + 1), :], osb[:])

    nc.compile()
    return nc


def _f8(x):
    return np.asarray(x, np.float32).astype(NPFP8)


def kernel(x, mask, wq, wkv, lk1, lk2, lv1, lv2, wout, bout, **kw):
    global LAST_RESULTS
    x = np.asarray(x, np.float32)
    mask = np.asarray(mask)
    wq = np.asarray(wq, np.float64)
    wkv = np.asarray(wkv, np.float64)
    lk1 = np.asarray(lk1, np.float64)
    lk2 = np.asarray(lk2, np.float64)
    lv1 = np.asarray(lv1, np.float64)
    lv2 = np.asarray(lv2, np.float64)
    wout = np.asarray(wout, np.float64)
    bout = np.asarray(bout, np.float64)

    act = [np.nonzero(np.asarray(mask[b]) == 1)[0] for b in range(B)]
    A = [len(a) for a in act]
    NB = max(1, (max(A) + 127) // 128)
    NK = NB * 128
    FPB = min(A) // 128          # first block that contains pad slots

    # column permutation for the DR-32 scores layout:
    # psum chunk (t,pl) partitions = [head 4t+g, dims 32pl..32pl+32]
    perm = np.array([64 * (4 * t + g) + 32 * pl + i
                     for t in range(4) for pl in range(2)
                     for g in range(4) for i in range(32)])

    w1k = wkv[:, :DIM] @ lk1
    w1v = wkv[:, DIM:] @ lv1
    w1k8 = _f8(32 * w1k)
    w1v8 = _f8(32 * w1v)
    lk28 = _f8(32 * lk2[:, perm])
    lv28 = _f8(32 * lv2)
    wq8 = _f8(wq[:, perm])
    woutb = np.asarray(wout / 4.0, np.float32).astype(NPBF16)

    # v-path centering: statistical mean of relu(w1v8 . x) per latent,
    # snapped to the fp8 grid so exact relu-zeros quantize exactly
    xr = float(np.sqrt((x.astype(np.float64) ** 2).mean()))
    colv = np.sqrt((w1v8.astype(np.float64) ** 2).sum(0)) * xr
    cv32 = _f8(0.39894228 * colv).astype(np.float64)       # scale-32 units
    v0 = (cv32 / 32.0) @ lv2                               # exact lv2
    bout2 = bout + v0 @ wout
    bout2_t = np.ascontiguousarray(
        bout2.reshape(8, 128).T.astype(np.float32))
    cv_t = np.ascontiguousarray(
        cv32.reshape(4, 128).T.astype(np.float32))

    # exp biases per key slot (per batch)
    kbt = np.full((B, NK), NEGB, np.float32)
    kbt2 = np.full((B, NK), KB2R + A8 * (NEGB + C_SHIFT), np.float32)
    for b in range(B):
        kbt[b, :A[b]] = -C_SHIFT
        kbt2[b, :A[b]] = KB2R
    kbt_t = [np.ascontiguousarray(kbt[b].reshape(NB, 128).T) for b in range(B)]
    kbt2_t = [np.ascontiguousarray(kbt2[b].reshape(NB, 128).T) for b in range(B)]

    key = (NB, FPB)
    if key not in _cache:
        _cache[key] = _build(NB, FPB)
    nc = _cache[key]

    x_flat = x.reshape(B * N, DIM)
    xkv_b = []
    for b in range(B):
        xkv = np.zeros((DIM, NK), NPFP8)
        xkv[:, :A[b]] = _f8(x_flat[b * N + act[b]].T)
        xkv_b.append(xkv)

    in_maps = []
    for c in range(NC):
        b = c // 4
        in_maps.append({
            "xq": np.ascontiguousarray(_f8(x_flat[c * T:(c + 1) * T].T)),
            "xkv": xkv_b[b],
            "wq": wq8, "w1k": w1k8, "w1v": w1v8, "lk2": lk28, "lv2": lv28,
            "wout": woutb, "bout2": bout2_t, "kbt": kbt_t[b],
            "kbt2": kbt2_t[b], "cv": cv_t,
        })

    res = run_bass_kernel_spmd(nc, in_maps, core_ids=list(range(NC)))
    LAST_RESULTS = res
    y = np.empty((B * N, DIM), np.float32)
    for c in range(NC):
        y[c * T:(c + 1) * T] = res.results[c]["yT"].T
    return y.reshape(B, N, DIM)


# revision 58
# speedup vs baseline: 2.0456x; 1.0025x over previous
"""Trainium2 Bass kernel for KeOps multi-head latent attention (v2).

Reference (B=2, N=2048, DIM=1024, LATENT=512, HEADS=16, HD=64):
    q = x @ wq * scale
    k = relu((x @ wkv[:, :D]) @ lk1) @ lk2   (folded: relu(x @ w1k) @ lk2)
    v = relu((x @ wkv[:, D:]) @ lv1) @ lv2
    per head: e = exp(q k^T + maskbias); out = (e @ v) / (e.sum + 1e-6)
    y = out @ wout + bout

Strategy (8 cores, one SPMD NEFF, NO collective):
  - queries: tokens sharded 512/core (cores 0-3 batch0, 4-7 batch1).
  - keys: masked keys compacted on host; EVERY core computes k/v for its
    batch's full active-key set (NB*128 slots) locally — redundant compute
    is far cheaper than the modeled AllGather (15us + 40GB/s).
  - all matmuls fp8e4m3 with DoubleRow (2x modeled PE throughput) except
    the output projection (wout bf16: fp8 weight quantization error passes
    straight to the output; score-side fp8 noise is attenuated by softmax
    averaging).
  - v-path mean-centering: h_v has positive mean (relu); subtracting a
    host-computed statistical mean per latent (fp8-snapped) removes the
    coherent component of the lv2-fp8 quantization error; the mean path
    v0 = c @ lv2 rides through the output bias in fp64 (exact since
    softmax weights sum to 1). K-path coherent errors cancel in softmax.
  - scores per head via DoubleRow on 32-partition quadrants
    (tile_position): head dims split 32+32 across the two DR planes.
  - denominator: 64 'ones' columns interleaved with v give a PE-broadcast
    denominator on psum partitions 64:127 (free), so normalize is one
    reciprocal + one multiply on DVE per head.
  - exp split between ACT (true exp, fp8 out) and DVE (Schraudolph uint8
    bit-trick -> fp8e4m3, bit-exact validated on HW) to balance engines.
"""

import sys

sys.path.insert(0, "/opt/trn_rl_repo")
import numpy as np
import ml_dtypes
import concourse.bass as bass
import concourse.mybir as mybir
import concourse.tile as tile
from concourse import bacc
from concourse.bass_utils import run_bass_kernel_spmd

DIM, LATENT, HEADS, HD = 1024, 512, 16, 64
B, N, NC, T = 2, 2048, 8, 512
SCALE = HD ** -0.5
BF16, F32, FP8 = mybir.dt.bfloat16, mybir.dt.float32, mybir.dt.float8e4
U8 = mybir.dt.uint8
NPBF16 = ml_dtypes.bfloat16
NPFP8 = ml_dtypes.float8_e4m3
DR = mybir.MatmulPerfMode.DoubleRow

LN2 = float(np.log(2.0))
C_SHIFT = 5 * LN2            # exp shift; e^-C folded via bias, 2^-5 exact
NEGB = -35.0                 # pad-kill bias
C8 = 0.0435                  # schraudolph tuning constant
A8 = 8.0 / LN2
A8S = A8 * SCALE / 2.0       # DVE schr multiplier on raw scores
KB2R = 8.0 * (7.0 - C8) - A8 * C_SHIFT   # schr bias, real keys

_cache: dict = {}
LAST_RESULTS = None


def _build(NB, FPB):
    """NB = key blocks of 128 per batch; FPB = first block containing pads
    (blocks < FPB use constant exp bias; blocks >= FPB use per-slot AP)."""
    NK = NB * 128
    Exp = mybir.ActivationFunctionType.Exp
    Relu = mybir.ActivationFunctionType.Relu
    Copy = mybir.ActivationFunctionType.Copy
    Ident = mybir.ActivationFunctionType.Identity
    Alu = mybir.AluOpType

    nc = bacc.Bacc("TRN2", target_bir_lowering=False, num_devices=NC)
    xq_d = nc.dram_tensor("xq", [DIM, T], FP8, kind="ExternalInput")
    xkv_d = nc.dram_tensor("xkv", [DIM, NK], FP8, kind="ExternalInput")
    wq_d = nc.dram_tensor("wq", [DIM, DIM], FP8, kind="ExternalInput")
    w1k_d = nc.dram_tensor("w1k", [DIM, LATENT], FP8, kind="ExternalInput")
    w1v_d = nc.dram_tensor("w1v", [DIM, LATENT], FP8, kind="ExternalInput")
    lk2_d = nc.dram_tensor("lk2", [LATENT, DIM], FP8, kind="ExternalInput")
    lv2_d = nc.dram_tensor("lv2", [LATENT, DIM], FP8, kind="ExternalInput")
    wout_d = nc.dram_tensor("wout", [DIM, DIM], BF16, kind="ExternalInput")
    bout_d = nc.dram_tensor("bout2", [128, 8], F32, kind="ExternalInput")
    kbt_d = nc.dram_tensor("kbt", [128, NB], F32, kind="ExternalInput")
    kbt2_d = nc.dram_tensor("kbt2", [128, NB], F32, kind="ExternalInput")
    cv_d = nc.dram_tensor("cv", [128, 4], F32, kind="ExternalInput")
    y_d = nc.dram_tensor("yT", [DIM, T], F32, kind="ExternalOutput")

    from contextlib import ExitStack
    with ExitStack() as ctx:
        tc = ctx.enter_context(tile.TileContext(nc))
        pool = lambda **kw: ctx.enter_context(tc.tile_pool(**kw))
        pw1 = pool(name="pw1", bufs=2)
        pl2 = pool(name="pl2", bufs=2)
        pwq = pool(name="pwq", bufs=1)
        pwo = pool(name="pwo", bufs=1)
        px = pool(name="px", bufs=1)
        ph = pool(name="ph", bufs=2)
        pkt = pool(name="pkt", bufs=1)
        pv = pool(name="pv", bufs=1)
        pqt = pool(name="pqt", bufs=1)
        patt = pool(name="patt", bufs=1)
        pe_ = pool(name="pe", bufs=6)
        pr = pool(name="pr", bufs=3)
        posb = pool(name="posb", bufs=3)
        psm = pool(name="psm", bufs=1)
        psA = pool(name="psA", bufs=3, space="PSUM")   # [128,1024] x3 = 12KB
        psN = pool(name="psN", bufs=2, space="PSUM")   # [128,512]  x2 =  4KB

        # ---------------- input DMAs ------------------------------------
        # split the critical first loads in halves so h matmuls start early
        w1k_sb = pw1.tile([128, 8 * LATENT], FP8, tag="w1")
        w1v_sb = pw1.tile([128, 8 * LATENT], FP8, tag="w1")
        xkv_sb = px.tile([128, 8 * NK], FP8, tag="xkv")
        cvt = psm.tile([128, 4], F32, tag="cv")
        xkv3v = xkv_sb[:].rearrange("p (d n) -> p d n", d=8)
        w1k3v = w1k_sb[:].rearrange("p (d l) -> p d l", d=8)
        w1v3v = w1v_sb[:].rearrange("p (d l) -> p d l", d=8)
        for hf in range(2):
            dd = slice(512 * hf, 512 * (hf + 1))
            nc.sync.dma_start(
                xkv3v[:, 4 * hf:4 * hf + 4],
                xkv_d.ap()[dd, :].rearrange("(d p) n -> p d n", p=128))
            nc.sync.dma_start(
                w1k3v[:, 4 * hf:4 * hf + 4],
                w1k_d.ap()[dd, :].rearrange("(d p) l -> p d l", p=128))
            nc.sync.dma_start(
                w1v3v[:, 4 * hf:4 * hf + 4],
                w1v_d.ap()[dd, :].rearrange("(d p) l -> p d l", p=128))
        nc.sync.dma_start(cvt[:], cv_d.ap())

        lk2_sb = pl2.tile([128, 4 * DIM], FP8, tag="l2")
        lv2_sb = pl2.tile([128, 4 * DIM], FP8, tag="l2")
        nc.sync.dma_start(
            lk2_sb[:].rearrange("p (l c) -> p l c", l=4),
            lk2_d.ap().rearrange("(l p) c -> p l c", p=128))
        nc.sync.dma_start(
            lv2_sb[:].rearrange("p (l c) -> p l c", l=4),
            lv2_d.ap().rearrange("(l p) c -> p l c", p=128))

        wq_sb = pwq.tile([128, 8 * DIM], FP8, tag="wq")
        xq_sb = px.tile([128, 8 * T], FP8, tag="xq")
        nc.sync.dma_start(
            wq_sb[:].rearrange("p (d c) -> p d c", d=8),
            wq_d.ap().rearrange("(d p) c -> p d c", p=128))
        nc.sync.dma_start(
            xq_sb[:].rearrange("p (d n) -> p d n", d=8),
            xq_d.ap().rearrange("(d p) n -> p d n", p=128))

        kbt = psm.tile([128, NB], F32, tag="kbt")
        kbt2 = psm.tile([128, NB], F32, tag="kbt2")
        nc.sync.dma_start(kbt[:], kbt_d.ap())
        nc.sync.dma_start(kbt2[:], kbt2_d.ap())

        wout_sb = pwo.tile([128, 8 * DIM], BF16, tag="wo")
        boutt = psm.tile([128, 8], F32, tag="bo")
        nc.sync.dma_start(
            wout_sb[:].rearrange("p (d c) -> p d c", d=8),
            wout_d.ap().rearrange("(d p) c -> p d c", p=128))
        nc.sync.dma_start(boutt[:], bout_d.ap())

        # 3D chunk-major views
        w1k3 = w1k_sb[:].rearrange("p (d l) -> p d l", d=8)
        w1v3 = w1v_sb[:].rearrange("p (d l) -> p d l", d=8)
        xkv3 = xkv_sb[:].rearrange("p (d n) -> p d n", d=8)
        lk23 = lk2_sb[:].rearrange("p (l c) -> p l c", l=4)
        lv23 = lv2_sb[:].rearrange("p (l c) -> p l c", l=4)
        wq3 = wq_sb[:].rearrange("p (d c) -> p d c", d=8)
        xq3 = xq_sb[:].rearrange("p (d n) -> p d n", d=8)

        hk_sb = ph.tile([128, 4 * NK], FP8, tag="h")
        hv_sb = ph.tile([128, 4 * NK], FP8, tag="h")
        hk3 = hk_sb[:].rearrange("p (l n) -> p l n", l=4)
        hv3 = hv_sb[:].rearrange("p (l n) -> p l n", l=4)
        kt_sb = pkt.tile([128, 8 * NK], FP8, tag="kt")
        v_sb = pv.tile([128, NB * 2048], FP8, tag="v")
        qt_sb = pqt.tile([128, 8 * T], FP8, tag="qt")
        att_sb = patt.tile([128, 8 * T], BF16, tag="att")

        KEY_CH = [(0, 1024)] + ([(1024, NK - 1024)] if NK > 1024 else [])

        # ones columns for the PE-broadcast denominator (Pool, idle engine)
        v4 = v_sb[:].rearrange("p (j h two d) -> p (j h) two d",
                               j=NB, h=HEADS, two=2)
        nc.gpsimd.memset(v4[:, :, 1, :], 1.0)

        def h_path_l(w13, dst3, is_v, l):
            if True:
                ps = psA.tile([128, 1024], F32, tag="big")
                pst = psN.tile([128, 512], F32, tag="nm")
                for g0 in (0, 512):
                    for dp in range(4):
                        nc.tensor.matmul(
                            ps[:, g0:g0 + 512],
                            w13[:, 2 * dp:2 * dp + 2, 128 * l:128 * (l + 1)],
                            xkv3[:, 2 * dp:2 * dp + 2, g0:g0 + 512],
                            start=(dp == 0), stop=(dp == 3), perf_mode=DR)
                if NK > 1024:
                    for dp in range(4):
                        nc.tensor.matmul(
                            pst[:, 0:NK - 1024],
                            w13[:, 2 * dp:2 * dp + 2, 128 * l:128 * (l + 1)],
                            xkv3[:, 2 * dp:2 * dp + 2, 1024:NK],
                            start=(dp == 0), stop=(dp == 3), perf_mode=DR)
                with nc.allow_low_precision(reason="fp8 latents"):
                    if is_v:
                        # (max(ps,0) - cv) on DVE; centering the v-latents
                        nc.vector.tensor_scalar(
                            dst3[:, l, 0:1024], ps[:], 0.0, cvt[:, l:l + 1],
                            Alu.max, Alu.subtract)
                        if NK > 1024:
                            nc.vector.tensor_scalar(
                                dst3[:, l, 1024:NK], pst[:, 0:NK - 1024],
                                0.0, cvt[:, l:l + 1], Alu.max, Alu.subtract)
                    else:
                        nc.scalar.activation(dst3[:, l, 0:1024], ps[:], Relu,
                                             scale=2.0 ** -5)
                        if NK > 1024:
                            nc.scalar.activation(dst3[:, l, 1024:NK],
                                                 pst[:, 0:NK - 1024], Relu,
                                                 scale=2.0 ** -5)

        for l in range(4):
            h_path_l(w1k3, hk3, False, l)
            h_path_l(w1v3, hv3, True, l)

        # ---------------- kT (8 chunks), v (NB blocks), q (4 pairs) ------
        def kt_chunk(c8):
            ps = psA.tile([128, 1024], F32, tag="big")
            pst = psN.tile([128, 512], F32, tag="nm")
            for g0 in (0, 512):
                for lp in range(2):
                    nc.tensor.matmul(
                        ps[:, g0:g0 + 512],
                        lk23[:, 2 * lp:2 * lp + 2, 128 * c8:128 * (c8 + 1)],
                        hk3[:, 2 * lp:2 * lp + 2, g0:g0 + 512],
                        start=(lp == 0), stop=(lp == 1), perf_mode=DR)
            if NK > 1024:
                for lp in range(2):
                    nc.tensor.matmul(
                        pst[:, 0:NK - 1024],
                        lk23[:, 2 * lp:2 * lp + 2, 128 * c8:128 * (c8 + 1)],
                        hk3[:, 2 * lp:2 * lp + 2, 1024:NK],
                        start=(lp == 0), stop=(lp == 1), perf_mode=DR)
            # kT evac on DVE (kv-phase engine balance: ACT has relu+v+q)
            with nc.allow_low_precision(reason="fp8 k"):
                nc.vector.tensor_scalar(
                    kt_sb[:, c8 * NK:c8 * NK + 1024], ps[:],
                    2.0 ** -4, None, Alu.mult)
                if NK > 1024:
                    nc.vector.tensor_scalar(
                        kt_sb[:, c8 * NK + 1024:(c8 + 1) * NK],
                        pst[:, 0:NK - 1024], 2.0 ** -4, None, Alu.mult)

        def v_block(j):
            ps = psA.tile([128, 1024], F32, tag="big")
            for ch in range(2):
                for lp in range(2):
                    nc.tensor.matmul(
                        ps[:, 512 * ch:512 * (ch + 1)],
                        hv3[:, 2 * lp:2 * lp + 2, 128 * j:128 * (j + 1)],
                        lv23[:, 2 * lp:2 * lp + 2, 512 * ch:512 * (ch + 1)],
                        start=(lp == 0), stop=(lp == 1), perf_mode=DR)
            dst = v_sb[:, j * 2048:(j + 1) * 2048] \
                .rearrange("p (h two d) -> p h two d", h=16, two=2)[:, :, 0, :]
            with nc.allow_low_precision(reason="fp8 v"):
                if j % 3 == 2:
                    nc.vector.tensor_scalar(
                        dst, ps[:].rearrange("p (h d) -> p h d", h=16),
                        2.0 ** -8, None, Alu.mult)
                else:
                    nc.scalar.activation(
                        dst, ps[:].rearrange("p (h d) -> p h d", h=16),
                        Copy, scale=2.0 ** -8)

        def q_pair(t):
            ps = psA.tile([128, 1024], F32, tag="big")
            for pl in range(2):
                for dp in range(4):
                    nc.tensor.matmul(
                        ps[:, 512 * pl:512 * (pl + 1)],
                        wq3[:, 2 * dp:2 * dp + 2,
                            (2 * t + pl) * 128:(2 * t + pl + 1) * 128],
                        xq3[:, 2 * dp:2 * dp + 2, :],
                        start=(dp == 0), stop=(dp == 3), perf_mode=DR)
            with nc.allow_low_precision(reason="fp8 q"):
                nc.scalar.activation(qt_sb[:, t * 1024:(t + 1) * 1024], ps[:],
                                     Copy)

        order = [("q", 0), ("q", 1), ("k", 0), ("q", 2), ("k", 1),
                 ("q", 3), ("k", 2), ("v", 0), ("k", 3), ("v", 1),
                 ("k", 4), ("v", 2), ("k", 5), ("v", 3), ("k", 6),
                 ("v", 4), ("k", 7), ("v", 5), ("v", 6), ("v", 7), ("v", 8)]
        order = [(k, i) for (k, i) in order if
                 (k != "v" or i < NB) and (k != "k" or i < 8)]
        for kind, i in order:
            (kt_chunk if kind == "k" else v_block if kind == "v" else q_pair)(i)

        def drain_kv(n):
            pass

        # ---------------- attention -------------------------------------
        # flat software pipeline across all (head, unit) pairs: scores run
        # two units ahead of exp; numer three behind; each head's normalize
        # is deferred until after the NEXT head's DVE exps so the
        # exp->numer->recip round trip never stalls DVE.  exp split: ACT
        # gets pad-free pairs 0,1,2 (const bias); DVE gets pair 3 + the
        # leftover block + reciprocal + normalize multiply.
        NPAIR = NB // 2
        NU = NPAIR + (1 if NB % 2 else 0)
        v3 = v_sb[:].rearrange("p (j x) -> p j x", j=NB)

        scs = {}
        es = {}
        nms = {}
        pending_norm = []

        def head_views(h):
            t, g = h // 4, h % 4
            kt3 = kt_sb[:, t * 2 * NK:(t + 1) * 2 * NK] \
                .rearrange("p (pl k) -> p pl k", pl=2)
            qt3 = qt_sb[:, t * 1024:(t + 1) * 1024] \
                .rearrange("p (pl n) -> p pl n", pl=2)
            return kt3, qt3[32 * g:32 * (g + 1), :, :], 32 * g

        def emit_sc(h, u):
            kt3, lhq, bp32 = head_views(h)
            if u < NPAIR:
                sc = psA.tile([128, 1024], F32, tag="big", name=f"sc{h}_{u}")
            else:
                # leftover unit lives in the psN ring so the psA ring
                # advances by 4 (not 5) per head: the next head's first sc
                # then reuses a slot freed by an early DVE exp, not ACT's
                # last one
                sc = psN.tile([128, 512], F32, tag="nm", name=f"sc{h}_{u}")
            scs[(h, u)] = sc
            nj = 2 if u < NPAIR else 1
            for half in range(nj):
                j = 2 * u + half
                nc.tensor.matmul(
                    sc[:, 512 * half:512 * (half + 1)],
                    kt3[bp32:bp32 + 32, :, 128 * j:128 * (j + 1)],
                    lhq, start=True, stop=True, perf_mode=DR,
                    tile_position=(bp32, 0))

        def emit_exp(h, u):
            sc = scs[(h, u)]
            if u < NPAIR:
                e = pe_.tile([128, 1024], FP8, tag="e")
                with nc.allow_low_precision(reason="fp8 softmax"):
                    if 2 * u + 1 < FPB and u < 3:
                        nc.scalar.activation(e[:], sc[:], Exp,
                                             bias=kbt[:, 0:1],
                                             scale=SCALE / 2)
                    else:
                        for half in range(2):
                            j = 2 * u + half
                            nc.vector.tensor_scalar(
                                e[:, 512 * half:512 * (half + 1)].bitcast(U8),
                                sc[:, 512 * half:512 * (half + 1)], A8S,
                                KB2R if j < FPB else kbt2[:, j:j + 1],
                                Alu.mult, Alu.add)
            else:
                j = 2 * NPAIR
                e = pe_.tile([128, 512], FP8, tag="e1")
                with nc.allow_low_precision(reason="fp8 softmax"):
                    if h % 2 == 0:
                        nc.scalar.activation(e[:], sc[:], Exp,
                                             bias=kbt[:, j:j + 1],
                                             scale=SCALE / 2)
                    else:
                        nc.vector.tensor_scalar(
                            e[:].bitcast(U8), sc[:], A8S,
                            KB2R if j < FPB else kbt2[:, j:j + 1],
                            Alu.mult, Alu.add)
            es[(h, u)] = e
            # emit the deferred normalize after this head's first DVE exp
            if u == 3 and pending_norm:
                pending_norm.pop(0)()

        def emit_nm(h, u):
            if u == 0:
                nms[h] = psN.tile([128, 512], F32, tag="nm", name=f"nm{h}")
            nm = nms[h]
            if u < NPAIR:
                nc.tensor.matmul(
                    nm[:], v3[:, 2 * u:2 * u + 2, 128 * h:128 * (h + 1)],
                    es[(h, u)][:].rearrange("p (two n) -> p two n", two=2),
                    start=(u == 0), stop=(u == NU - 1), perf_mode=DR)
            else:
                nc.tensor.matmul(
                    nm[:], v3[:, 2 * NPAIR, 128 * h:128 * (h + 1)],
                    es[(h, u)][:], start=False, stop=True,
                    skip_group_check=True)
            if u == NU - 1:
                def normalize(h=h, nm=nm):
                    rr = pr.tile([64, 512], F32, tag="r")
                    nc.vector.reciprocal(rr[:], nm[64:128, :])
                    with nc.allow_low_precision(reason="bf16 att"):
                        nc.vector.tensor_mul(
                            att_sb[64 * (h % 2):64 * (h % 2) + 64,
                                   (h // 2) * T:(h // 2 + 1) * T],
                            nm[0:64, :], rr[:])
                pending_norm.append(normalize)

        units = [(h, u) for h in range(HEADS) for u in range(NU)]
        LA_E, LA_N = 1, 3
        for i in range(len(units) + LA_N):
            if i < len(units):
                emit_sc(*units[i])
            if LA_E <= i and i - LA_E < len(units):
                emit_exp(*units[i - LA_E])
            drain_kv(1)
            if LA_N <= i and i - LA_N < len(units):
                emit_nm(*units[i - LA_N])
        while pending_norm:
            pending_norm.pop(0)()

        # ---------------- output projection ------------------------------
        for cb in range(8):
            ps = psA.tile([128, 1024], F32, tag="big")
            for c in range(8):
                nc.tensor.matmul(
                    ps[:, 0:512],
                    wout_sb[:, c * DIM + 128 * cb:c * DIM + 128 * (cb + 1)],
                    att_sb[:, c * T:(c + 1) * T],
                    start=(c == 0), stop=(c == 7))
            osb = posb.tile([128, 512], F32, tag="osb")
            nc.scalar.activation(osb[:], ps[:, 0:512], Ident,
                                 bias=boutt[:, cb:cb + 1])
            nc.sync.dma_start(y_d.ap()[128 * cb:128 * (cb + 1), :], osb[:])

    nc.compile()
    return nc


def _f8(x):
    return np.asarray(x, np.float32).astype(NPFP8)


def kernel(x, mask, wq, wkv, lk1, lk2, lv1, lv2, wout, bout, **kw):
    global LAST_RESULTS
    x = np.asarray(x, np.float32)
    mask = np.asarray(mask)
    wq = np.asarray(wq, np.float64)
    wkv = np.asarray(wkv, np.float64)
    lk1 = np.asarray(lk1, np.float64)
    lk2 = np.asarray(lk2, np.float64)
    lv1 = np.asarray(lv1, np.float64)
    lv2 = np.asarray(lv2, np.float64)
    wout = np.asarray(wout, np.float64)
    bout = np.asarray(bout, np.float64)

    act = [np.nonzero(np.asarray(mask[b]) == 1)[0] for b in range(B)]
    A = [len(a) for a in act]
    NB = max(1, (max(A) + 127) // 128)
    NK = NB * 128
    FPB = min(A) // 128          # first block that contains pad slots

    # column permutation for the DR-32 scores layout:
    # psum chunk (t,pl) partitions = [head 4t+g, dims 32pl..32pl+32]
    perm = np.array([64 * (4 * t + g) + 32 * pl + i
                     for t in range(4) for pl in range(2)
                     for g in range(4) for i in range(32)])

    w1k = wkv[:, :DIM] @ lk1
    w1v = wkv[:, DIM:] @ lv1
    w1k8 = _f8(32 * w1k)
    w1v8 = _f8(32 * w1v)
    lk28 = _f8(32 * lk2[:, perm])
    lv28 = _f8(32 * lv2)
    wq8 = _f8(wq[:, perm])
    woutb = np.asarray(wout / 4.0, np.float32).astype(NPBF16)

    # v-path centering: statistical mean of relu(w1v8 . x) per latent,
    # snapped to the fp8 grid so exact relu-zeros quantize exactly
    xr = float(np.sqrt((x.astype(np.float64) ** 2).mean()))
    colv = np.sqrt((w1v8.astype(np.float64) ** 2).sum(0)) * xr
    cv32 = _f8(0.39894228 * colv).astype(np.float64)       # scale-32 units
    v0 = (cv32 / 32.0) @ lv2                               # exact lv2
    bout2 = bout + v0 @ wout
    bout2_t = np.ascontiguousarray(
        bout2.reshape(8, 128).T.astype(np.float32))
    cv_t = np.ascontiguousarray(
        cv32.reshape(4, 128).T.astype(np.float32))

    # exp biases per key slot (per batch)
    kbt = np.full((B, NK), NEGB, np.float32)
    kbt2 = np.full((B, NK), KB2R + A8 * (NEGB + C_SHIFT), np.float32)
    for b in range(B):
        kbt[b, :A[b]] = -C_SHIFT
        kbt2[b, :A[b]] = KB2R
    kbt_t = [np.ascontiguousarray(kbt[b].reshape(NB, 128).T) for b in range(B)]
    kbt2_t = [np.ascontiguousarray(kbt2[b].reshape(NB, 128).T) for b in range(B)]

    key = (NB, FPB)
    if key not in _cache:
        _cache[key] = _build(NB, FPB)
    nc = _cache[key]

    x_flat = x.reshape(B * N, DIM)
    xkv_b = []
    for b in range(B):
        xkv = np.zeros((DIM, NK), NPFP8)
        xkv[:, :A[b]] = _f8(x_flat[b * N + act[b]].T)
        xkv_b.append(xkv)

    in_maps = []
    for c in range(NC):
        b = c // 4
        in_maps.append({
            "xq": np.ascontiguousarray(_f8(x_flat[c * T:(c + 1) * T].T)),
            "xkv": xkv_b[b],
            "wq": wq8, "w1k": w1k8, "w1v": w1v8, "lk2": lk28, "lv2": lv28,
            "wout": woutb, "bout2": bout2_t, "kbt": kbt_t[b],
            "kbt2": kbt2_t[b], "cv": cv_t,
        })

    res = run_bass_kernel_spmd(nc, in_maps, core_ids=list(range(NC)))
    LAST_RESULTS = res
    y = np.empty((B * N, DIM), np.float32)
    for c in range(NC):
        y[c * T:(c + 1) * T] = res.results[c]["yT"].T
    return y.reshape(B, N, DIM)


# revision 59
# speedup vs baseline: 2.0738x; 1.0138x over previous
"""Trainium2 Bass kernel for KeOps multi-head latent attention (v2).

Reference (B=2, N=2048, DIM=1024, LATENT=512, HEADS=16, HD=64):
    q = x @ wq * scale
    k = relu((x @ wkv[:, :D]) @ lk1) @ lk2   (folded: relu(x @ w1k) @ lk2)
    v = relu((x @ wkv[:, D:]) @ lv1) @ lv2
    per head: e = exp(q k^T + maskbias); out = (e @ v) / (e.sum + 1e-6)
    y = out @ wout + bout

Strategy (8 cores, one SPMD NEFF, NO collective):
  - queries: tokens sharded 512/core (cores 0-3 batch0, 4-7 batch1).
  - keys: masked keys compacted on host; EVERY core computes k/v for its
    batch's full active-key set (NB*128 slots) locally — redundant compute
    is far cheaper than the modeled AllGather (15us + 40GB/s).
  - all matmuls fp8e4m3 with DoubleRow (2x modeled PE throughput) except
    the output projection (wout bf16: fp8 weight quantization error passes
    straight to the output; score-side fp8 noise is attenuated by softmax
    averaging).
  - v-path mean-centering: h_v has positive mean (relu); subtracting a
    host-computed statistical mean per latent (fp8-snapped) removes the
    coherent component of the lv2-fp8 quantization error; the mean path
    v0 = c @ lv2 rides through the output bias in fp64 (exact since
    softmax weights sum to 1). K-path coherent errors cancel in softmax.
  - scores per head via DoubleRow on 32-partition quadrants
    (tile_position): head dims split 32+32 across the two DR planes.
  - denominator: 64 'ones' columns interleaved with v give a PE-broadcast
    denominator on psum partitions 64:127 (free), so normalize is one
    reciprocal + one multiply on DVE per head.
  - exp split between ACT (true exp, fp8 out) and DVE (Schraudolph uint8
    bit-trick -> fp8e4m3, bit-exact validated on HW) to balance engines.
"""

import sys

sys.path.insert(0, "/opt/trn_rl_repo")
import numpy as np
import ml_dtypes
import concourse.bass as bass
import concourse.mybir as mybir
import concourse.tile as tile
from concourse import bacc
from concourse.bass_utils import run_bass_kernel_spmd

DIM, LATENT, HEADS, HD = 1024, 512, 16, 64
B, N, NC, T = 2, 2048, 8, 512
SCALE = HD ** -0.5
BF16, F32, FP8 = mybir.dt.bfloat16, mybir.dt.float32, mybir.dt.float8e4
U8 = mybir.dt.uint8
NPBF16 = ml_dtypes.bfloat16
NPFP8 = ml_dtypes.float8_e4m3
DR = mybir.MatmulPerfMode.DoubleRow

LN2 = float(np.log(2.0))
C_SHIFT = 5 * LN2            # exp shift; e^-C folded via bias, 2^-5 exact
NEGB = -35.0                 # pad-kill bias
C8 = 0.0435                  # schraudolph tuning constant
A8 = 8.0 / LN2
A8S = A8 * SCALE / 2.0       # DVE schr multiplier on raw scores
KB2R = 8.0 * (7.0 - C8) - A8 * C_SHIFT   # schr bias, real keys

_cache: dict = {}
LAST_RESULTS = None


def _build(NB, FPB):
    """NB = key blocks of 128 per batch; FPB = first block containing pads
    (blocks < FPB use constant exp bias; blocks >= FPB use per-slot AP)."""
    NK = NB * 128
    Exp = mybir.ActivationFunctionType.Exp
    Relu = mybir.ActivationFunctionType.Relu
    Copy = mybir.ActivationFunctionType.Copy
    Ident = mybir.ActivationFunctionType.Identity
    Alu = mybir.AluOpType

    nc = bacc.Bacc("TRN2", target_bir_lowering=False, num_devices=NC)
    xq_d = nc.dram_tensor("xq", [DIM, T], FP8, kind="ExternalInput")
    xkv_d = nc.dram_tensor("xkv", [DIM, NK], FP8, kind="ExternalInput")
    wq_d = nc.dram_tensor("wq", [DIM, DIM], FP8, kind="ExternalInput")
    w1k_d = nc.dram_tensor("w1k", [DIM, LATENT], FP8, kind="ExternalInput")
    w1v_d = nc.dram_tensor("w1v", [DIM, LATENT], FP8, kind="ExternalInput")
    lk2_d = nc.dram_tensor("lk2", [LATENT, DIM], FP8, kind="ExternalInput")
    lv2_d = nc.dram_tensor("lv2", [LATENT, DIM], FP8, kind="ExternalInput")
    wout_d = nc.dram_tensor("wout", [DIM, DIM], FP8, kind="ExternalInput")
    bout_d = nc.dram_tensor("bout2", [128, 8], F32, kind="ExternalInput")
    kbt_d = nc.dram_tensor("kbt", [128, NB], F32, kind="ExternalInput")
    kbt2_d = nc.dram_tensor("kbt2", [128, NB], F32, kind="ExternalInput")
    cv_d = nc.dram_tensor("cv", [128, 4], F32, kind="ExternalInput")
    y_d = nc.dram_tensor("yT", [DIM, T], F32, kind="ExternalOutput")

    from contextlib import ExitStack
    with ExitStack() as ctx:
        tc = ctx.enter_context(tile.TileContext(nc))
        pool = lambda **kw: ctx.enter_context(tc.tile_pool(**kw))
        pw1 = pool(name="pw1", bufs=2)
        pl2 = pool(name="pl2", bufs=2)
        pwq = pool(name="pwq", bufs=1)
        pwo = pool(name="pwo", bufs=1)
        px = pool(name="px", bufs=1)
        ph = pool(name="ph", bufs=2)
        pkt = pool(name="pkt", bufs=1)
        pv = pool(name="pv", bufs=1)
        pqt = pool(name="pqt", bufs=1)
        patt = pool(name="patt", bufs=1)
        pe_ = pool(name="pe", bufs=6)
        pr = pool(name="pr", bufs=3)
        posb = pool(name="posb", bufs=3)
        psm = pool(name="psm", bufs=1)
        psA = pool(name="psA", bufs=3, space="PSUM")   # [128,1024] x3 = 12KB
        psN = pool(name="psN", bufs=2, space="PSUM")   # [128,512]  x2 =  4KB

        # ---------------- input DMAs ------------------------------------
        # split the critical first loads in halves so h matmuls start early
        w1k_sb = pw1.tile([128, 8 * LATENT], FP8, tag="w1")
        w1v_sb = pw1.tile([128, 8 * LATENT], FP8, tag="w1")
        xkv_sb = px.tile([128, 8 * NK], FP8, tag="xkv")
        cvt = psm.tile([128, 4], F32, tag="cv")
        xkv3v = xkv_sb[:].rearrange("p (d n) -> p d n", d=8)
        w1k3v = w1k_sb[:].rearrange("p (d l) -> p d l", d=8)
        w1v3v = w1v_sb[:].rearrange("p (d l) -> p d l", d=8)
        for hf in range(2):
            dd = slice(512 * hf, 512 * (hf + 1))
            nc.sync.dma_start(
                xkv3v[:, 4 * hf:4 * hf + 4],
                xkv_d.ap()[dd, :].rearrange("(d p) n -> p d n", p=128))
            nc.sync.dma_start(
                w1k3v[:, 4 * hf:4 * hf + 4],
                w1k_d.ap()[dd, :].rearrange("(d p) l -> p d l", p=128))
            nc.sync.dma_start(
                w1v3v[:, 4 * hf:4 * hf + 4],
                w1v_d.ap()[dd, :].rearrange("(d p) l -> p d l", p=128))
        nc.sync.dma_start(cvt[:], cv_d.ap())

        lk2_sb = pl2.tile([128, 4 * DIM], FP8, tag="l2")
        lv2_sb = pl2.tile([128, 4 * DIM], FP8, tag="l2")
        nc.sync.dma_start(
            lk2_sb[:].rearrange("p (l c) -> p l c", l=4),
            lk2_d.ap().rearrange("(l p) c -> p l c", p=128))
        nc.sync.dma_start(
            lv2_sb[:].rearrange("p (l c) -> p l c", l=4),
            lv2_d.ap().rearrange("(l p) c -> p l c", p=128))

        wq_sb = pwq.tile([128, 8 * DIM], FP8, tag="wq")
        xq_sb = px.tile([128, 8 * T], FP8, tag="xq")
        nc.sync.dma_start(
            wq_sb[:].rearrange("p (d c) -> p d c", d=8),
            wq_d.ap().rearrange("(d p) c -> p d c", p=128))
        nc.sync.dma_start(
            xq_sb[:].rearrange("p (d n) -> p d n", d=8),
            xq_d.ap().rearrange("(d p) n -> p d n", p=128))

        kbt = psm.tile([128, NB], F32, tag="kbt")
        kbt2 = psm.tile([128, NB], F32, tag="kbt2")
        nc.sync.dma_start(kbt[:], kbt_d.ap())
        nc.sync.dma_start(kbt2[:], kbt2_d.ap())

        wout_sb = pwo.tile([128, 8 * DIM], FP8, tag="wo")
        boutt = psm.tile([128, 8], F32, tag="bo")
        nc.sync.dma_start(
            wout_sb[:].rearrange("p (d c) -> p d c", d=8),
            wout_d.ap().rearrange("(d p) c -> p d c", p=128))
        nc.sync.dma_start(boutt[:], bout_d.ap())

        # 3D chunk-major views
        w1k3 = w1k_sb[:].rearrange("p (d l) -> p d l", d=8)
        w1v3 = w1v_sb[:].rearrange("p (d l) -> p d l", d=8)
        xkv3 = xkv_sb[:].rearrange("p (d n) -> p d n", d=8)
        lk23 = lk2_sb[:].rearrange("p (l c) -> p l c", l=4)
        lv23 = lv2_sb[:].rearrange("p (l c) -> p l c", l=4)
        wq3 = wq_sb[:].rearrange("p (d c) -> p d c", d=8)
        xq3 = xq_sb[:].rearrange("p (d n) -> p d n", d=8)

        hk_sb = ph.tile([128, 4 * NK], FP8, tag="h")
        hv_sb = ph.tile([128, 4 * NK], FP8, tag="h")
        hk3 = hk_sb[:].rearrange("p (l n) -> p l n", l=4)
        hv3 = hv_sb[:].rearrange("p (l n) -> p l n", l=4)
        kt_sb = pkt.tile([128, 8 * NK], FP8, tag="kt")
        v_sb = pv.tile([128, NB * 2048], FP8, tag="v")
        qt_sb = pqt.tile([128, 8 * T], FP8, tag="qt")
        att_sb = patt.tile([128, 8 * T], FP8, tag="att")

        KEY_CH = [(0, 1024)] + ([(1024, NK - 1024)] if NK > 1024 else [])

        # ones columns for the PE-broadcast denominator (Pool, idle engine)
        v4 = v_sb[:].rearrange("p (j h two d) -> p (j h) two d",
                               j=NB, h=HEADS, two=2)
        nc.gpsimd.memset(v4[:, :, 1, :], 1.0)

        def h_path_l(w13, dst3, is_v, l):
            if True:
                ps = psA.tile([128, 1024], F32, tag="big")
                pst = psN.tile([128, 512], F32, tag="nm")
                for g0 in (0, 512):
                    for dp in range(4):
                        nc.tensor.matmul(
                            ps[:, g0:g0 + 512],
                            w13[:, 2 * dp:2 * dp + 2, 128 * l:128 * (l + 1)],
                            xkv3[:, 2 * dp:2 * dp + 2, g0:g0 + 512],
                            start=(dp == 0), stop=(dp == 3), perf_mode=DR)
                if NK > 1024:
                    for dp in range(4):
                        nc.tensor.matmul(
                            pst[:, 0:NK - 1024],
                            w13[:, 2 * dp:2 * dp + 2, 128 * l:128 * (l + 1)],
                            xkv3[:, 2 * dp:2 * dp + 2, 1024:NK],
                            start=(dp == 0), stop=(dp == 3), perf_mode=DR)
                with nc.allow_low_precision(reason="fp8 latents"):
                    if is_v:
                        # (max(ps,0) - cv) on DVE; centering the v-latents
                        nc.vector.tensor_scalar(
                            dst3[:, l, 0:1024], ps[:], 0.0, cvt[:, l:l + 1],
                            Alu.max, Alu.subtract)
                        if NK > 1024:
                            nc.vector.tensor_scalar(
                                dst3[:, l, 1024:NK], pst[:, 0:NK - 1024],
                                0.0, cvt[:, l:l + 1], Alu.max, Alu.subtract)
                    else:
                        nc.scalar.activation(dst3[:, l, 0:1024], ps[:], Relu,
                                             scale=2.0 ** -5)
                        if NK > 1024:
                            nc.scalar.activation(dst3[:, l, 1024:NK],
                                                 pst[:, 0:NK - 1024], Relu,
                                                 scale=2.0 ** -5)

        for l in range(4):
            h_path_l(w1k3, hk3, False, l)
            h_path_l(w1v3, hv3, True, l)

        # ---------------- kT (8 chunks), v (NB blocks), q (4 pairs) ------
        def kt_chunk(c8):
            ps = psA.tile([128, 1024], F32, tag="big")
            pst = psN.tile([128, 512], F32, tag="nm")
            for g0 in (0, 512):
                for lp in range(2):
                    nc.tensor.matmul(
                        ps[:, g0:g0 + 512],
                        lk23[:, 2 * lp:2 * lp + 2, 128 * c8:128 * (c8 + 1)],
                        hk3[:, 2 * lp:2 * lp + 2, g0:g0 + 512],
                        start=(lp == 0), stop=(lp == 1), perf_mode=DR)
            if NK > 1024:
                for lp in range(2):
                    nc.tensor.matmul(
                        pst[:, 0:NK - 1024],
                        lk23[:, 2 * lp:2 * lp + 2, 128 * c8:128 * (c8 + 1)],
                        hk3[:, 2 * lp:2 * lp + 2, 1024:NK],
                        start=(lp == 0), stop=(lp == 1), perf_mode=DR)
            # kT evac on DVE (kv-phase engine balance: ACT has relu+v+q)
            with nc.allow_low_precision(reason="fp8 k"):
                nc.vector.tensor_scalar(
                    kt_sb[:, c8 * NK:c8 * NK + 1024], ps[:],
                    2.0 ** -4, None, Alu.mult)
                if NK > 1024:
                    nc.vector.tensor_scalar(
                        kt_sb[:, c8 * NK + 1024:(c8 + 1) * NK],
                        pst[:, 0:NK - 1024], 2.0 ** -4, None, Alu.mult)

        def v_block(j):
            ps = psA.tile([128, 1024], F32, tag="big")
            for ch in range(2):
                for lp in range(2):
                    nc.tensor.matmul(
                        ps[:, 512 * ch:512 * (ch + 1)],
                        hv3[:, 2 * lp:2 * lp + 2, 128 * j:128 * (j + 1)],
                        lv23[:, 2 * lp:2 * lp + 2, 512 * ch:512 * (ch + 1)],
                        start=(lp == 0), stop=(lp == 1), perf_mode=DR)
            dst = v_sb[:, j * 2048:(j + 1) * 2048] \
                .rearrange("p (h two d) -> p h two d", h=16, two=2)[:, :, 0, :]
            with nc.allow_low_precision(reason="fp8 v"):
                if j % 3 == 2:
                    nc.vector.tensor_scalar(
                        dst, ps[:].rearrange("p (h d) -> p h d", h=16),
                        2.0 ** -8, None, Alu.mult)
                else:
                    nc.scalar.activation(
                        dst, ps[:].rearrange("p (h d) -> p h d", h=16),
                        Copy, scale=2.0 ** -8)

        def q_pair(t):
            ps = psA.tile([128, 1024], F32, tag="big")
            for pl in range(2):
                for dp in range(4):
                    nc.tensor.matmul(
                        ps[:, 512 * pl:512 * (pl + 1)],
                        wq3[:, 2 * dp:2 * dp + 2,
                            (2 * t + pl) * 128:(2 * t + pl + 1) * 128],
                        xq3[:, 2 * dp:2 * dp + 2, :],
                        start=(dp == 0), stop=(dp == 3), perf_mode=DR)
            with nc.allow_low_precision(reason="fp8 q"):
                nc.scalar.activation(qt_sb[:, t * 1024:(t + 1) * 1024], ps[:],
                                     Copy)

        order = [("q", 0), ("q", 1), ("k", 0), ("q", 2), ("k", 1),
                 ("q", 3), ("k", 2), ("v", 0), ("k", 3), ("v", 1),
                 ("k", 4), ("v", 2), ("k", 5), ("v", 3), ("k", 6),
                 ("v", 4), ("k", 7), ("v", 5), ("v", 6), ("v", 7), ("v", 8)]
        order = [(k, i) for (k, i) in order if
                 (k != "v" or i < NB) and (k != "k" or i < 8)]
        for kind, i in order:
            (kt_chunk if kind == "k" else v_block if kind == "v" else q_pair)(i)

        def drain_kv(n):
            pass

        # ---------------- attention -------------------------------------
        # flat software pipeline across all (head, unit) pairs: scores run
        # two units ahead of exp; numer three behind; each head's normalize
        # is deferred until after the NEXT head's DVE exps so the
        # exp->numer->recip round trip never stalls DVE.  exp split: ACT
        # gets pad-free pairs 0,1,2 (const bias); DVE gets pair 3 + the
        # leftover block + reciprocal + normalize multiply.
        NPAIR = NB // 2
        NU = NPAIR + (1 if NB % 2 else 0)
        v3 = v_sb[:].rearrange("p (j x) -> p j x", j=NB)

        scs = {}
        es = {}
        nms = {}
        pending_norm = []

        def head_views(h):
            t, g = h // 4, h % 4
            kt3 = kt_sb[:, t * 2 * NK:(t + 1) * 2 * NK] \
                .rearrange("p (pl k) -> p pl k", pl=2)
            qt3 = qt_sb[:, t * 1024:(t + 1) * 1024] \
                .rearrange("p (pl n) -> p pl n", pl=2)
            return kt3, qt3[32 * g:32 * (g + 1), :, :], 32 * g

        def emit_sc(h, u):
            kt3, lhq, bp32 = head_views(h)
            if u < NPAIR:
                sc = psA.tile([128, 1024], F32, tag="big", name=f"sc{h}_{u}")
            else:
                # leftover unit lives in the psN ring so the psA ring
                # advances by 4 (not 5) per head: the next head's first sc
                # then reuses a slot freed by an early DVE exp, not ACT's
                # last one
                sc = psN.tile([128, 512], F32, tag="nm", name=f"sc{h}_{u}")
            scs[(h, u)] = sc
            nj = 2 if u < NPAIR else 1
            for half in range(nj):
                j = 2 * u + half
                nc.tensor.matmul(
                    sc[:, 512 * half:512 * (half + 1)],
                    kt3[bp32:bp32 + 32, :, 128 * j:128 * (j + 1)],
                    lhq, start=True, stop=True, perf_mode=DR,
                    tile_position=(bp32, 0))

        def emit_exp(h, u):
            sc = scs[(h, u)]
            if u < NPAIR:
                e = pe_.tile([128, 1024], FP8, tag="e")
                with nc.allow_low_precision(reason="fp8 softmax"):
                    if 2 * u + 1 < FPB and u < 3:
                        nc.scalar.activation(e[:], sc[:], Exp,
                                             bias=kbt[:, 0:1],
                                             scale=SCALE / 2)
                    else:
                        for half in range(2):
                            j = 2 * u + half
                            nc.vector.tensor_scalar(
                                e[:, 512 * half:512 * (half + 1)].bitcast(U8),
                                sc[:, 512 * half:512 * (half + 1)], A8S,
                                KB2R if j < FPB else kbt2[:, j:j + 1],
                                Alu.mult, Alu.add)
            else:
                j = 2 * NPAIR
                e = pe_.tile([128, 512], FP8, tag="e1")
                with nc.allow_low_precision(reason="fp8 softmax"):
                    if h % 2 == 0:
                        nc.scalar.activation(e[:], sc[:], Exp,
                                             bias=kbt[:, j:j + 1],
                                             scale=SCALE / 2)
                    else:
                        nc.vector.tensor_scalar(
                            e[:].bitcast(U8), sc[:], A8S,
                            KB2R if j < FPB else kbt2[:, j:j + 1],
                            Alu.mult, Alu.add)
            es[(h, u)] = e
            # emit the deferred normalize after this head's first DVE exp
            if u == 3 and pending_norm:
                pending_norm.pop(0)()

        def emit_nm(h, u):
            if u == 0:
                nms[h] = psN.tile([128, 512], F32, tag="nm", name=f"nm{h}")
            nm = nms[h]
            if u < NPAIR:
                nc.tensor.matmul(
                    nm[:], v3[:, 2 * u:2 * u + 2, 128 * h:128 * (h + 1)],
                    es[(h, u)][:].rearrange("p (two n) -> p two n", two=2),
                    start=(u == 0), stop=(u == NU - 1), perf_mode=DR)
            else:
                nc.tensor.matmul(
                    nm[:], v3[:, 2 * NPAIR, 128 * h:128 * (h + 1)],
                    es[(h, u)][:], start=False, stop=True,
                    skip_group_check=True)
            if u == NU - 1:
                def normalize(h=h, nm=nm):
                    rr = pr.tile([64, 512], F32, tag="r")
                    nc.vector.reciprocal(rr[:], nm[64:128, :])
                    with nc.allow_low_precision(reason="bf16 att"):
                        nc.vector.tensor_mul(
                            att_sb[64 * (h % 2):64 * (h % 2) + 64,
                                   (h // 2) * T:(h // 2 + 1) * T],
                            nm[0:64, :], rr[:])
                pending_norm.append(normalize)

        units = [(h, u) for h in range(HEADS) for u in range(NU)]
        LA_E, LA_N = 1, 3
        for i in range(len(units) + LA_N):
            if i < len(units):
                emit_sc(*units[i])
            if LA_E <= i and i - LA_E < len(units):
                emit_exp(*units[i - LA_E])
            drain_kv(1)
            if LA_N <= i and i - LA_N < len(units):
                emit_nm(*units[i - LA_N])
        while pending_norm:
            pending_norm.pop(0)()

        # ---------------- output projection ------------------------------
        wo3 = wout_sb[:].rearrange("p (d c) -> p d c", d=8)
        att3 = att_sb[:].rearrange("p (c n) -> p c n", c=8)
        for cb in range(8):
            ps = psA.tile([128, 1024], F32, tag="big")
            for c2 in range(4):
                nc.tensor.matmul(
                    ps[:, 0:512],
                    wo3[:, 2 * c2:2 * c2 + 2, 128 * cb:128 * (cb + 1)],
                    att3[:, 2 * c2:2 * c2 + 2, :],
                    start=(c2 == 0), stop=(c2 == 3), perf_mode=DR)
            osb = posb.tile([128, 512], F32, tag="osb")
            nc.scalar.activation(osb[:], ps[:, 0:512], Ident,
                                 scale=2.0 ** -7, bias=boutt[:, cb:cb + 1])
            nc.sync.dma_start(y_d.ap()[128 * cb:128 * (cb + 1), :], osb[:])

    nc.compile()
    return nc


def _f8(x):
    return np.asarray(x, np.float32).astype(NPFP8)


def kernel(x, mask, wq, wkv, lk1, lk2, lv1, lv2, wout, bout, **kw):
    global LAST_RESULTS
    x = np.asarray(x, np.float32)
    mask = np.asarray(mask)
    wq = np.asarray(wq, np.float64)
    wkv = np.asarray(wkv, np.float64)
    lk1 = np.asarray(lk1, np.float64)
    lk2 = np.asarray(lk2, np.float64)
    lv1 = np.asarray(lv1, np.float64)
    lv2 = np.asarray(lv2, np.float64)
    wout = np.asarray(wout, np.float64)
    bout = np.asarray(bout, np.float64)

    act = [np.nonzero(np.asarray(mask[b]) == 1)[0] for b in range(B)]
    A = [len(a) for a in act]
    NB = max(1, (max(A) + 127) // 128)
    NK = NB * 128
    FPB = min(A) // 128          # first block that contains pad slots

    # column permutation for the DR-32 scores layout:
    # psum chunk (t,pl) partitions = [head 4t+g, dims 32pl..32pl+32]
    perm = np.array([64 * (4 * t + g) + 32 * pl + i
                     for t in range(4) for pl in range(2)
                     for g in range(4) for i in range(32)])

    w1k = wkv[:, :DIM] @ lk1
    w1v = wkv[:, DIM:] @ lv1
    w1k8 = _f8(32 * w1k)
    w1v8 = _f8(32 * w1v)
    lk28 = _f8(32 * lk2[:, perm])
    lv28 = _f8(32 * lv2)
    wq8 = _f8(wq[:, perm])
    woutb = _f8(32.0 * wout)

    # v-path centering: statistical mean of relu(w1v8 . x) per latent,
    # snapped to the fp8 grid so exact relu-zeros quantize exactly
    xr = float(np.sqrt((x.astype(np.float64) ** 2).mean()))
    colv = np.sqrt((w1v8.astype(np.float64) ** 2).sum(0)) * xr
    cv32 = _f8(0.39894228 * colv).astype(np.float64)       # scale-32 units
    v0 = (cv32 / 32.0) @ lv2                               # exact lv2
    bout2 = bout + v0 @ wout
    bout2_t = np.ascontiguousarray(
        bout2.reshape(8, 128).T.astype(np.float32))
    cv_t = np.ascontiguousarray(
        cv32.reshape(4, 128).T.astype(np.float32))

    # exp biases per key slot (per batch)
    kbt = np.full((B, NK), NEGB, np.float32)
    kbt2 = np.full((B, NK), KB2R + A8 * (NEGB + C_SHIFT), np.float32)
    for b in range(B):
        kbt[b, :A[b]] = -C_SHIFT
        kbt2[b, :A[b]] = KB2R
    kbt_t = [np.ascontiguousarray(kbt[b].reshape(NB, 128).T) for b in range(B)]
    kbt2_t = [np.ascontiguousarray(kbt2[b].reshape(NB, 128).T) for b in range(B)]

    key = (NB, FPB)
    if key not in _cache:
        _cache[key] = _build(NB, FPB)
    nc = _cache[key]

    x_flat = x.reshape(B * N, DIM)
    xkv_b = []
    for b in range(B):
        xkv = np.zeros((DIM, NK), NPFP8)
        xkv[:, :A[b]] = _f8(x_flat[b * N + act[b]].T)
        xkv_b.append(xkv)

    in_maps = []
    for c in range(NC):
        b = c // 4
        in_maps.append({
            "xq": np.ascontiguousarray(_f8(x_flat[c * T:(c + 1) * T].T)),
            "xkv": xkv_b[b],
            "wq": wq8, "w1k": w1k8, "w1v": w1v8, "lk2": lk28, "lv2": lv28,
            "wout": woutb, "bout2": bout2_t, "kbt": kbt_t[b],
            "kbt2": kbt2_t[b], "cv": cv_t,
        })

    res = run_bass_kernel_spmd(nc, in_maps, core_ids=list(range(NC)))
    LAST_RESULTS = res
    y = np.empty((B * N, DIM), np.float32)
    for c in range(NC):
        y[c * T:(c + 1) * T] = res.results[c]["yT"].T
    return y.reshape(B, N, DIM)


# revision 81
# speedup vs baseline: 2.2051x; 1.0633x over previous
"""Trainium2 Bass kernel for KeOps multi-head latent attention (v2).

Reference (B=2, N=2048, DIM=1024, LATENT=512, HEADS=16, HD=64):
    q = x @ wq * scale
    k = relu((x @ wkv[:, :D]) @ lk1) @ lk2   (folded: relu(x @ w1k) @ lk2)
    v = relu((x @ wkv[:, D:]) @ lv1) @ lv2
    per head: e = exp(q k^T + maskbias); out = (e @ v) / (e.sum + 1e-6)
    y = out @ wout + bout

Strategy (8 cores, one SPMD NEFF, NO collective):
  - queries: tokens sharded 512/core (cores 0-3 batch0, 4-7 batch1).
  - keys: masked keys compacted on host; EVERY core computes k/v for its
    batch's full active-key set (NB*128 slots) locally — redundant compute
    is far cheaper than the modeled AllGather (15us + 40GB/s).
  - all matmuls fp8e4m3 with DoubleRow (2x modeled PE throughput),
    including the output projection: v-path mean-centering makes att/wout
    fp8 affordable (the mean path rides the exact fp64 output bias).
  - v-path mean-centering: h_v has positive mean (relu); subtracting a
    host-computed statistical mean per latent (fp8-snapped) removes the
    coherent component of the lv2-fp8 quantization error; the mean path
    v0 = c @ lv2 rides through the output bias in fp64 (exact since
    softmax weights sum to 1). K-path coherent errors cancel in softmax.
  - scores per head via DoubleRow on 32-partition quadrants
    (tile_position): head dims split 32+32 across the two DR planes.
  - denominator: 64 'ones' columns interleaved with v give a PE-broadcast
    denominator on psum partitions 64:127 (free), so normalize is one
    reciprocal + one multiply on DVE per head.
  - exp split between ACT (true exp, fp8 out) and DVE (Schraudolph uint8
    bit-trick -> fp8e4m3, bit-exact validated on HW) to balance engines.
"""

import sys

sys.path.insert(0, "/opt/trn_rl_repo")
import numpy as np
import ml_dtypes
import concourse.bass as bass
import concourse.mybir as mybir
import concourse.tile as tile
from concourse import bacc
from concourse.bass_utils import run_bass_kernel_spmd

DIM, LATENT, HEADS, HD = 1024, 512, 16, 64
B, N, NC, T = 2, 2048, 8, 512
SCALE = HD ** -0.5
BF16, F32, FP8 = mybir.dt.bfloat16, mybir.dt.float32, mybir.dt.float8e4
U8 = mybir.dt.uint8
NPBF16 = ml_dtypes.bfloat16
NPFP8 = ml_dtypes.float8_e4m3
DR = mybir.MatmulPerfMode.DoubleRow

LN2 = float(np.log(2.0))
C_SHIFT = 5 * LN2            # exp shift; e^-C folded via bias, 2^-5 exact
NEGB = -35.0                 # pad-kill bias
C8 = 0.0435                  # schraudolph tuning constant
A8 = 8.0 / LN2
A8S = A8 * SCALE / 2.0       # DVE schr multiplier on raw scores
KB2R = 8.0 * (7.0 - C8) - A8 * C_SHIFT   # schr bias, real keys

_cache: dict = {}
LAST_RESULTS = None


def _build(NB, FPB):
    """NB = key blocks of 128 per batch; FPB = first block containing pads
    (blocks < FPB use constant exp bias; blocks >= FPB use per-slot AP)."""
    NK = NB * 128
    Exp = mybir.ActivationFunctionType.Exp
    Relu = mybir.ActivationFunctionType.Relu
    Copy = mybir.ActivationFunctionType.Copy
    Ident = mybir.ActivationFunctionType.Identity
    Alu = mybir.AluOpType

    nc = bacc.Bacc("TRN2", target_bir_lowering=False, num_devices=NC)
    xq_d = nc.dram_tensor("xq", [DIM, T], FP8, kind="ExternalInput")
    xkv_d = nc.dram_tensor("xkv", [DIM, NK], FP8, kind="ExternalInput")
    wq_d = nc.dram_tensor("wq", [DIM, DIM], FP8, kind="ExternalInput")
    w1k_d = nc.dram_tensor("w1k", [DIM, LATENT], FP8, kind="ExternalInput")
    w1v_d = nc.dram_tensor("w1v", [DIM, LATENT], FP8, kind="ExternalInput")
    lk2_d = nc.dram_tensor("lk2", [LATENT, DIM], FP8, kind="ExternalInput")
    lv2_d = nc.dram_tensor("lv2", [LATENT, DIM], FP8, kind="ExternalInput")
    wout_d = nc.dram_tensor("wout", [DIM, DIM], FP8, kind="ExternalInput")
    bout_d = nc.dram_tensor("bout2", [128, 8], F32, kind="ExternalInput")
    kbt_d = nc.dram_tensor("kbt", [128, NB], F32, kind="ExternalInput")
    kbt2_d = nc.dram_tensor("kbt2", [128, NB], F32, kind="ExternalInput")
    cv_d = nc.dram_tensor("cv", [128, 4], F32, kind="ExternalInput")
    y_d = nc.dram_tensor("yT", [DIM, T], BF16, kind="ExternalOutput")

    from contextlib import ExitStack
    with ExitStack() as ctx:
        tc = ctx.enter_context(tile.TileContext(nc))
        pool = lambda **kw: ctx.enter_context(tc.tile_pool(**kw))
        pw1 = pool(name="pw1", bufs=2)
        pl2 = pool(name="pl2", bufs=2)
        pwq = pool(name="pwq", bufs=1)
        pwo = pool(name="pwo", bufs=1)
        px = pool(name="px", bufs=1)
        ph = pool(name="ph", bufs=2)
        pkt = pool(name="pkt", bufs=1)
        pv = pool(name="pv", bufs=1)
        pqt = pool(name="pqt", bufs=1)
        patt = pool(name="patt", bufs=1)
        pe_ = pool(name="pe", bufs=8)
        pr = pool(name="pr", bufs=3)
        posb = pool(name="posb", bufs=5)
        psm = pool(name="psm", bufs=1)
        psA = pool(name="psA", bufs=3, space="PSUM")
        psN = pool(name="psN", bufs=2, space="PSUM")

        # ---------------- input DMAs ------------------------------------
        # split the critical first loads in halves so h matmuls start early
        w1k_sb = pw1.tile([128, 8 * LATENT], FP8, tag="w1")
        w1v_sb = pw1.tile([128, 8 * LATENT], FP8, tag="w1")
        xkv_sb = px.tile([128, 8 * NK], FP8, tag="xkv")
        cvt = psm.tile([128, 4], F32, tag="cv")
        xkv3v = xkv_sb[:].rearrange("p (d n) -> p d n", d=8)
        w1k3v = w1k_sb[:].rearrange("p (d l) -> p d l", d=8)
        w1v3v = w1v_sb[:].rearrange("p (d l) -> p d l", d=8)
        for hf in range(2):
            dd = slice(512 * hf, 512 * (hf + 1))
            nc.sync.dma_start(
                xkv3v[:, 4 * hf:4 * hf + 4],
                xkv_d.ap()[dd, :].rearrange("(d p) n -> p d n", p=128))
            nc.sync.dma_start(
                w1k3v[:, 4 * hf:4 * hf + 4],
                w1k_d.ap()[dd, :].rearrange("(d p) l -> p d l", p=128))
            nc.sync.dma_start(
                w1v3v[:, 4 * hf:4 * hf + 4],
                w1v_d.ap()[dd, :].rearrange("(d p) l -> p d l", p=128))
        nc.sync.dma_start(cvt[:], cv_d.ap())

        lk2_sb = pl2.tile([128, 4 * DIM], FP8, tag="l2")
        lv2_sb = pl2.tile([128, 4 * DIM], FP8, tag="l2")
        nc.sync.dma_start(
            lk2_sb[:].rearrange("p (l c) -> p l c", l=4),
            lk2_d.ap().rearrange("(l p) c -> p l c", p=128))
        nc.sync.dma_start(
            lv2_sb[:].rearrange("p (l c) -> p l c", l=4),
            lv2_d.ap().rearrange("(l p) c -> p l c", p=128))

        wq_sb = pwq.tile([128, 8 * DIM], FP8, tag="wq")
        xq_sb = px.tile([128, 8 * T], FP8, tag="xq")
        nc.sync.dma_start(
            wq_sb[:].rearrange("p (d c) -> p d c", d=8),
            wq_d.ap().rearrange("(d p) c -> p d c", p=128))
        nc.sync.dma_start(
            xq_sb[:].rearrange("p (d n) -> p d n", d=8),
            xq_d.ap().rearrange("(d p) n -> p d n", p=128))

        kbt = psm.tile([128, NB], F32, tag="kbt")
        kbt2 = psm.tile([128, NB], F32, tag="kbt2")
        nc.sync.dma_start(kbt[:], kbt_d.ap())
        nc.sync.dma_start(kbt2[:], kbt2_d.ap())

        wout_sb = pwo.tile([128, 8 * DIM], FP8, tag="wo")
        boutt = psm.tile([128, 8], F32, tag="bo")
        nc.sync.dma_start(
            wout_sb[:].rearrange("p (d c) -> p d c", d=8),
            wout_d.ap().rearrange("(d p) c -> p d c", p=128))
        nc.sync.dma_start(boutt[:], bout_d.ap())

        # 3D chunk-major views
        w1k3 = w1k_sb[:].rearrange("p (d l) -> p d l", d=8)
        w1v3 = w1v_sb[:].rearrange("p (d l) -> p d l", d=8)
        xkv3 = xkv_sb[:].rearrange("p (d n) -> p d n", d=8)
        lk23 = lk2_sb[:].rearrange("p (l c) -> p l c", l=4)
        lv23 = lv2_sb[:].rearrange("p (l c) -> p l c", l=4)
        wq3 = wq_sb[:].rearrange("p (d c) -> p d c", d=8)
        xq3 = xq_sb[:].rearrange("p (d n) -> p d n", d=8)

        hk_sb = ph.tile([128, 4 * NK], FP8, tag="h")
        hv_sb = ph.tile([128, 4 * NK], FP8, tag="h")
        hk3 = hk_sb[:].rearrange("p (l n) -> p l n", l=4)
        hv3 = hv_sb[:].rearrange("p (l n) -> p l n", l=4)
        kt_sb = pkt.tile([128, 8 * NK], FP8, tag="kt")
        v_sb = pv.tile([128, NB * 2048], FP8, tag="v")
        qt_sb = pqt.tile([128, 8 * T], FP8, tag="qt")
        att_sb = patt.tile([128, 8 * T], FP8, tag="att")

        KEY_CH = [(0, 1024)] + ([(1024, NK - 1024)] if NK > 1024 else [])

        # ones columns for the PE-broadcast denominator (Pool, idle engine)
        v4 = v_sb[:].rearrange("p (j h two d) -> p (j h) two d",
                               j=NB, h=HEADS, two=2)
        nc.gpsimd.memset(v4[:, :, 1, :], 1.0)

        def h_path_l(w13, dst3, is_v, l):
            if True:
                ps = psA.tile([128, 1024], F32, tag="big")
                pst = psN.tile([128, 512], F32, tag="nm")
                for g0 in (0, 512):
                    for dp in range(4):
                        nc.tensor.matmul(
                            ps[:, g0:g0 + 512],
                            w13[:, 2 * dp:2 * dp + 2, 128 * l:128 * (l + 1)],
                            xkv3[:, 2 * dp:2 * dp + 2, g0:g0 + 512],
                            start=(dp == 0), stop=(dp == 3), perf_mode=DR)
                if NK > 1024:
                    for dp in range(4):
                        nc.tensor.matmul(
                            pst[:, 0:NK - 1024],
                            w13[:, 2 * dp:2 * dp + 2, 128 * l:128 * (l + 1)],
                            xkv3[:, 2 * dp:2 * dp + 2, 1024:NK],
                            start=(dp == 0), stop=(dp == 3), perf_mode=DR)
                with nc.allow_low_precision(reason="fp8 latents"):
                    if is_v:
                        # (max(ps,0) - cv) on DVE; centering the v-latents
                        nc.vector.tensor_scalar(
                            dst3[:, l, 0:1024], ps[:], 0.0, cvt[:, l:l + 1],
                            Alu.max, Alu.subtract)
                        if NK > 1024:
                            nc.vector.tensor_scalar(
                                dst3[:, l, 1024:NK], pst[:, 0:NK - 1024],
                                0.0, cvt[:, l:l + 1], Alu.max, Alu.subtract)
                    else:
                        nc.scalar.activation(dst3[:, l, 0:1024], ps[:], Relu,
                                             scale=2.0 ** -5)
                        if NK > 1024:
                            nc.scalar.activation(dst3[:, l, 1024:NK],
                                                 pst[:, 0:NK - 1024], Relu,
                                                 scale=2.0 ** -5)

        for l in range(4):
            h_path_l(w1k3, hk3, False, l)
            h_path_l(w1v3, hv3, True, l)

        # ---------------- kT (8 chunks), v (NB blocks), q (4 pairs) ------
        def kt_chunk(c8):
            ps = psA.tile([128, 1024], F32, tag="big")
            pst = psN.tile([128, 512], F32, tag="nm")
            for g0 in (0, 512):
                for lp in range(2):
                    nc.tensor.matmul(
                        ps[:, g0:g0 + 512],
                        lk23[:, 2 * lp:2 * lp + 2, 128 * c8:128 * (c8 + 1)],
                        hk3[:, 2 * lp:2 * lp + 2, g0:g0 + 512],
                        start=(lp == 0), stop=(lp == 1), perf_mode=DR)
            if NK > 1024:
                for lp in range(2):
                    nc.tensor.matmul(
                        pst[:, 0:NK - 1024],
                        lk23[:, 2 * lp:2 * lp + 2, 128 * c8:128 * (c8 + 1)],
                        hk3[:, 2 * lp:2 * lp + 2, 1024:NK],
                        start=(lp == 0), stop=(lp == 1), perf_mode=DR)
            # kT evac on DVE (kv-phase engine balance: ACT has relu+v+q)
            with nc.allow_low_precision(reason="fp8 k"):
                nc.vector.tensor_scalar(
                    kt_sb[:, c8 * NK:c8 * NK + 1024], ps[:],
                    2.0 ** -4, None, Alu.mult)
                if NK > 1024:
                    nc.vector.tensor_scalar(
                        kt_sb[:, c8 * NK + 1024:(c8 + 1) * NK],
                        pst[:, 0:NK - 1024], 2.0 ** -4, None, Alu.mult)

        def v_block(j):
            ps = psA.tile([128, 1024], F32, tag="big")
            for ch in range(2):
                for lp in range(2):
                    nc.tensor.matmul(
                        ps[:, 512 * ch:512 * (ch + 1)],
                        hv3[:, 2 * lp:2 * lp + 2, 128 * j:128 * (j + 1)],
                        lv23[:, 2 * lp:2 * lp + 2, 512 * ch:512 * (ch + 1)],
                        start=(lp == 0), stop=(lp == 1), perf_mode=DR)
            dst = v_sb[:, j * 2048:(j + 1) * 2048] \
                .rearrange("p (h two d) -> p h two d", h=16, two=2)[:, :, 0, :]
            with nc.allow_low_precision(reason="fp8 v"):
                if j % 3 == 2:
                    nc.vector.tensor_scalar(
                        dst, ps[:].rearrange("p (h d) -> p h d", h=16),
                        2.0 ** -8, None, Alu.mult)
                else:
                    nc.scalar.activation(
                        dst, ps[:].rearrange("p (h d) -> p h d", h=16),
                        Copy, scale=2.0 ** -8)

        def q_pair(t):
            ps = psA.tile([128, 1024], F32, tag="big")
            for pl in range(2):
                for dp in range(4):
                    nc.tensor.matmul(
                        ps[:, 512 * pl:512 * (pl + 1)],
                        wq3[:, 2 * dp:2 * dp + 2,
                            (2 * t + pl) * 128:(2 * t + pl + 1) * 128],
                        xq3[:, 2 * dp:2 * dp + 2, :],
                        start=(dp == 0), stop=(dp == 3), perf_mode=DR)
            with nc.allow_low_precision(reason="fp8 q"):
                nc.scalar.activation(qt_sb[:, t * 1024:(t + 1) * 1024], ps[:],
                                     Copy)

        order = [("q", 0), ("q", 1), ("k", 0), ("k", 1), ("v", 0),
                 ("k", 2), ("v", 1), ("k", 3), ("v", 2), ("q", 2),
                 ("k", 4), ("v", 3), ("k", 5), ("v", 4), ("q", 3),
                 ("k", 6), ("v", 5), ("k", 7), ("v", 6), ("v", 7), ("v", 8)]
        order = [(k, i) for (k, i) in order if
                 (k != "v" or i < NB) and (k != "k" or i < 8)]
        for kind, i in order:
            (kt_chunk if kind == "k" else v_block if kind == "v" else q_pair)(i)

        def drain_kv(n):
            pass

        # ---------------- attention -------------------------------------
        # flat software pipeline across all (head, unit) pairs: scores run
        # two units ahead of exp; numer three behind; each head's normalize
        # is deferred until after the NEXT head's DVE exps so the
        # exp->numer->recip round trip never stalls DVE.  exp split: ACT
        # gets pad-free pairs 0,1,2 (const bias); DVE gets pair 3 + the
        # leftover block + reciprocal + normalize multiply.
        NPAIR = NB // 2
        NU = NPAIR + (1 if NB % 2 else 0)
        v3 = v_sb[:].rearrange("p (j x) -> p j x", j=NB)

        scs = {}
        es = {}
        nms = {}
        pending_norm = []

        def head_views(h):
            t, g = h // 4, h % 4
            kt3 = kt_sb[:, t * 2 * NK:(t + 1) * 2 * NK] \
                .rearrange("p (pl k) -> p pl k", pl=2)
            qt3 = qt_sb[:, t * 1024:(t + 1) * 1024] \
                .rearrange("p (pl n) -> p pl n", pl=2)
            return kt3, qt3[32 * g:32 * (g + 1), :, :], 32 * g

        def emit_sc(h, u):
            kt3, lhq, bp32 = head_views(h)
            if u < NPAIR:
                sc = psA.tile([128, 1024], F32, tag="big", name=f"sc{h}_{u}")
            else:
                # leftover unit lives in the psN ring so the psA ring
                # advances by 4 (not 5) per head: the next head's first sc
                # then reuses a slot freed by an early DVE exp, not ACT's
                # last one
                sc = psN.tile([128, 512], F32, tag="nm", name=f"sc{h}_{u}")
            scs[(h, u)] = sc
            nj = 2 if u < NPAIR else 1
            for half in range(nj):
                j = 2 * u + half
                nc.tensor.matmul(
                    sc[:, 512 * half:512 * (half + 1)],
                    kt3[bp32:bp32 + 32, :, 128 * j:128 * (j + 1)],
                    lhq, start=True, stop=True, perf_mode=DR,
                    tile_position=(bp32, 0))

        def emit_exp(h, u):
            sc = scs[(h, u)]
            if u < NPAIR:
                e = pe_.tile([128, 1024], FP8, tag="e")
                with nc.allow_low_precision(reason="fp8 softmax"):
                    if 2 * u + 1 < FPB and u < 3:
                        nc.scalar.activation(e[:], sc[:], Exp,
                                             bias=kbt[:, 0:1],
                                             scale=SCALE / 2)
                    else:
                        for half in range(2):
                            j = 2 * u + half
                            nc.vector.tensor_scalar(
                                e[:, 512 * half:512 * (half + 1)].bitcast(U8),
                                sc[:, 512 * half:512 * (half + 1)], A8S,
                                KB2R if j < FPB else kbt2[:, j:j + 1],
                                Alu.mult, Alu.add)
            else:
                j = 2 * NPAIR
                e = pe_.tile([128, 512], FP8, tag="e1")
                with nc.allow_low_precision(reason="fp8 softmax"):
                    if h % 2 == 1:
                        nc.scalar.activation(e[:], sc[:], Exp,
                                             bias=kbt[:, j:j + 1],
                                             scale=SCALE / 2)
                    else:
                        nc.vector.tensor_scalar(
                            e[:].bitcast(U8), sc[:], A8S,
                            KB2R if j < FPB else kbt2[:, j:j + 1],
                            Alu.mult, Alu.add)
            es[(h, u)] = e
            # emit the deferred normalize after this head's first DVE exp
            if u == 3 and pending_norm:
                pending_norm.pop(0)()

        def emit_nm(h, u):
            if u == 0:
                nms[h] = psN.tile([128, 512], F32, tag="nm", name=f"nm{h}")
            nm = nms[h]
            if u < NPAIR:
                nc.tensor.matmul(
                    nm[:], v3[:, 2 * u:2 * u + 2, 128 * h:128 * (h + 1)],
                    es[(h, u)][:].rearrange("p (two n) -> p two n", two=2),
                    start=(u == 0), stop=(u == NU - 1), perf_mode=DR)
            else:
                nc.tensor.matmul(
                    nm[:], v3[:, 2 * NPAIR, 128 * h:128 * (h + 1)],
                    es[(h, u)][:], start=False, stop=True,
                    skip_group_check=True)
            if u == NU - 1:
                def normalize(h=h, nm=nm):
                    rr = pr.tile([64, 512], F32, tag="r")
                    nc.vector.reciprocal(rr[:], nm[64:128, :])
                    with nc.allow_low_precision(reason="bf16 att"):
                        nc.vector.tensor_mul(
                            att_sb[64 * (h % 2):64 * (h % 2) + 64,
                                   (h // 2) * T:(h // 2 + 1) * T],
                            nm[0:64, :], rr[:])
                pending_norm.append(normalize)

        units = [(h, u) for h in range(HEADS) for u in range(NU)]
        LA_E, LA_N = 1, 5
        for i in range(len(units) + LA_N):
            if i < len(units):
                emit_sc(*units[i])
            if LA_E <= i and i - LA_E < len(units):
                emit_exp(*units[i - LA_E])
            drain_kv(1)
            if LA_N <= i and i - LA_N < len(units):
                emit_nm(*units[i - LA_N])
        while pending_norm:
            pending_norm.pop(0)()

        # ---------------- output projection ------------------------------
        wo3 = wout_sb[:].rearrange("p (d c) -> p d c", d=8)
        att3 = att_sb[:].rearrange("p (c n) -> p c n", c=8)
        for cb in range(8):
            ps = psA.tile([128, 1024], F32, tag="big")
            for c2 in range(4):
                nc.tensor.matmul(
                    ps[:, 0:512],
                    wo3[:, 2 * c2:2 * c2 + 2, 128 * cb:128 * (cb + 1)],
                    att3[:, 2 * c2:2 * c2 + 2, :],
                    start=(c2 == 0), stop=(c2 == 3), perf_mode=DR)
            osb = posb.tile([128, 512], BF16, tag="osb")
            with nc.allow_low_precision(reason="bf16 output"):
                nc.vector.tensor_scalar(osb[:], ps[:, 0:512],
                                        2.0 ** -7, boutt[:, cb:cb + 1],
                                        Alu.mult, Alu.add)
            nc.sync.dma_start(y_d.ap()[128 * cb:128 * (cb + 1), :], osb[:])

    nc.compile()
    return nc


def _f8(x):
    return np.asarray(x, np.float32).astype(NPFP8)


def kernel(x, mask, wq, wkv, lk1, lk2, lv1, lv2, wout, bout, **kw):
    global LAST_RESULTS
    x = np.asarray(x, np.float32)
    mask = np.asarray(mask)
    wq = np.asarray(wq, np.float64)
    wkv = np.asarray(wkv, np.float64)
    lk1 = np.asarray(lk1, np.float64)
    lk2 = np.asarray(lk2, np.float64)
    lv1 = np.asarray(lv1, np.float64)
    lv2 = np.asarray(lv2, np.float64)
    wout = np.asarray(wout, np.float64)
    bout = np.asarray(bout, np.float64)

    act = [np.nonzero(np.asarray(mask[b]) == 1)[0] for b in range(B)]
    A = [len(a) for a in act]
    NB = max(1, (max(A) + 127) // 128)
    NK = NB * 128
    FPB = min(A) // 128          # first block that contains pad slots

    # column permutation for the DR-32 scores layout:
    # psum chunk (t,pl) partitions = [head 4t+g, dims 32pl..32pl+32]
    perm = np.array([64 * (4 * t + g) + 32 * pl + i
                     for t in range(4) for pl in range(2)
                     for g in range(4) for i in range(32)])

    w1k = wkv[:, :DIM] @ lk1
    w1v = wkv[:, DIM:] @ lv1
    w1k8 = _f8(32 * w1k)
    w1v8 = _f8(32 * w1v)
    lk28 = _f8(32 * lk2[:, perm])
    lv28 = _f8(32 * lv2)
    wq8 = _f8(wq[:, perm])
    woutb = _f8(32.0 * wout)

    # v-path centering: statistical mean of relu(w1v8 . x) per latent,
    # snapped to the fp8 grid so exact relu-zeros quantize exactly
    xr = float(np.sqrt((x.astype(np.float64) ** 2).mean()))
    colv = np.sqrt((w1v8.astype(np.float64) ** 2).sum(0)) * xr
    cv32 = _f8(0.39894228 * colv).astype(np.float64)       # scale-32 units
    v0 = (cv32 / 32.0) @ lv2                               # exact lv2
    bout2 = bout + v0 @ wout
    bout2_t = np.ascontiguousarray(
        bout2.reshape(8, 128).T.astype(np.float32))
    cv_t = np.ascontiguousarray(
        cv32.reshape(4, 128).T.astype(np.float32))

    # exp biases per key slot (per batch)
    kbt = np.full((B, NK), NEGB, np.float32)
    kbt2 = np.full((B, NK), KB2R + A8 * (NEGB + C_SHIFT), np.float32)
    for b in range(B):
        kbt[b, :A[b]] = -C_SHIFT
        kbt2[b, :A[b]] = KB2R
    kbt_t = [np.ascontiguousarray(kbt[b].reshape(NB, 128).T) for b in range(B)]
    kbt2_t = [np.ascontiguousarray(kbt2[b].reshape(NB, 128).T) for b in range(B)]

    key = (NB, FPB)
    if key not in _cache:
        _cache[key] = _build(NB, FPB)
    nc = _cache[key]

    x_flat = x.reshape(B * N, DIM)
    xkv_b = []
    for b in range(B):
        xkv = np.zeros((DIM, NK), NPFP8)
        xkv[:, :A[b]] = _f8(x_flat[b * N + act[b]].T)
        xkv_b.append(xkv)

    in_maps = []
    for c in range(NC):
        b = c // 4
        in_maps.append({
            "xq": np.ascontiguousarray(_f8(x_flat[c * T:(c + 1) * T].T)),
            "xkv": xkv_b[b],
            "wq": wq8, "w1k": w1k8, "w1v": w1v8, "lk2": lk28, "lv2": lv28,
            "wout": woutb, "bout2": bout2_t, "kbt": kbt_t[b],
            "kbt2": kbt2_t[b], "cv": cv_t,
        })

    res = run_bass_kernel_spmd(nc, in_maps, core_ids=list(range(NC)))
    LAST_RESULTS = res
    y = np.empty((B * N, DIM), np.float32)
    for c in range(NC):
        y[c * T:(c + 1) * T] = res.results[c]["yT"].T.astype(np.float32)
    return y.reshape(B, N, DIM)


# revision 89
# speedup vs baseline: 2.2216x; 1.0075x over previous
"""Trainium2 Bass kernel for KeOps multi-head latent attention (v2).

Reference (B=2, N=2048, DIM=1024, LATENT=512, HEADS=16, HD=64):
    q = x @ wq * scale
    k = relu((x @ wkv[:, :D]) @ lk1) @ lk2   (folded: relu(x @ w1k) @ lk2)
    v = relu((x @ wkv[:, D:]) @ lv1) @ lv2
    per head: e = exp(q k^T + maskbias); out = (e @ v) / (e.sum + 1e-6)
    y = out @ wout + bout

Strategy (8 cores, one SPMD NEFF, NO collective):
  - queries: tokens sharded 512/core (cores 0-3 batch0, 4-7 batch1).
  - keys: masked keys compacted on host; EVERY core computes k/v for its
    batch's full active-key set (NB*128 slots) locally — redundant compute
    is far cheaper than the modeled AllGather (15us + 40GB/s).
  - all matmuls fp8e4m3 with DoubleRow (2x modeled PE throughput),
    including the output projection: v-path mean-centering makes att/wout
    fp8 affordable (the mean path rides the exact fp64 output bias).
  - v-path mean-centering: h_v has positive mean (relu); subtracting a
    host-computed statistical mean per latent (fp8-snapped) removes the
    coherent component of the lv2-fp8 quantization error; the mean path
    v0 = c @ lv2 rides through the output bias in fp64 (exact since
    softmax weights sum to 1). K-path coherent errors cancel in softmax.
  - scores per head via DoubleRow on 32-partition quadrants
    (tile_position): head dims split 32+32 across the two DR planes.
  - denominator: 64 'ones' columns interleaved with v give a PE-broadcast
    denominator on psum partitions 64:127 (free), so normalize is one
    reciprocal + one multiply on DVE per head.
  - exp split between ACT (true exp, fp8 out) and DVE (Schraudolph uint8
    bit-trick -> fp8e4m3, bit-exact validated on HW) to balance engines.
"""

import sys

sys.path.insert(0, "/opt/trn_rl_repo")
import numpy as np
import ml_dtypes
import concourse.bass as bass
import concourse.mybir as mybir
import concourse.tile as tile
from concourse import bacc
from concourse.bass_utils import run_bass_kernel_spmd

DIM, LATENT, HEADS, HD = 1024, 512, 16, 64
B, N, NC, T = 2, 2048, 8, 512
SCALE = HD ** -0.5
BF16, F32, FP8 = mybir.dt.bfloat16, mybir.dt.float32, mybir.dt.float8e4
U8 = mybir.dt.uint8
NPBF16 = ml_dtypes.bfloat16
NPFP8 = ml_dtypes.float8_e4m3
DR = mybir.MatmulPerfMode.DoubleRow

LN2 = float(np.log(2.0))
C_SHIFT = 5 * LN2            # exp shift; e^-C folded via bias, 2^-5 exact
NEGB = -35.0                 # pad-kill bias
C8 = 0.0435                  # schraudolph tuning constant
A8 = 8.0 / LN2
A8S = A8 * SCALE / 2.0       # DVE schr multiplier on raw scores
KB2R = 8.0 * (7.0 - C8) - A8 * C_SHIFT   # schr bias, real keys

_cache: dict = {}
LAST_RESULTS = None


def _build(NB, FPB):
    """NB = key blocks of 128 per batch; FPB = first block containing pads
    (blocks < FPB use constant exp bias; blocks >= FPB use per-slot AP)."""
    NK = NB * 128
    Exp = mybir.ActivationFunctionType.Exp
    Relu = mybir.ActivationFunctionType.Relu
    Copy = mybir.ActivationFunctionType.Copy
    Ident = mybir.ActivationFunctionType.Identity
    Alu = mybir.AluOpType

    nc = bacc.Bacc("TRN2", target_bir_lowering=False, num_devices=NC)
    xq_d = nc.dram_tensor("xq", [DIM, T], FP8, kind="ExternalInput")
    xkv_d = nc.dram_tensor("xkv", [DIM, NK], FP8, kind="ExternalInput")
    wq_d = nc.dram_tensor("wq", [DIM, DIM], FP8, kind="ExternalInput")
    w1k_d = nc.dram_tensor("w1k", [DIM, LATENT], FP8, kind="ExternalInput")
    w1v_d = nc.dram_tensor("w1v", [DIM, LATENT], FP8, kind="ExternalInput")
    lk2_d = nc.dram_tensor("lk2", [LATENT, DIM], FP8, kind="ExternalInput")
    lv2_d = nc.dram_tensor("lv2", [LATENT, DIM], FP8, kind="ExternalInput")
    wout_d = nc.dram_tensor("wout", [DIM, DIM], FP8, kind="ExternalInput")
    bout_d = nc.dram_tensor("bout2", [128, 8], F32, kind="ExternalInput")
    kbt_d = nc.dram_tensor("kbt", [128, NB], F32, kind="ExternalInput")
    kbt2_d = nc.dram_tensor("kbt2", [128, NB], F32, kind="ExternalInput")
    cv_d = nc.dram_tensor("cv", [128, 4], F32, kind="ExternalInput")
    y_d = nc.dram_tensor("yT", [DIM, T], BF16, kind="ExternalOutput")

    from contextlib import ExitStack
    with ExitStack() as ctx:
        tc = ctx.enter_context(tile.TileContext(nc))
        pool = lambda **kw: ctx.enter_context(tc.tile_pool(**kw))
        pw1 = pool(name="pw1", bufs=2)
        pl2 = pool(name="pl2", bufs=2)
        pwq = pool(name="pwq", bufs=1)
        pwo = pool(name="pwo", bufs=1)
        px = pool(name="px", bufs=1)
        ph = pool(name="ph", bufs=2)
        pkt = pool(name="pkt", bufs=1)
        pv = pool(name="pv", bufs=1)
        pqt = pool(name="pqt", bufs=1)
        patt = pool(name="patt", bufs=1)
        pe_ = pool(name="pe", bufs=8)
        pr = pool(name="pr", bufs=3)
        posb = pool(name="posb", bufs=5)
        psm = pool(name="psm", bufs=1)
        psA = pool(name="psA", bufs=3, space="PSUM")
        psN = pool(name="psN", bufs=2, space="PSUM")

        # ---------------- input DMAs ------------------------------------
        # split the critical first loads in halves so h matmuls start early
        w1k_sb = pw1.tile([128, 8 * LATENT], FP8, tag="w1")
        w1v_sb = pw1.tile([128, 8 * LATENT], FP8, tag="w1")
        xkv_sb = px.tile([128, 8 * NK], FP8, tag="xkv")
        cvt = psm.tile([128, 4], F32, tag="cv")
        xkv3v = xkv_sb[:].rearrange("p (d n) -> p d n", d=8)
        w1k3v = w1k_sb[:].rearrange("p (d l) -> p d l", d=8)
        w1v3v = w1v_sb[:].rearrange("p (d l) -> p d l", d=8)
        for hf in range(2):
            dd = slice(512 * hf, 512 * (hf + 1))
            nc.sync.dma_start(
                xkv3v[:, 4 * hf:4 * hf + 4],
                xkv_d.ap()[dd, :].rearrange("(d p) n -> p d n", p=128))
            nc.sync.dma_start(
                w1k3v[:, 4 * hf:4 * hf + 4],
                w1k_d.ap()[dd, :].rearrange("(d p) l -> p d l", p=128))
            nc.sync.dma_start(
                w1v3v[:, 4 * hf:4 * hf + 4],
                w1v_d.ap()[dd, :].rearrange("(d p) l -> p d l", p=128))
        nc.sync.dma_start(cvt[:], cv_d.ap())

        lk2_sb = pl2.tile([128, 4 * DIM], FP8, tag="l2")
        lv2_sb = pl2.tile([128, 4 * DIM], FP8, tag="l2")
        nc.sync.dma_start(
            lk2_sb[:].rearrange("p (l c) -> p l c", l=4),
            lk2_d.ap().rearrange("(l p) c -> p l c", p=128))
        nc.sync.dma_start(
            lv2_sb[:].rearrange("p (l c) -> p l c", l=4),
            lv2_d.ap().rearrange("(l p) c -> p l c", p=128))

        wq_sb = pwq.tile([128, 8 * DIM], FP8, tag="wq")
        xq_sb = px.tile([128, 8 * T], FP8, tag="xq")
        nc.sync.dma_start(
            wq_sb[:].rearrange("p (d c) -> p d c", d=8),
            wq_d.ap().rearrange("(d p) c -> p d c", p=128))
        nc.sync.dma_start(
            xq_sb[:].rearrange("p (d n) -> p d n", d=8),
            xq_d.ap().rearrange("(d p) n -> p d n", p=128))

        kbt = psm.tile([128, NB], F32, tag="kbt")
        kbt2 = psm.tile([128, NB], F32, tag="kbt2")
        nc.sync.dma_start(kbt[:], kbt_d.ap())
        nc.sync.dma_start(kbt2[:], kbt2_d.ap())

        wout_sb = pwo.tile([128, 8 * DIM], FP8, tag="wo")
        boutt = psm.tile([128, 8], F32, tag="bo")
        nc.sync.dma_start(
            wout_sb[:].rearrange("p (d c) -> p d c", d=8),
            wout_d.ap().rearrange("(d p) c -> p d c", p=128))
        nc.sync.dma_start(boutt[:], bout_d.ap())

        # 3D chunk-major views
        w1k3 = w1k_sb[:].rearrange("p (d l) -> p d l", d=8)
        w1v3 = w1v_sb[:].rearrange("p (d l) -> p d l", d=8)
        xkv3 = xkv_sb[:].rearrange("p (d n) -> p d n", d=8)
        lk23 = lk2_sb[:].rearrange("p (l c) -> p l c", l=4)
        lv23 = lv2_sb[:].rearrange("p (l c) -> p l c", l=4)
        wq3 = wq_sb[:].rearrange("p (d c) -> p d c", d=8)
        xq3 = xq_sb[:].rearrange("p (d n) -> p d n", d=8)

        hk_sb = ph.tile([128, 4 * NK], FP8, tag="h")
        hv_sb = ph.tile([128, 4 * NK], FP8, tag="h")
        hk3 = hk_sb[:].rearrange("p (l n) -> p l n", l=4)
        hv3 = hv_sb[:].rearrange("p (l n) -> p l n", l=4)
        kt_sb = pkt.tile([128, 8 * NK], FP8, tag="kt")
        v_sb = pv.tile([128, NB * 2048], FP8, tag="v")
        qt_sb = pqt.tile([128, 8 * T], FP8, tag="qt")
        att_sb = patt.tile([128, 8 * T], FP8, tag="att")

        KEY_CH = [(0, 1024)] + ([(1024, NK - 1024)] if NK > 1024 else [])

        # ones columns for the PE-broadcast denominator (Pool, idle engine)
        v4 = v_sb[:].rearrange("p (j h two d) -> p (j h) two d",
                               j=NB, h=HEADS, two=2)
        nc.gpsimd.memset(v4[:, :, 1, :], 1.0)

        def h_path_l(w13, dst3, is_v, l):
            if True:
                ps = psA.tile([128, 1024], F32, tag="big")
                pst = psN.tile([128, 512], F32, tag="nm")
                for g0 in (0, 512):
                    for dp in range(4):
                        nc.tensor.matmul(
                            ps[:, g0:g0 + 512],
                            w13[:, 2 * dp:2 * dp + 2, 128 * l:128 * (l + 1)],
                            xkv3[:, 2 * dp:2 * dp + 2, g0:g0 + 512],
                            start=(dp == 0), stop=(dp == 3), perf_mode=DR)
                if NK > 1024:
                    for dp in range(4):
                        nc.tensor.matmul(
                            pst[:, 0:NK - 1024],
                            w13[:, 2 * dp:2 * dp + 2, 128 * l:128 * (l + 1)],
                            xkv3[:, 2 * dp:2 * dp + 2, 1024:NK],
                            start=(dp == 0), stop=(dp == 3), perf_mode=DR)
                with nc.allow_low_precision(reason="fp8 latents"):
                    if is_v:
                        # (max(ps,0) - cv) on DVE; centering the v-latents
                        nc.vector.tensor_scalar(
                            dst3[:, l, 0:1024], ps[:], 0.0, cvt[:, l:l + 1],
                            Alu.max, Alu.subtract)
                        if NK > 1024:
                            nc.vector.tensor_scalar(
                                dst3[:, l, 1024:NK], pst[:, 0:NK - 1024],
                                0.0, cvt[:, l:l + 1], Alu.max, Alu.subtract)
                    else:
                        nc.scalar.activation(dst3[:, l, 0:1024], ps[:], Relu,
                                             scale=2.0 ** -5)
                        if NK > 1024:
                            nc.scalar.activation(dst3[:, l, 1024:NK],
                                                 pst[:, 0:NK - 1024], Relu,
                                                 scale=2.0 ** -5)

        for l in range(4):
            h_path_l(w1k3, hk3, False, l)
            h_path_l(w1v3, hv3, True, l)

        # ---------------- kT (8 chunks), v (NB blocks), q (4 pairs) ------
        def kt_chunk(c8):
            ps = psA.tile([128, 1024], F32, tag="big")
            pst = psN.tile([128, 512], F32, tag="nm")
            for g0 in (0, 512):
                for lp in range(2):
                    nc.tensor.matmul(
                        ps[:, g0:g0 + 512],
                        lk23[:, 2 * lp:2 * lp + 2, 128 * c8:128 * (c8 + 1)],
                        hk3[:, 2 * lp:2 * lp + 2, g0:g0 + 512],
                        start=(lp == 0), stop=(lp == 1), perf_mode=DR)
            if NK > 1024:
                for lp in range(2):
                    nc.tensor.matmul(
                        pst[:, 0:NK - 1024],
                        lk23[:, 2 * lp:2 * lp + 2, 128 * c8:128 * (c8 + 1)],
                        hk3[:, 2 * lp:2 * lp + 2, 1024:NK],
                        start=(lp == 0), stop=(lp == 1), perf_mode=DR)
            # kT evac on DVE (kv-phase engine balance: ACT has relu+v+q)
            with nc.allow_low_precision(reason="fp8 k"):
                nc.vector.tensor_scalar(
                    kt_sb[:, c8 * NK:c8 * NK + 1024], ps[:],
                    2.0 ** -4, None, Alu.mult)
                if NK > 1024:
                    nc.vector.tensor_scalar(
                        kt_sb[:, c8 * NK + 1024:(c8 + 1) * NK],
                        pst[:, 0:NK - 1024], 2.0 ** -4, None, Alu.mult)

        def v_block(j):
            ps = psA.tile([128, 1024], F32, tag="big")
            for ch in range(2):
                for lp in range(2):
                    nc.tensor.matmul(
                        ps[:, 512 * ch:512 * (ch + 1)],
                        hv3[:, 2 * lp:2 * lp + 2, 128 * j:128 * (j + 1)],
                        lv23[:, 2 * lp:2 * lp + 2, 512 * ch:512 * (ch + 1)],
                        start=(lp == 0), stop=(lp == 1), perf_mode=DR)
            dst = v_sb[:, j * 2048:(j + 1) * 2048] \
                .rearrange("p (h two d) -> p h two d", h=16, two=2)[:, :, 0, :]
            with nc.allow_low_precision(reason="fp8 v"):
                if j % 3 == 2:
                    nc.vector.tensor_scalar(
                        dst, ps[:].rearrange("p (h d) -> p h d", h=16),
                        2.0 ** -8, None, Alu.mult)
                else:
                    nc.scalar.activation(
                        dst, ps[:].rearrange("p (h d) -> p h d", h=16),
                        Copy, scale=2.0 ** -8)

        def q_pair(t):
            ps = psA.tile([128, 1024], F32, tag="big")
            for pl in range(2):
                for dp in range(4):
                    nc.tensor.matmul(
                        ps[:, 512 * pl:512 * (pl + 1)],
                        wq3[:, 2 * dp:2 * dp + 2,
                            (2 * t + pl) * 128:(2 * t + pl + 1) * 128],
                        xq3[:, 2 * dp:2 * dp + 2, :],
                        start=(dp == 0), stop=(dp == 3), perf_mode=DR)
            with nc.allow_low_precision(reason="fp8 q"):
                nc.scalar.activation(qt_sb[:, t * 1024:(t + 1) * 1024], ps[:],
                                     Copy)

        order = [("k", 0), ("q", 0), ("q", 1), ("k", 1), ("v", 0),
                 ("k", 2), ("v", 1), ("k", 3), ("v", 2), ("q", 2),
                 ("k", 4), ("v", 3), ("k", 5), ("v", 4), ("q", 3),
                 ("k", 6), ("v", 5), ("k", 7), ("v", 6), ("v", 7), ("v", 8)]
        order = [(k, i) for (k, i) in order if
                 (k != "v" or i < NB) and (k != "k" or i < 8)]
        for kind, i in order:
            (kt_chunk if kind == "k" else v_block if kind == "v" else q_pair)(i)

        def drain_kv(n):
            pass

        # ---------------- attention -------------------------------------
        # flat software pipeline across all (head, unit) pairs: scores run
        # two units ahead of exp; numer three behind; each head's normalize
        # is deferred until after the NEXT head's DVE exps so the
        # exp->numer->recip round trip never stalls DVE.  exp split: ACT
        # gets pad-free pairs 0,1,2 (const bias); DVE gets pair 3 + the
        # leftover block + reciprocal + normalize multiply.
        NPAIR = NB // 2
        NU = NPAIR + (1 if NB % 2 else 0)
        v3 = v_sb[:].rearrange("p (j x) -> p j x", j=NB)

        scs = {}
        es = {}
        nms = {}
        pending_norm = []

        def head_views(h):
            t, g = h // 4, h % 4
            kt3 = kt_sb[:, t * 2 * NK:(t + 1) * 2 * NK] \
                .rearrange("p (pl k) -> p pl k", pl=2)
            qt3 = qt_sb[:, t * 1024:(t + 1) * 1024] \
                .rearrange("p (pl n) -> p pl n", pl=2)
            return kt3, qt3[32 * g:32 * (g + 1), :, :], 32 * g

        def emit_sc(h, u):
            kt3, lhq, bp32 = head_views(h)
            if u < NPAIR:
                sc = psA.tile([128, 1024], F32, tag="big", name=f"sc{h}_{u}")
            else:
                # leftover unit lives in the psN ring so the psA ring
                # advances by 4 (not 5) per head: the next head's first sc
                # then reuses a slot freed by an early DVE exp, not ACT's
                # last one
                sc = psN.tile([128, 512], F32, tag="nm", name=f"sc{h}_{u}")
            scs[(h, u)] = sc
            nj = 2 if u < NPAIR else 1
            for half in range(nj):
                j = 2 * u + half
                nc.tensor.matmul(
                    sc[:, 512 * half:512 * (half + 1)],
                    kt3[bp32:bp32 + 32, :, 128 * j:128 * (j + 1)],
                    lhq, start=True, stop=True, perf_mode=DR,
                    tile_position=(bp32, 0))

        def emit_exp(h, u):
            sc = scs[(h, u)]
            if u < NPAIR:
                e = pe_.tile([128, 1024], FP8, tag="e")
                with nc.allow_low_precision(reason="fp8 softmax"):
                    if 2 * u + 1 < FPB and u < 3:
                        nc.scalar.activation(e[:], sc[:], Exp,
                                             bias=kbt[:, 0:1],
                                             scale=SCALE / 2)
                    else:
                        for half in range(2):
                            j = 2 * u + half
                            nc.vector.tensor_scalar(
                                e[:, 512 * half:512 * (half + 1)].bitcast(U8),
                                sc[:, 512 * half:512 * (half + 1)], A8S,
                                KB2R if j < FPB else kbt2[:, j:j + 1],
                                Alu.mult, Alu.add)
            else:
                j = 2 * NPAIR
                e = pe_.tile([128, 512], FP8, tag="e1")
                with nc.allow_low_precision(reason="fp8 softmax"):
                    if h % 2 == 1:
                        nc.scalar.activation(e[:], sc[:], Exp,
                                             bias=kbt[:, j:j + 1],
                                             scale=SCALE / 2)
                    else:
                        nc.vector.tensor_scalar(
                            e[:].bitcast(U8), sc[:], A8S,
                            KB2R if j < FPB else kbt2[:, j:j + 1],
                            Alu.mult, Alu.add)
            es[(h, u)] = e
            # emit the deferred normalize after this head's first DVE exp
            if u == 3 and pending_norm:
                pending_norm.pop(0)()

        def emit_nm(h, u):
            if u == 0:
                nms[h] = psN.tile([128, 512], F32, tag="nm", name=f"nm{h}")
            nm = nms[h]
            if u < NPAIR:
                nc.tensor.matmul(
                    nm[:], v3[:, 2 * u:2 * u + 2, 128 * h:128 * (h + 1)],
                    es[(h, u)][:].rearrange("p (two n) -> p two n", two=2),
                    start=(u == 0), stop=(u == NU - 1), perf_mode=DR)
            else:
                nc.tensor.matmul(
                    nm[:], v3[:, 2 * NPAIR, 128 * h:128 * (h + 1)],
                    es[(h, u)][:], start=False, stop=True,
                    skip_group_check=True)
            if u == NU - 1:
                def normalize(h=h, nm=nm):
                    rr = pr.tile([64, 512], F32, tag="r")
                    nc.vector.reciprocal(rr[:], nm[64:128, :])
                    with nc.allow_low_precision(reason="bf16 att"):
                        nc.vector.tensor_mul(
                            att_sb[64 * (h % 2):64 * (h % 2) + 64,
                                   (h // 2) * T:(h // 2 + 1) * T],
                            nm[0:64, :], rr[:])
                pending_norm.append(normalize)

        units = [(h, u) for h in range(HEADS) for u in range(NU)]
        LA_E, LA_N = 1, 5
        for i in range(len(units) + LA_N):
            if i < len(units):
                emit_sc(*units[i])
            if LA_E <= i and i - LA_E < len(units):
                emit_exp(*units[i - LA_E])
            drain_kv(1)
            if LA_N <= i and i - LA_N < len(units):
                emit_nm(*units[i - LA_N])
        while pending_norm:
            pending_norm.pop(0)()

        # ---------------- output projection ------------------------------
        wo3 = wout_sb[:].rearrange("p (d c) -> p d c", d=8)
        att3 = att_sb[:].rearrange("p (c n) -> p c n", c=8)
        for cb in range(8):
            ps = psA.tile([128, 1024], F32, tag="big")
            for c2 in range(4):
                nc.tensor.matmul(
                    ps[:, 0:512],
                    wo3[:, 2 * c2:2 * c2 + 2, 128 * cb:128 * (cb + 1)],
                    att3[:, 2 * c2:2 * c2 + 2, :],
                    start=(c2 == 0), stop=(c2 == 3), perf_mode=DR)
            osb = posb.tile([128, 512], BF16, tag="osb")
            with nc.allow_low_precision(reason="bf16 output"):
                nc.vector.tensor_scalar(osb[:], ps[:, 0:512],
                                        2.0 ** -7, boutt[:, cb:cb + 1],
                                        Alu.mult, Alu.add)
            nc.sync.dma_start(y_d.ap()[128 * cb:128 * (cb + 1), :], osb[:])

    nc.compile()
    return nc


def _f8(x):
    return np.asarray(x, np.float32).astype(NPFP8)


def kernel(x, mask, wq, wkv, lk1, lk2, lv1, lv2, wout, bout, **kw):
    global LAST_RESULTS
    x = np.asarray(x, np.float32)
    mask = np.asarray(mask)
    wq = np.asarray(wq, np.float64)
    wkv = np.asarray(wkv, np.float64)
    lk1 = np.asarray(lk1, np.float64)
    lk2 = np.asarray(lk2, np.float64)
    lv1 = np.asarray(lv1, np.float64)
    lv2 = np.asarray(lv2, np.float64)
    wout = np.asarray(wout, np.float64)
    bout = np.asarray(bout, np.float64)

    act = [np.nonzero(np.asarray(mask[b]) == 1)[0] for b in range(B)]
    A = [len(a) for a in act]
    NB = max(1, (max(A) + 127) // 128)
    NK = NB * 128
    FPB = min(A) // 128          # first block that contains pad slots

    # column permutation for the DR-32 scores layout:
    # psum chunk (t,pl) partitions = [head 4t+g, dims 32pl..32pl+32]
    perm = np.array([64 * (4 * t + g) + 32 * pl + i
                     for t in range(4) for pl in range(2)
                     for g in range(4) for i in range(32)])

    w1k = wkv[:, :DIM] @ lk1
    w1v = wkv[:, DIM:] @ lv1
    w1k8 = _f8(32 * w1k)
    w1v8 = _f8(32 * w1v)
    lk28 = _f8(32 * lk2[:, perm])
    lv28 = _f8(32 * lv2)
    wq8 = _f8(wq[:, perm])
    woutb = _f8(32.0 * wout)

    # v-path centering: statistical mean of relu(w1v8 . x) per latent,
    # snapped to the fp8 grid so exact relu-zeros quantize exactly
    xr = float(np.sqrt((x.astype(np.float64) ** 2).mean()))
    colv = np.sqrt((w1v8.astype(np.float64) ** 2).sum(0)) * xr
    cv32 = _f8(0.39894228 * colv).astype(np.float64)       # scale-32 units
    v0 = (cv32 / 32.0) @ lv2                               # exact lv2
    bout2 = bout + v0 @ wout
    bout2_t = np.ascontiguousarray(
        bout2.reshape(8, 128).T.astype(np.float32))
    cv_t = np.ascontiguousarray(
        cv32.reshape(4, 128).T.astype(np.float32))

    # exp biases per key slot (per batch)
    kbt = np.full((B, NK), NEGB, np.float32)
    kbt2 = np.full((B, NK), KB2R + A8 * (NEGB + C_SHIFT), np.float32)
    for b in range(B):
        kbt[b, :A[b]] = -C_SHIFT
        kbt2[b, :A[b]] = KB2R
    kbt_t = [np.ascontiguousarray(kbt[b].reshape(NB, 128).T) for b in range(B)]
    kbt2_t = [np.ascontiguousarray(kbt2[b].reshape(NB, 128).T) for b in range(B)]

    key = (NB, FPB)
    if key not in _cache:
        _cache[key] = _build(NB, FPB)
    nc = _cache[key]

    x_flat = x.reshape(B * N, DIM)
    xkv_b = []
    for b in range(B):
        xkv = np.zeros((DIM, NK), NPFP8)
        xkv[:, :A[b]] = _f8(x_flat[b * N + act[b]].T)
        xkv_b.append(xkv)

    in_maps = []
    for c in range(NC):
        b = c // 4
        in_maps.append({
            "xq": np.ascontiguousarray(_f8(x_flat[c * T:(c + 1) * T].T)),
            "xkv": xkv_b[b],
            "wq": wq8, "w1k": w1k8, "w1v": w1v8, "lk2": lk28, "lv2": lv28,
            "wout": woutb, "bout2": bout2_t, "kbt": kbt_t[b],
            "kbt2": kbt2_t[b], "cv": cv_t,
        })

    res = run_bass_kernel_spmd(nc, in_maps, core_ids=list(range(NC)))
    LAST_RESULTS = res
    y = np.empty((B * N, DIM), np.float32)
    for c in range(NC):
        y[c * T:(c + 1) * T] = res.results[c]["yT"].T.astype(np.float32)
    return y.reshape(B, N, DIM)


# revision 93
# speedup vs baseline: 2.2227x; 1.0005x over previous
"""Trainium2 Bass kernel for KeOps multi-head latent attention (v2).

Reference (B=2, N=2048, DIM=1024, LATENT=512, HEADS=16, HD=64):
    q = x @ wq * scale
    k = relu((x @ wkv[:, :D]) @ lk1) @ lk2   (folded: relu(x @ w1k) @ lk2)
    v = relu((x @ wkv[:, D:]) @ lv1) @ lv2
    per head: e = exp(q k^T + maskbias); out = (e @ v) / (e.sum + 1e-6)
    y = out @ wout + bout

Strategy (8 cores, one SPMD NEFF, NO collective):
  - queries: tokens sharded 512/core (cores 0-3 batch0, 4-7 batch1).
  - keys: masked keys compacted on host; EVERY core computes k/v for its
    batch's full active-key set (NB*128 slots) locally — redundant compute
    is far cheaper than the modeled AllGather (15us + 40GB/s).
  - all matmuls fp8e4m3 with DoubleRow (2x modeled PE throughput),
    including the output projection: v-path mean-centering makes att/wout
    fp8 affordable (the mean path rides the exact fp64 output bias).
  - v-path mean-centering: h_v has positive mean (relu); subtracting a
    host-computed statistical mean per latent (fp8-snapped) removes the
    coherent component of the lv2-fp8 quantization error; the mean path
    v0 = c @ lv2 rides through the output bias in fp64 (exact since
    softmax weights sum to 1). K-path coherent errors cancel in softmax.
  - scores per head via DoubleRow on 32-partition quadrants
    (tile_position): head dims split 32+32 across the two DR planes.
  - denominator: 64 'ones' columns interleaved with v give a PE-broadcast
    denominator on psum partitions 64:127 (free), so normalize is one
    reciprocal + one multiply on DVE per head.
  - exp split between ACT (true exp, fp8 out) and DVE (Schraudolph uint8
    bit-trick -> fp8e4m3, bit-exact validated on HW) to balance engines.
"""

import sys

sys.path.insert(0, "/opt/trn_rl_repo")
import numpy as np
import ml_dtypes
import concourse.bass as bass
import concourse.mybir as mybir
import concourse.tile as tile
from concourse import bacc
from concourse.bass_utils import run_bass_kernel_spmd

DIM, LATENT, HEADS, HD = 1024, 512, 16, 64
B, N, NC, T = 2, 2048, 8, 512
SCALE = HD ** -0.5
BF16, F32, FP8 = mybir.dt.bfloat16, mybir.dt.float32, mybir.dt.float8e4
U8 = mybir.dt.uint8
NPBF16 = ml_dtypes.bfloat16
NPFP8 = ml_dtypes.float8_e4m3
DR = mybir.MatmulPerfMode.DoubleRow

LN2 = float(np.log(2.0))
C_SHIFT = 5 * LN2            # exp shift; e^-C folded via bias, 2^-5 exact
NEGB = -35.0                 # pad-kill bias
C8 = 0.0435                  # schraudolph tuning constant
A8 = 8.0 / LN2
A8S = A8 * SCALE / 2.0       # DVE schr multiplier on raw scores
KB2R = 8.0 * (7.0 - C8) - A8 * C_SHIFT   # schr bias, real keys

_cache: dict = {}
LAST_RESULTS = None


def _build(NB, FPB):
    """NB = key blocks of 128 per batch; FPB = first block containing pads
    (blocks < FPB use constant exp bias; blocks >= FPB use per-slot AP)."""
    NK = NB * 128
    Exp = mybir.ActivationFunctionType.Exp
    Relu = mybir.ActivationFunctionType.Relu
    Copy = mybir.ActivationFunctionType.Copy
    Ident = mybir.ActivationFunctionType.Identity
    Alu = mybir.AluOpType

    nc = bacc.Bacc("TRN2", target_bir_lowering=False, num_devices=NC)
    xq_d = nc.dram_tensor("xq", [DIM, T], FP8, kind="ExternalInput")
    xkv_d = nc.dram_tensor("xkv", [DIM, NK], FP8, kind="ExternalInput")
    wq_d = nc.dram_tensor("wq", [DIM, DIM], FP8, kind="ExternalInput")
    w1k_d = nc.dram_tensor("w1k", [DIM, LATENT], FP8, kind="ExternalInput")
    w1v_d = nc.dram_tensor("w1v", [DIM, LATENT], FP8, kind="ExternalInput")
    lk2_d = nc.dram_tensor("lk2", [LATENT, DIM], FP8, kind="ExternalInput")
    lv2_d = nc.dram_tensor("lv2", [LATENT, DIM], FP8, kind="ExternalInput")
    wout_d = nc.dram_tensor("wout", [DIM, DIM], FP8, kind="ExternalInput")
    bout_d = nc.dram_tensor("bout2", [128, 8], F32, kind="ExternalInput")
    kbt_d = nc.dram_tensor("kbt", [128, NB], F32, kind="ExternalInput")
    kbt2_d = nc.dram_tensor("kbt2", [128, NB], F32, kind="ExternalInput")
    cv_d = nc.dram_tensor("cv", [128, 4], F32, kind="ExternalInput")
    y_d = nc.dram_tensor("yT", [DIM, T], BF16, kind="ExternalOutput")

    from contextlib import ExitStack
    with ExitStack() as ctx:
        tc = ctx.enter_context(tile.TileContext(nc))
        pool = lambda **kw: ctx.enter_context(tc.tile_pool(**kw))
        pw1 = pool(name="pw1", bufs=2)
        pl2 = pool(name="pl2", bufs=2)
        pwq = pool(name="pwq", bufs=1)
        pwo = pool(name="pwo", bufs=1)
        px = pool(name="px", bufs=1)
        ph = pool(name="ph", bufs=2)
        pkt = pool(name="pkt", bufs=1)
        pv = pool(name="pv", bufs=1)
        pqt = pool(name="pqt", bufs=1)
        patt = pool(name="patt", bufs=1)
        pe_ = pool(name="pe", bufs=8)
        pr = pool(name="pr", bufs=3)
        posb = pool(name="posb", bufs=5)
        psm = pool(name="psm", bufs=1)
        psA = pool(name="psA", bufs=3, space="PSUM")
        psN = pool(name="psN", bufs=2, space="PSUM")

        # ---------------- input DMAs ------------------------------------
        # split the critical first loads in halves so h matmuls start early
        w1k_sb = pw1.tile([128, 8 * LATENT], FP8, tag="w1")
        w1v_sb = pw1.tile([128, 8 * LATENT], FP8, tag="w1")
        xkv_sb = px.tile([128, 8 * NK], FP8, tag="xkv")
        cvt = psm.tile([128, 4], F32, tag="cv")
        xkv3v = xkv_sb[:].rearrange("p (d n) -> p d n", d=8)
        w1k3v = w1k_sb[:].rearrange("p (d l) -> p d l", d=8)
        w1v3v = w1v_sb[:].rearrange("p (d l) -> p d l", d=8)
        for hf in range(2):
            dd = slice(512 * hf, 512 * (hf + 1))
            nc.sync.dma_start(
                xkv3v[:, 4 * hf:4 * hf + 4],
                xkv_d.ap()[dd, :].rearrange("(d p) n -> p d n", p=128))
            nc.sync.dma_start(
                w1k3v[:, 4 * hf:4 * hf + 4],
                w1k_d.ap()[dd, :].rearrange("(d p) l -> p d l", p=128))
            nc.sync.dma_start(
                w1v3v[:, 4 * hf:4 * hf + 4],
                w1v_d.ap()[dd, :].rearrange("(d p) l -> p d l", p=128))
        nc.sync.dma_start(cvt[:], cv_d.ap())

        lk2_sb = pl2.tile([128, 4 * DIM], FP8, tag="l2")
        lv2_sb = pl2.tile([128, 4 * DIM], FP8, tag="l2")
        nc.sync.dma_start(
            lk2_sb[:].rearrange("p (l c) -> p l c", l=4),
            lk2_d.ap().rearrange("(l p) c -> p l c", p=128))
        nc.sync.dma_start(
            lv2_sb[:].rearrange("p (l c) -> p l c", l=4),
            lv2_d.ap().rearrange("(l p) c -> p l c", p=128))

        wq_sb = pwq.tile([128, 8 * DIM], FP8, tag="wq")
        xq_sb = px.tile([128, 8 * T], FP8, tag="xq")
        nc.sync.dma_start(
            wq_sb[:].rearrange("p (d c) -> p d c", d=8),
            wq_d.ap().rearrange("(d p) c -> p d c", p=128))
        nc.sync.dma_start(
            xq_sb[:].rearrange("p (d n) -> p d n", d=8),
            xq_d.ap().rearrange("(d p) n -> p d n", p=128))

        kbt = psm.tile([128, NB], F32, tag="kbt")
        kbt2 = psm.tile([128, NB], F32, tag="kbt2")
        nc.sync.dma_start(kbt[:], kbt_d.ap())
        nc.sync.dma_start(kbt2[:], kbt2_d.ap())

        wout_sb = pwo.tile([128, 8 * DIM], FP8, tag="wo")
        boutt = psm.tile([128, 8], F32, tag="bo")
        nc.sync.dma_start(
            wout_sb[:].rearrange("p (d c) -> p d c", d=8),
            wout_d.ap().rearrange("(d p) c -> p d c", p=128))
        nc.sync.dma_start(boutt[:], bout_d.ap())

        # 3D chunk-major views
        w1k3 = w1k_sb[:].rearrange("p (d l) -> p d l", d=8)
        w1v3 = w1v_sb[:].rearrange("p (d l) -> p d l", d=8)
        xkv3 = xkv_sb[:].rearrange("p (d n) -> p d n", d=8)
        lk23 = lk2_sb[:].rearrange("p (l c) -> p l c", l=4)
        lv23 = lv2_sb[:].rearrange("p (l c) -> p l c", l=4)
        wq3 = wq_sb[:].rearrange("p (d c) -> p d c", d=8)
        xq3 = xq_sb[:].rearrange("p (d n) -> p d n", d=8)

        hk_sb = ph.tile([128, 4 * NK], FP8, tag="h")
        hv_sb = ph.tile([128, 4 * NK], FP8, tag="h")
        hk3 = hk_sb[:].rearrange("p (l n) -> p l n", l=4)
        hv3 = hv_sb[:].rearrange("p (l n) -> p l n", l=4)
        kt_sb = pkt.tile([128, 8 * NK], FP8, tag="kt")
        v_sb = pv.tile([128, NB * 2048], FP8, tag="v")
        qt_sb = pqt.tile([128, 8 * T], FP8, tag="qt")
        att_sb = patt.tile([128, 8 * T], FP8, tag="att")

        KEY_CH = [(0, 1024)] + ([(1024, NK - 1024)] if NK > 1024 else [])

        # ones columns for the PE-broadcast denominator (Pool, idle engine)
        v4 = v_sb[:].rearrange("p (j h two d) -> p (j h) two d",
                               j=NB, h=HEADS, two=2)
        nc.gpsimd.memset(v4[:, :, 1, :], 1.0)

        def h_path_l(w13, dst3, is_v, l):
            if True:
                ps = psA.tile([128, 1024], F32, tag="big")
                pst = psN.tile([128, 512], F32, tag="nm")
                for g0 in (0, 512):
                    for dp in range(4):
                        nc.tensor.matmul(
                            ps[:, g0:g0 + 512],
                            w13[:, 2 * dp:2 * dp + 2, 128 * l:128 * (l + 1)],
                            xkv3[:, 2 * dp:2 * dp + 2, g0:g0 + 512],
                            start=(dp == 0), stop=(dp == 3), perf_mode=DR)
                if NK > 1024:
                    for dp in range(4):
                        nc.tensor.matmul(
                            pst[:, 0:NK - 1024],
                            w13[:, 2 * dp:2 * dp + 2, 128 * l:128 * (l + 1)],
                            xkv3[:, 2 * dp:2 * dp + 2, 1024:NK],
                            start=(dp == 0), stop=(dp == 3), perf_mode=DR)
                with nc.allow_low_precision(reason="fp8 latents"):
                    if is_v:
                        # (max(ps,0) - cv) on DVE; centering the v-latents
                        nc.vector.tensor_scalar(
                            dst3[:, l, 0:1024], ps[:], 0.0, cvt[:, l:l + 1],
                            Alu.max, Alu.subtract)
                        if NK > 1024:
                            nc.vector.tensor_scalar(
                                dst3[:, l, 1024:NK], pst[:, 0:NK - 1024],
                                0.0, cvt[:, l:l + 1], Alu.max, Alu.subtract)
                    else:
                        nc.scalar.activation(dst3[:, l, 0:1024], ps[:], Relu,
                                             scale=2.0 ** -5)
                        if NK > 1024:
                            nc.scalar.activation(dst3[:, l, 1024:NK],
                                                 pst[:, 0:NK - 1024], Relu,
                                                 scale=2.0 ** -5)

        for l in range(4):
            h_path_l(w1k3, hk3, False, l)
            h_path_l(w1v3, hv3, True, l)

        # ---------------- kT (8 chunks), v (NB blocks), q (4 pairs) ------
        def kt_chunk(c8):
            ps = psA.tile([128, 1024], F32, tag="big")
            pst = psN.tile([128, 512], F32, tag="nm")
            for g0 in (0, 512):
                for lp in range(2):
                    nc.tensor.matmul(
                        ps[:, g0:g0 + 512],
                        lk23[:, 2 * lp:2 * lp + 2, 128 * c8:128 * (c8 + 1)],
                        hk3[:, 2 * lp:2 * lp + 2, g0:g0 + 512],
                        start=(lp == 0), stop=(lp == 1), perf_mode=DR)
            if NK > 1024:
                for lp in range(2):
                    nc.tensor.matmul(
                        pst[:, 0:NK - 1024],
                        lk23[:, 2 * lp:2 * lp + 2, 128 * c8:128 * (c8 + 1)],
                        hk3[:, 2 * lp:2 * lp + 2, 1024:NK],
                        start=(lp == 0), stop=(lp == 1), perf_mode=DR)
            # kT evac on DVE (kv-phase engine balance: ACT has relu+v+q)
            with nc.allow_low_precision(reason="fp8 k"):
                nc.vector.tensor_scalar(
                    kt_sb[:, c8 * NK:c8 * NK + 1024], ps[:],
                    2.0 ** -4, None, Alu.mult)
                if NK > 1024:
                    nc.vector.tensor_scalar(
                        kt_sb[:, c8 * NK + 1024:(c8 + 1) * NK],
                        pst[:, 0:NK - 1024], 2.0 ** -4, None, Alu.mult)

        def v_block(j):
            ps = psA.tile([128, 1024], F32, tag="big")
            for ch in range(2):
                for lp in range(2):
                    nc.tensor.matmul(
                        ps[:, 512 * ch:512 * (ch + 1)],
                        hv3[:, 2 * lp:2 * lp + 2, 128 * j:128 * (j + 1)],
                        lv23[:, 2 * lp:2 * lp + 2, 512 * ch:512 * (ch + 1)],
                        start=(lp == 0), stop=(lp == 1), perf_mode=DR)
            dst = v_sb[:, j * 2048:(j + 1) * 2048] \
                .rearrange("p (h two d) -> p h two d", h=16, two=2)[:, :, 0, :]
            with nc.allow_low_precision(reason="fp8 v"):
                if j % 3 == 2:
                    nc.vector.tensor_scalar(
                        dst, ps[:].rearrange("p (h d) -> p h d", h=16),
                        2.0 ** -8, None, Alu.mult)
                else:
                    nc.scalar.activation(
                        dst, ps[:].rearrange("p (h d) -> p h d", h=16),
                        Copy, scale=2.0 ** -8)

        def q_pair(t):
            ps = psA.tile([128, 1024], F32, tag="big")
            for pl in range(2):
                for dp in range(4):
                    nc.tensor.matmul(
                        ps[:, 512 * pl:512 * (pl + 1)],
                        wq3[:, 2 * dp:2 * dp + 2,
                            (2 * t + pl) * 128:(2 * t + pl + 1) * 128],
                        xq3[:, 2 * dp:2 * dp + 2, :],
                        start=(dp == 0), stop=(dp == 3), perf_mode=DR)
            with nc.allow_low_precision(reason="fp8 q"):
                nc.scalar.activation(qt_sb[:, t * 1024:(t + 1) * 1024], ps[:],
                                     Copy)

        order = [("k", 0), ("q", 0), ("k", 1), ("v", 0), ("q", 1),
                 ("k", 2), ("v", 1), ("k", 3), ("v", 2), ("q", 2),
                 ("k", 4), ("v", 3), ("k", 5), ("v", 4), ("q", 3),
                 ("k", 6), ("v", 5), ("k", 7), ("v", 6), ("v", 7), ("v", 8)]
        order = [(k, i) for (k, i) in order if
                 (k != "v" or i < NB) and (k != "k" or i < 8)]
        for kind, i in order:
            (kt_chunk if kind == "k" else v_block if kind == "v" else q_pair)(i)

        def drain_kv(n):
            pass

        # ---------------- attention -------------------------------------
        # flat software pipeline across all (head, unit) pairs: scores run
        # two units ahead of exp; numer three behind; each head's normalize
        # is deferred until after the NEXT head's DVE exps so the
        # exp->numer->recip round trip never stalls DVE.  exp split: ACT
        # gets pad-free pairs 0,1,2 (const bias); DVE gets pair 3 + the
        # leftover block + reciprocal + normalize multiply.
        NPAIR = NB // 2
        NU = NPAIR + (1 if NB % 2 else 0)
        v3 = v_sb[:].rearrange("p (j x) -> p j x", j=NB)

        scs = {}
        es = {}
        nms = {}
        pending_norm = []

        def head_views(h):
            t, g = h // 4, h % 4
            kt3 = kt_sb[:, t * 2 * NK:(t + 1) * 2 * NK] \
                .rearrange("p (pl k) -> p pl k", pl=2)
            qt3 = qt_sb[:, t * 1024:(t + 1) * 1024] \
                .rearrange("p (pl n) -> p pl n", pl=2)
            return kt3, qt3[32 * g:32 * (g + 1), :, :], 32 * g

        def emit_sc(h, u):
            kt3, lhq, bp32 = head_views(h)
            if u < NPAIR:
                sc = psA.tile([128, 1024], F32, tag="big", name=f"sc{h}_{u}")
            else:
                # leftover unit lives in the psN ring so the psA ring
                # advances by 4 (not 5) per head: the next head's first sc
                # then reuses a slot freed by an early DVE exp, not ACT's
                # last one
                sc = psN.tile([128, 512], F32, tag="nm", name=f"sc{h}_{u}")
            scs[(h, u)] = sc
            nj = 2 if u < NPAIR else 1
            for half in range(nj):
                j = 2 * u + half
                nc.tensor.matmul(
                    sc[:, 512 * half:512 * (half + 1)],
                    kt3[bp32:bp32 + 32, :, 128 * j:128 * (j + 1)],
                    lhq, start=True, stop=True, perf_mode=DR,
                    tile_position=(bp32, 0))

        def emit_exp(h, u):
            sc = scs[(h, u)]
            if u < NPAIR:
                e = pe_.tile([128, 1024], FP8, tag="e")
                with nc.allow_low_precision(reason="fp8 softmax"):
                    if 2 * u + 1 < FPB and u < 3:
                        nc.scalar.activation(e[:], sc[:], Exp,
                                             bias=kbt[:, 0:1],
                                             scale=SCALE / 2)
                    else:
                        for half in range(2):
                            j = 2 * u + half
                            nc.vector.tensor_scalar(
                                e[:, 512 * half:512 * (half + 1)].bitcast(U8),
                                sc[:, 512 * half:512 * (half + 1)], A8S,
                                KB2R if j < FPB else kbt2[:, j:j + 1],
                                Alu.mult, Alu.add)
            else:
                j = 2 * NPAIR
                e = pe_.tile([128, 512], FP8, tag="e1")
                with nc.allow_low_precision(reason="fp8 softmax"):
                    if h % 2 == 1:
                        nc.scalar.activation(e[:], sc[:], Exp,
                                             bias=kbt[:, j:j + 1],
                                             scale=SCALE / 2)
                    else:
                        nc.vector.tensor_scalar(
                            e[:].bitcast(U8), sc[:], A8S,
                            KB2R if j < FPB else kbt2[:, j:j + 1],
                            Alu.mult, Alu.add)
            es[(h, u)] = e
            # emit the deferred normalize after this head's first DVE exp
            if u == 3 and pending_norm:
                pending_norm.pop(0)()

        def emit_nm(h, u):
            if u == 0:
                nms[h] = psN.tile([128, 512], F32, tag="nm", name=f"nm{h}")
            nm = nms[h]
            if u < NPAIR:
                nc.tensor.matmul(
                    nm[:], v3[:, 2 * u:2 * u + 2, 128 * h:128 * (h + 1)],
                    es[(h, u)][:].rearrange("p (two n) -> p two n", two=2),
                    start=(u == 0), stop=(u == NU - 1), perf_mode=DR)
            else:
                nc.tensor.matmul(
                    nm[:], v3[:, 2 * NPAIR, 128 * h:128 * (h + 1)],
                    es[(h, u)][:], start=False, stop=True,
                    skip_group_check=True)
            if u == NU - 1:
                def normalize(h=h, nm=nm):
                    rr = pr.tile([64, 512], F32, tag="r")
                    nc.vector.reciprocal(rr[:], nm[64:128, :])
                    with nc.allow_low_precision(reason="bf16 att"):
                        nc.vector.tensor_mul(
                            att_sb[64 * (h % 2):64 * (h % 2) + 64,
                                   (h // 2) * T:(h // 2 + 1) * T],
                            nm[0:64, :], rr[:])
                pending_norm.append(normalize)

        units = [(h, u) for h in range(HEADS) for u in range(NU)]
        LA_E, LA_N = 1, 5
        for i in range(len(units) + LA_N):
            if i < len(units):
                emit_sc(*units[i])
            if LA_E <= i and i - LA_E < len(units):
                emit_exp(*units[i - LA_E])
            drain_kv(1)
            if LA_N <= i and i - LA_N < len(units):
                emit_nm(*units[i - LA_N])
        while pending_norm:
            pending_norm.pop(0)()

        # ---------------- output projection ------------------------------
        wo3 = wout_sb[:].rearrange("p (d c) -> p d c", d=8)
        att3 = att_sb[:].rearrange("p (c n) -> p c n", c=8)
        for cb in range(8):
            ps = psA.tile([128, 1024], F32, tag="big")
            for c2 in range(4):
                nc.tensor.matmul(
                    ps[:, 0:512],
                    wo3[:, 2 * c2:2 * c2 + 2, 128 * cb:128 * (cb + 1)],
                    att3[:, 2 * c2:2 * c2 + 2, :],
                    start=(c2 == 0), stop=(c2 == 3), perf_mode=DR)
            osb = posb.tile([128, 512], BF16, tag="osb")
            with nc.allow_low_precision(reason="bf16 output"):
                nc.vector.tensor_scalar(osb[:], ps[:, 0:512],
                                        2.0 ** -7, boutt[:, cb:cb + 1],
                                        Alu.mult, Alu.add)
            nc.sync.dma_start(y_d.ap()[128 * cb:128 * (cb + 1), :], osb[:])

    nc.compile()
    return nc


def _f8(x):
    return np.asarray(x, np.float32).astype(NPFP8)


def kernel(x, mask, wq, wkv, lk1, lk2, lv1, lv2, wout, bout, **kw):
    global LAST_RESULTS
    x = np.asarray(x, np.float32)
    mask = np.asarray(mask)
    wq = np.asarray(wq, np.float64)
    wkv = np.asarray(wkv, np.float64)
    lk1 = np.asarray(lk1, np.float64)
    lk2 = np.asarray(lk2, np.float64)
    lv1 = np.asarray(lv1, np.float64)
    lv2 = np.asarray(lv2, np.float64)
    wout = np.asarray(wout, np.float64)
    bout = np.asarray(bout, np.float64)

    act = [np.nonzero(np.asarray(mask[b]) == 1)[0] for b in range(B)]
    A = [len(a) for a in act]
    NB = max(1, (max(A) + 127) // 128)
    NK = NB * 128
    FPB = min(A) // 128          # first block that contains pad slots

    # column permutation for the DR-32 scores layout:
    # psum chunk (t,pl) partitions = [head 4t+g, dims 32pl..32pl+32]
    perm = np.array([64 * (4 * t + g) + 32 * pl + i
                     for t in range(4) for pl in range(2)
                     for g in range(4) for i in range(32)])

    w1k = wkv[:, :DIM] @ lk1
    w1v = wkv[:, DIM:] @ lv1
    w1k8 = _f8(32 * w1k)
    w1v8 = _f8(32 * w1v)
    lk28 = _f8(32 * lk2[:, perm])
    lv28 = _f8(32 * lv2)
    wq8 = _f8(wq[:, perm])
    woutb = _f8(32.0 * wout)

    # v-path centering: statistical mean of relu(w1v8 . x) per latent,
    # snapped to the fp8 grid so exact relu-zeros quantize exactly
    xr = float(np.sqrt((x.astype(np.float64) ** 2).mean()))
    colv = np.sqrt((w1v8.astype(np.float64) ** 2).sum(0)) * xr
    cv32 = _f8(0.39894228 * colv).astype(np.float64)       # scale-32 units
    v0 = (cv32 / 32.0) @ lv2                               # exact lv2
    bout2 = bout + v0 @ wout
    bout2_t = np.ascontiguousarray(
        bout2.reshape(8, 128).T.astype(np.float32))
    cv_t = np.ascontiguousarray(
        cv32.reshape(4, 128).T.astype(np.float32))

    # exp biases per key slot (per batch)
    kbt = np.full((B, NK), NEGB, np.float32)
    kbt2 = np.full((B, NK), KB2R + A8 * (NEGB + C_SHIFT), np.float32)
    for b in range(B):
        kbt[b, :A[b]] = -C_SHIFT
        kbt2[b, :A[b]] = KB2R
    kbt_t = [np.ascontiguousarray(kbt[b].reshape(NB, 128).T) for b in range(B)]
    kbt2_t = [np.ascontiguousarray(kbt2[b].reshape(NB, 128).T) for b in range(B)]

    key = (NB, FPB)
    if key not in _cache:
        _cache[key] = _build(NB, FPB)
    nc = _cache[key]

    x_flat = x.reshape(B * N, DIM)
    xkv_b = []
    for b in range(B):
        xkv = np.zeros((DIM, NK), NPFP8)
        xkv[:, :A[b]] = _f8(x_flat[b * N + act[b]].T)
        xkv_b.append(xkv)

    in_maps = []
    for c in range(NC):
        b = c // 4
        in_maps.append({
            "xq": np.ascontiguousarray(_f8(x_flat[c * T:(c + 1) * T].T)),
            "xkv": xkv_b[b],
            "wq": wq8, "w1k": w1k8, "w1v": w1v8, "lk2": lk28, "lv2": lv28,
            "wout": woutb, "bout2": bout2_t, "kbt": kbt_t[b],
            "kbt2": kbt2_t[b], "cv": cv_t,
        })

    res = run_bass_kernel_spmd(nc, in_maps, core_ids=list(range(NC)))
    LAST_RESULTS = res
    y = np.empty((B * N, DIM), np.float32)
    for c in range(NC):
        y[c * T:(c + 1) * T] = res.results[c]["yT"].T.astype(np.float32)
    return y.reshape(B, N, DIM)
